# revision 5
# baseline (speedup 1.0000x reference)
"""GCN encoder (2-layer GCNConv, PyG-style) on 8 Trainium2 NeuronCores.

Sharding: nodes row-sharded 6250/core; edges partitioned by destination-node
owner; per-core segment-sum over 128-dst-slot windows via selection-matrix
matmuls.

v2: aggregate-first layer 1. Since segment_sum commutes with @W1, the layer-1
gather table is just dinv.*x (bf16, prepared on host, uploaded as input) —
no on-device table build, no replicated GEMM, and gathers start at t=0.
Per window, after the raw-feature aggregation:
  A[dst,256]  = sum_e xd[src_e] + xd[dst]          (S-matmuls + identity matmul)
  g~          = dinv^2 .* relu(A @ W1)             (transpose, GEMM, relu-scale)
  table2 rows = g~ @ W2                            (transpose, GEMM)
Layer 2 stays transform-first (OUTC < HID): table2 is all-gathered, split
into two collectives (sub-tables A/B) that overlap with remaining L1 work.

norm = dinv[src]*dinv[dst] folding (b1 == b2 == 0):
  xd    = dinv .* x
  g~    = dinv^2 .* relu(segsum(xd[src]) @ W1) = dinv .* h
  out   = dinv .* segsum((g~ @ W2)[src])

Self-loop messages never go through the gather path: their contribution to a
window's segment-sum is the core's own xd / table2 rows, added with one
identity matmul per window from SBUF-resident copies.

Sub-tables (for int16 gather indices and collective splitting): local row
l < 3200 (windows 0-24) -> sub A (8*3200 = 25600 rows); l >= 3200
(windows 25-48) -> sub B (8*3072 = 24576 rows). Both < 2**15.
"""

import os
import numpy as np
import ml_dtypes

import concourse.bacc as bacc
import concourse.tile as tile
from concourse import bass, mybir
from concourse.bass_utils import run_bass_kernel_spmd
from concourse.library_config import mlp

N = 50000
INC, HID, OUTC = 256, 256, 128
NCORES = 8
RPC = N // NCORES            # 6250 rows per core
WPC = (RPC + 127) // 128     # 49 windows per core
RPAD = WPC * 128             # 6272
LSPL = 3200                  # sub-table split on local row (windows 0..24 | 25..48)
NA = NCORES * LSPL           # 25600 rows in sub-table A
NB = NCORES * (RPAD - LSPL)  # 24576 rows in sub-table B
WA = LSPL // 128             # 25 windows in A
GRP = 2                      # windows per supergather group
NGRP = (WPC + GRP - 1) // GRP
# L1 processes B-side groups first so AG2(B) can launch early.
NGA = WA // GRP
GORDER = list(range(NGA, NGRP)) + list(range(0, NGA))

SINGLE_PACKET = bool(int(os.environ.get("GCN_SINGLE_PACKET", "0")))


def _preprocess(edge_index):
    """Edge partitioning / ordering and normalization constants (host, index-only)."""
    src = np.asarray(edge_index[0], np.int64)
    dst = np.asarray(edge_index[1], np.int64)

    # degrees include the self-loops the reference adds
    deg = (np.bincount(dst, minlength=N) + 1).astype(np.float64)
    dinv = (1.0 / np.sqrt(deg)).astype(np.float32)

    owner = dst // RPC
    dstl = dst - owner * RPC
    win = dstl >> 7
    slot = dstl & 127
    srho = src // RPC
    srl = src - srho * RPC
    sub = (srl >= LSPL).astype(np.int64)
    gl = np.where(sub == 0, srho * LSPL + srl,
                  srho * (RPAD - LSPL) + (srl - LSPL)).astype(np.int32)

    key = (owner * WPC + win) * 2 + sub
    order = np.argsort(key, kind="stable")
    key_s = key[order]
    gl_s = gl[order]
    slot_s = slot[order].astype(np.int32)

    nbuck = NCORES * WPC * 2
    counts = np.bincount(key_s, minlength=nbuck).reshape(NCORES, WPC, 2)
    starts_flat = np.concatenate([[0], np.cumsum(counts.reshape(-1))])

    # tiles per (window, sub): max over cores so one SPMD program fits all
    Twh = (counts.max(axis=0) + 127) // 128     # [WPC, 2]
    TT = int(Twh.sum())
    # stream order: group -> sub -> window in group -> tiles
    base = np.zeros((WPC, 2), np.int64)
    pos = 0
    for gi in range(NGRP):
        ws = range(gi * GRP, min((gi + 1) * GRP, WPC))
        for h in range(2):
            for w in ws:
                base[w, h] = pos
                pos += Twh[w, h]
    assert pos == TT

    idx_seq = np.zeros((NCORES, TT * 128), np.int32)
    slot_seq = np.full((NCORES, TT * 128), 128, np.int32)  # 128 = dropped sentinel
    for c in range(NCORES):
        for w in range(WPC):
            for h in range(2):
                n = counts[c, w, h]
                if n == 0:
                    continue
                s0 = starts_flat[(c * WPC + w) * 2 + h]
                p0 = base[w, h] * 128
                idx_seq[c, p0 : p0 + n] = gl_s[s0 : s0 + n]
                slot_seq[c, p0 : p0 + n] = slot_s[s0 : s0 + n]

    # wrapped int16 gather-index layout: element j at [j%16, j//16], replicated x8
    idx16 = np.empty((NCORES, 128, TT * 8), np.int16)
    slots = np.empty((NCORES, 128, TT), np.float32)
    for c in range(NCORES):
        a = idx_seq[c].astype(np.int16).reshape(-1, 16).T
        idx16[c] = np.tile(a, (8, 1))
        slots[c] = slot_seq[c].astype(np.float32).reshape(TT, 128).T

    # per-core per-window dinv columns for own rows
    dcol1 = np.zeros((NCORES, 128, WPC), np.float32)
    for c in range(NCORES):
        d = np.zeros(RPAD, np.float32)
        d[:RPC] = dinv[c * RPC : (c + 1) * RPC]
        dcol1[c] = d.reshape(WPC, 128).T
    dcol2 = dcol1 * dcol1

    return idx16, slots, Twh, base, TT, dcol1, dcol2, dinv


def _xd_tables(x, dinv):
    """dinv.*x rows in [A | B] rank-major padded order, bf16; plus per-core
    own-row blocks in partition-major [128, WPC*256] layout."""
    xd = (x * dinv[:, None]).astype(np.float32)
    xda = np.zeros((NA, INC), np.float32)
    xdb = np.zeros((NB, INC), np.float32)
    nb = RPAD - LSPL
    for rho in range(NCORES):
        xs = xd[rho * RPC : (rho + 1) * RPC]         # [6250, 256]
        xda[rho * LSPL : (rho + 1) * LSPL] = xs[:LSPL]
        xdb[rho * nb : rho * nb + (RPC - LSPL)] = xs[LSPL:]
    ownx = np.zeros((NCORES, 128, WPC, INC), np.float32)
    for c in range(NCORES):
        blk = np.zeros((RPAD, INC), np.float32)
        blk[:RPC] = xd[c * RPC : (c + 1) * RPC]
        ownx[c] = blk.reshape(WPC, 128, INC).transpose(1, 0, 2)
    return (xda.astype(ml_dtypes.bfloat16), xdb.astype(ml_dtypes.bfloat16),
            ownx.reshape(NCORES, 128, WPC * INC).astype(ml_dtypes.bfloat16))


def _build(TT, Twh, base):
    nc = bacc.Bacc("TRN2", num_devices=NCORES, num_swdge_queues=4)
    f32 = mybir.dt.float32
    bf = mybir.dt.bfloat16

    xda_d = nc.dram_tensor("xda", [NA, INC], bf, kind="ExternalInput")
    xdb_d = nc.dram_tensor("xdb", [NB, INC], bf, kind="ExternalInput")
    ownx_d = nc.dram_tensor("ownx", [128, WPC * INC], bf, kind="ExternalInput")
    w1_d = nc.dram_tensor("w1", [2, 128, HID], bf, kind="ExternalInput")
    w2_d = nc.dram_tensor("w2", [2, 128, OUTC], bf, kind="ExternalInput")
    iota_d = nc.dram_tensor("iota", [128, 128], bf, kind="ExternalInput")
    ident_d = nc.dram_tensor("ident", [128, 128], bf, kind="ExternalInput")
    dc1_d = nc.dram_tensor("dcol1", [128, WPC], f32, kind="ExternalInput")
    dc2_d = nc.dram_tensor("dcol2", [128, WPC], f32, kind="ExternalInput")
    idx_d = nc.dram_tensor("idx", [128, TT * 8], mybir.dt.int16, kind="ExternalInput")
    slots_d = nc.dram_tensor("slots", [128, TT], bf, kind="ExternalInput")
    out_d = nc.dram_tensor("out", [RPAD, OUTC], f32, kind="ExternalOutput")

    # tiles per supergather (group, sub)
    Tg = np.zeros((NGRP, 2), np.int64)
    for gi in range(NGRP):
        ws = range(gi * GRP, min((gi + 1) * GRP, WPC))
        for h in range(2):
            Tg[gi, h] = sum(int(Twh[w, h]) for w in ws)

    with tile.TileContext(nc) as tc:
        nc.gpsimd.load_library(mlp)
        with (
            tc.tile_pool(name="const", bufs=1) as cpool,
            tc.tile_pool(name="own", bufs=1) as opool,
            tc.tile_pool(name="evac", bufs=4) as epool,
            tc.tile_pool(name="att", bufs=4) as apool,
            tc.tile_pool(name="msg", bufs=8) as mpool,
            tc.tile_pool(name="sel", bufs=6) as spool,
            tc.tile_pool(name="part", bufs=WPC) as ppool,
            tc.tile_pool(name="p256", bufs=4, space="PSUM") as p256,
            tc.tile_pool(name="p128", bufs=3, space="PSUM") as p128,
            tc.tile_pool(name="ptr", bufs=1, space="PSUM") as ptr,
            tc.tile_pool(name="dram", bufs=1, space="DRAM") as dram,
        ):
            # ---- constants to SBUF
            w1_s = cpool.tile([128, 2, HID], bf)
            w2_s = cpool.tile([128, 2, OUTC], bf)
            iota_s = cpool.tile([128, 128], bf)
            ident_s = cpool.tile([128, 128], bf)
            dc1_s = cpool.tile([128, WPC], f32)
            dc2_s = cpool.tile([128, WPC], f32)
            idx_s = cpool.tile([128, TT * 8], mybir.dt.int16)
            slots_s = cpool.tile([128, TT], bf)
            ownx_s = opool.tile([128, WPC, INC], bf)    # own xd rows per window
            own2_s = opool.tile([128, WPC, OUTC], bf)   # own table2 rows
            for k in range(2):
                nc.sync.dma_start(w1_s[:, k, :], w1_d[k])
                nc.sync.dma_start(w2_s[:, k, :], w2_d[k])
            nc.sync.dma_start(iota_s[:], iota_d[:])
            nc.sync.dma_start(ident_s[:], ident_d[:])
            nc.sync.dma_start(dc1_s[:], dc1_d[:])
            nc.sync.dma_start(dc2_s[:], dc2_d[:])
            nc.sync.dma_start(idx_s[:], idx_d[:])
            nc.sync.dma_start(slots_s[:], slots_d[:])
            nc.scalar.dma_start(
                ownx_s[:], ownx_d[:].rearrange("p (w c) -> p w c", w=WPC))

            ag2a_in = dram.tile([LSPL, OUTC], bf)
            ag2b_in = dram.tile([RPAD - LSPL, OUTC], bf)
            tb2a = dram.tile([NA, OUTC], bf)
            tb2b = dram.tile([NB, OUTC], bf)

            # ---- edge aggregation unit: gathers + S build for one (group, sub)
            def gather_unit(gi, h, tbl, width, qctr):
                T = int(Tg[gi, h])
                if T == 0:
                    return None, None
                ws = list(range(gi * GRP, min((gi + 1) * GRP, WPC)))
                b = int(base[ws[0], h])
                m_s = mpool.tile([128, T, width], bf, tag="msg")
                nc.gpsimd.dma_gather(
                    m_s[:], tbl[:, :], idx_s[:, b * 8 : (b + T) * 8],
                    T * 128, T * 128, width,
                    single_packet=SINGLE_PACKET, queue_num=qctr[0] % 4)
                qctr[0] += 1
                S_s = spool.tile([128, T, 128], bf, tag="sel")
                nc.vector.tensor_tensor(
                    out=S_s[:],
                    in0=slots_s[:, b : b + T, None].to_broadcast([128, T, 128]),
                    in1=iota_s[:, None, :].to_broadcast([128, T, 128]),
                    op=mybir.AluOpType.is_equal)
                return m_s, S_s

            def win_mms(w, h, ps, m_s, S_s, first, last):
                gw0 = (w // GRP) * GRP
                b = int(base[gw0, h])
                n = int(Twh[w, h])
                for t in range(n):
                    tt = int(base[w, h]) - b + t
                    nc.tensor.matmul(ps[:], lhsT=S_s[:, tt, :], rhs=m_s[:, tt, :],
                                     start=(first and t == 0),
                                     stop=(last and t == n - 1))

            qctr = [0]

            # ---- layer-1 aggregation (aggregate-first; B-side groups first)
            def l1_group(gi):
                ws = list(range(gi * GRP, min((gi + 1) * GRP, WPC)))
                units = {}
                for h in range(2):
                    units[h] = gather_unit(gi, h, xda_d if h == 0 else xdb_d,
                                           INC, qctr)
                pss = {}
                for w in ws:
                    ps = p256.tile([128, INC], f32, tag="p256")
                    pss[w] = ps
                    started = False
                    for h in range(2):
                        m_s, S_s = units[h]
                        if m_s is None or Twh[w, h] == 0:
                            continue
                        win_mms(w, h, ps, m_s, S_s, not started, False)
                        started = True
                    # self-loop contribution: own xd rows
                    nc.tensor.matmul(ps[:], lhsT=ident_s[:], rhs=ownx_s[:, w, :],
                                     start=not started, stop=True)
                for w in ws:
                    ps = pss[w]
                    # raw aggregate -> bf16 -> transpose -> @W1 -> relu*dinv^2
                    a_s = epool.tile([128, INC], bf, tag="a")
                    nc.scalar.activation(a_s[:], ps[:],
                                         mybir.ActivationFunctionType.Copy)
                    at_s = apool.tile([128, 2, 128], bf, tag="at")
                    for k in range(2):
                        pt = ptr.tile([128, 128], bf, tag="pt")
                        nc.tensor.transpose(pt[:], a_s[:, k * 128 : (k + 1) * 128],
                                            ident_s[:])
                        nc.vector.tensor_copy(at_s[:, k, :], pt[:])
                    psH = p256.tile([128, HID], f32, tag="p256")
                    for k in range(2):
                        nc.tensor.matmul(psH[:], lhsT=at_s[:, k, :],
                                         rhs=w1_s[:, k, :],
                                         start=(k == 0), stop=(k == 1))
                    g_s = epool.tile([128, HID], bf, tag="g")
                    nc.scalar.activation(g_s[:], psH[:],
                                         mybir.ActivationFunctionType.Relu,
                                         scale=dc2_s[:, w : w + 1])
                    # g~ -> transpose -> @W2 -> own table2 rows
                    gt_s = apool.tile([128, 2, 128], bf, tag="at")
                    for k in range(2):
                        pt = ptr.tile([128, 128], bf, tag="pt")
                        nc.tensor.transpose(pt[:], g_s[:, k * 128 : (k + 1) * 128],
                                            ident_s[:])
                        nc.vector.tensor_copy(gt_s[:, k, :], pt[:])
                    ps2 = p128.tile([128, OUTC], f32, tag="p128")
                    for k in range(2):
                        nc.tensor.matmul(ps2[:], lhsT=gt_s[:, k, :],
                                         rhs=w2_s[:, k, :],
                                         start=(k == 0), stop=(k == 1))
                    nc.vector.tensor_copy(own2_s[:, w, :], ps2[:])
                    if w < WA:
                        nc.sync.dma_start(ag2a_in[w * 128 : (w + 1) * 128, :],
                                          own2_s[:, w, :])
                    else:
                        nc.sync.dma_start(ag2b_in[(w - WA) * 128 : (w - WA + 1) * 128, :],
                                          own2_s[:, w, :])

            # ---- layer-2 stage-1 unit: self + sub-B messages -> partial
            partials = {}

            def p6b_unit(gi):
                ws = list(range(gi * GRP, min((gi + 1) * GRP, WPC)))
                m_s, S_s = gather_unit(gi, 1, tb2b, OUTC, qctr)
                for w in ws:
                    ps = p128.tile([128, OUTC], f32, tag="p128")
                    started = False
                    if m_s is not None and Twh[w, 1] > 0:
                        win_mms(w, 1, ps, m_s, S_s, True, False)
                        started = True
                    nc.tensor.matmul(ps[:], lhsT=ident_s[:], rhs=own2_s[:, w, :],
                                     start=not started, stop=True)
                    pp = ppool.tile([128, OUTC], bf, tag="partial")
                    nc.scalar.activation(pp[:], ps[:],
                                         mybir.ActivationFunctionType.Copy)
                    partials[w] = pp

            with nc.named_scope("p3_l1b"):
                for gi in GORDER[: NGRP - NGA]:
                    l1_group(gi)
            # AG2 for sub-table B launches while L1 still works on A-side groups
            with nc.named_scope("ag2b"):
                nc.gpsimd.collective_compute(
                    "AllGather", mybir.AluOpType.bypass,
                    replica_groups=[list(range(NCORES))],
                    ins=[ag2b_in.opt()], outs=[tb2b.opt()])
            # A-side L1 groups interleaved with p6b units (which only need
            # tb2b + own2 rows of already-finished groups)
            pending = list(GORDER[: NGRP - NGA])
            with nc.named_scope("p3_l1a_mix"):
                for j, gi in enumerate(GORDER[NGRP - NGA :]):
                    l1_group(gi)
                    pending.append(gi)
                    if j >= 2:
                        for _ in range(2):
                            if pending:
                                p6b_unit(pending.pop(0))
                for gi in pending:
                    p6b_unit(gi)
            with nc.named_scope("ag2a"):
                nc.gpsimd.collective_compute(
                    "AllGather", mybir.AluOpType.bypass,
                    replica_groups=[list(range(NCORES))],
                    ins=[ag2a_in.opt()], outs=[tb2a.opt()])

            with nc.named_scope("p6_a"):
                # stage 2: partial + sub-A messages -> output (accumulated on PE)
                for gi in range(NGRP):
                    ws = list(range(gi * GRP, min((gi + 1) * GRP, WPC)))
                    m_s, S_s = gather_unit(gi, 0, tb2a, OUTC, qctr)
                    for w in ws:
                        has_msg = m_s is not None and Twh[w, 0] > 0
                        ps = p128.tile([128, OUTC], f32, tag="p128")
                        nc.tensor.matmul(ps[:], lhsT=ident_s[:], rhs=partials[w][:],
                                         start=True, stop=not has_msg)
                        if has_msg:
                            win_mms(w, 0, ps, m_s, S_s, False, True)
                        o_s = epool.tile([128, OUTC], f32, tag="o")
                        nc.scalar.activation(o_s[:], ps[:],
                                             mybir.ActivationFunctionType.Copy,
                                             scale=dc1_s[:, w : w + 1])
                        nc.sync.dma_start(out_d[w * 128 : (w + 1) * 128, :], o_s[:])

    nc.compile()
    return nc


def kernel(x, edge_index, W1, b1, W2, b2):
    x = np.asarray(x, np.float32)
    W1 = np.asarray(W1, np.float32)
    W2 = np.asarray(W2, np.float32)
    assert not np.any(np.asarray(b1)) and not np.any(np.asarray(b2)), \
        "kernel assumes zero biases (as in the reference setup)"

    idx16, slots, Twh, base, TT, dcol1, dcol2, dinv = _preprocess(np.asarray(edge_index))
    nc = _build(TT, Twh, base)

    iota = np.broadcast_to(np.arange(128, dtype=np.float32), (128, 128)).astype(ml_dtypes.bfloat16)
    ident = np.eye(128, dtype=np.float32).astype(ml_dtypes.bfloat16)
    w1_in = np.ascontiguousarray(W1.reshape(2, 128, HID)).astype(ml_dtypes.bfloat16)
    w2_in = np.ascontiguousarray(W2.reshape(2, 128, OUTC)).astype(ml_dtypes.bfloat16)
    xda, xdb, ownx = _xd_tables(x, dinv)
    slots_bf = slots.astype(ml_dtypes.bfloat16)

    in_maps = []
    for c in range(NCORES):
        in_maps.append({
            "xda": xda, "xdb": xdb, "ownx": ownx[c],
            "w1": w1_in, "w2": w2_in, "iota": iota, "ident": ident,
            "dcol1": dcol1[c], "dcol2": dcol2[c],
            "idx": idx16[c], "slots": slots_bf[c],
        })

    trace = bool(int(os.environ.get("GCN_KERNEL_TRACE", "0")))
    try:
        res = run_bass_kernel_spmd(nc, in_maps, core_ids=list(range(NCORES)), trace=trace)
    except Exception:
        # rare transient NRT exec failure: retry once on a fresh dispatch
        time_mod = __import__("time"); time_mod.sleep(2.0)
        res = run_bass_kernel_spmd(nc, in_maps, core_ids=list(range(NCORES)), trace=False)
    kernel.last_results = res
    if trace:
        print(f"HW exec time: {res.exec_time_ns} ns")
        kernel.last_exec_time_ns = res.exec_time_ns

    out = np.concatenate([res.results[c]["out"][:RPC] for c in range(NCORES)], axis=0)
    return out.astype(np.float32)


# revision 8
# speedup vs baseline: 1.0135x; 1.0135x over previous
"""GCN encoder (2-layer GCNConv, PyG-style) on 8 Trainium2 NeuronCores.

Sharding: nodes row-sharded 6250/core; edges partitioned by destination-node
owner; per-core segment-sum over 128-dst-slot windows via selection-matrix
matmuls.

v2: aggregate-first layer 1. Since segment_sum commutes with @W1, the layer-1
gather table is just dinv.*x (bf16, prepared on host, uploaded as input) —
no on-device table build, no replicated GEMM, and gathers start at t=0.
Per window, after the raw-feature aggregation:
  A[dst,256]  = sum_e xd[src_e] + xd[dst]          (S-matmuls + identity matmul)
  g~          = dinv^2 .* relu(A @ W1)             (transpose, GEMM, relu-scale)
  table2 rows = g~ @ W2                            (transpose, GEMM)
Layer 2 stays transform-first (OUTC < HID): table2 is all-gathered, split
into two collectives (sub-tables A/B) that overlap with remaining L1 work.

norm = dinv[src]*dinv[dst] folding (b1 == b2 == 0):
  xd    = dinv .* x
  g~    = dinv^2 .* relu(segsum(xd[src]) @ W1) = dinv .* h
  out   = dinv .* segsum((g~ @ W2)[src])

Self-loop messages never go through the gather path: their contribution to a
window's segment-sum is the core's own xd / table2 rows, added with one
identity matmul per window from SBUF-resident copies.

Sub-tables (for int16 gather indices and collective splitting): local row
l < 3200 (windows 0-24) -> sub A (8*3200 = 25600 rows); l >= 3200
(windows 25-48) -> sub B (8*3072 = 24576 rows). Both < 2**15.
"""

import os
import numpy as np
import ml_dtypes

import concourse.bacc as bacc
import concourse.tile as tile
from concourse import bass, mybir
from concourse.bass_utils import run_bass_kernel_spmd
from concourse.library_config import mlp

N = 50000
INC, HID, OUTC = 256, 256, 128
NCORES = 8
RPC = N // NCORES            # 6250 rows per core
WPC = (RPC + 127) // 128     # 49 windows per core
RPAD = WPC * 128             # 6272
LSPL = 3200                  # sub-table split on local row (windows 0..24 | 25..48)
NA = NCORES * LSPL           # 25600 rows in sub-table A
NB = NCORES * (RPAD - LSPL)  # 24576 rows in sub-table B
WA = LSPL // 128             # 25 windows in A
GRP = 2                      # windows per supergather group
NGRP = (WPC + GRP - 1) // GRP
# L1 processes B-side groups first so AG2(B) can launch early.
NGA = WA // GRP
GORDER = list(range(NGA, NGRP)) + list(range(0, NGA))

SINGLE_PACKET = bool(int(os.environ.get("GCN_SINGLE_PACKET", "0")))


def _preprocess(edge_index):
    """Edge partitioning / ordering and normalization constants (host, index-only)."""
    src = np.asarray(edge_index[0], np.int64)
    dst = np.asarray(edge_index[1], np.int64)

    # degrees include the self-loops the reference adds
    deg = (np.bincount(dst, minlength=N) + 1).astype(np.float64)
    dinv = (1.0 / np.sqrt(deg)).astype(np.float32)

    owner = dst // RPC
    dstl = dst - owner * RPC
    win = dstl >> 7
    slot = dstl & 127
    srho = src // RPC
    srl = src - srho * RPC
    sub = (srl >= LSPL).astype(np.int64)
    gl = np.where(sub == 0, srho * LSPL + srl,
                  srho * (RPAD - LSPL) + (srl - LSPL)).astype(np.int32)

    # sort by (bucket, src row): ascending addresses within each bucket make
    # the gather's HBM access pattern row-buffer friendly
    order = np.lexsort((gl, (owner * WPC + win) * 2 + sub))
    key_s = ((owner * WPC + win) * 2 + sub)[order]
    gl_s = gl[order]
    slot_s = slot[order].astype(np.int32)

    nbuck = NCORES * WPC * 2
    counts = np.bincount(key_s, minlength=nbuck).reshape(NCORES, WPC, 2)
    starts_flat = np.concatenate([[0], np.cumsum(counts.reshape(-1))])

    # tiles per (window, sub): max over cores so one SPMD program fits all
    Twh = (counts.max(axis=0) + 127) // 128     # [WPC, 2]
    TT = int(Twh.sum())
    # stream order: group -> sub -> window in group -> tiles
    base = np.zeros((WPC, 2), np.int64)
    pos = 0
    for gi in range(NGRP):
        ws = range(gi * GRP, min((gi + 1) * GRP, WPC))
        for h in range(2):
            for w in ws:
                base[w, h] = pos
                pos += Twh[w, h]
    assert pos == TT

    idx_seq = np.zeros((NCORES, TT * 128), np.int32)
    slot_seq = np.full((NCORES, TT * 128), 128, np.int32)  # 128 = dropped sentinel
    for c in range(NCORES):
        for w in range(WPC):
            for h in range(2):
                n = counts[c, w, h]
                if n == 0:
                    continue
                s0 = starts_flat[(c * WPC + w) * 2 + h]
                p0 = base[w, h] * 128
                idx_seq[c, p0 : p0 + n] = gl_s[s0 : s0 + n]
                slot_seq[c, p0 : p0 + n] = slot_s[s0 : s0 + n]

    # wrapped int16 gather-index layout: element j at [j%16, j//16], replicated x8
    idx16 = np.empty((NCORES, 128, TT * 8), np.int16)
    slots = np.empty((NCORES, 128, TT), np.float32)
    for c in range(NCORES):
        a = idx_seq[c].astype(np.int16).reshape(-1, 16).T
        idx16[c] = np.tile(a, (8, 1))
        slots[c] = slot_seq[c].astype(np.float32).reshape(TT, 128).T

    # per-core per-window dinv columns for own rows
    dcol1 = np.zeros((NCORES, 128, WPC), np.float32)
    for c in range(NCORES):
        d = np.zeros(RPAD, np.float32)
        d[:RPC] = dinv[c * RPC : (c + 1) * RPC]
        dcol1[c] = d.reshape(WPC, 128).T
    dcol2 = dcol1 * dcol1

    return idx16, slots, Twh, base, TT, dcol1, dcol2, dinv


def _xd_tables(x, dinv):
    """dinv.*x rows in [A | B] rank-major padded order, bf16; plus per-core
    own-row blocks in partition-major [128, WPC*256] layout."""
    xd = (x * dinv[:, None]).astype(np.float32)
    xda = np.zeros((NA, INC), np.float32)
    xdb = np.zeros((NB, INC), np.float32)
    nb = RPAD - LSPL
    for rho in range(NCORES):
        xs = xd[rho * RPC : (rho + 1) * RPC]         # [6250, 256]
        xda[rho * LSPL : (rho + 1) * LSPL] = xs[:LSPL]
        xdb[rho * nb : rho * nb + (RPC - LSPL)] = xs[LSPL:]
    ownx = np.zeros((NCORES, 128, WPC, INC), np.float32)
    for c in range(NCORES):
        blk = np.zeros((RPAD, INC), np.float32)
        blk[:RPC] = xd[c * RPC : (c + 1) * RPC]
        ownx[c] = blk.reshape(WPC, 128, INC).transpose(1, 0, 2)
    return (xda.astype(ml_dtypes.bfloat16), xdb.astype(ml_dtypes.bfloat16),
            ownx.reshape(NCORES, 128, WPC * INC).astype(ml_dtypes.bfloat16))


def _build(TT, Twh, base):
    nc = bacc.Bacc("TRN2", num_devices=NCORES, num_swdge_queues=4)
    f32 = mybir.dt.float32
    bf = mybir.dt.bfloat16

    xda_d = nc.dram_tensor("xda", [NA, INC], bf, kind="ExternalInput")
    xdb_d = nc.dram_tensor("xdb", [NB, INC], bf, kind="ExternalInput")
    ownx_d = nc.dram_tensor("ownx", [128, WPC * INC], bf, kind="ExternalInput")
    w1_d = nc.dram_tensor("w1", [2, 128, HID], bf, kind="ExternalInput")
    w2_d = nc.dram_tensor("w2", [2, 128, OUTC], bf, kind="ExternalInput")
    iota_d = nc.dram_tensor("iota", [128, 128], bf, kind="ExternalInput")
    ident_d = nc.dram_tensor("ident", [128, 128], bf, kind="ExternalInput")
    dc1_d = nc.dram_tensor("dcol1", [128, WPC], f32, kind="ExternalInput")
    dc2_d = nc.dram_tensor("dcol2", [128, WPC], f32, kind="ExternalInput")
    idx_d = nc.dram_tensor("idx", [128, TT * 8], mybir.dt.int16, kind="ExternalInput")
    slots_d = nc.dram_tensor("slots", [128, TT], bf, kind="ExternalInput")
    out_d = nc.dram_tensor("out", [RPAD, OUTC], f32, kind="ExternalOutput")

    # tiles per supergather (group, sub)
    Tg = np.zeros((NGRP, 2), np.int64)
    for gi in range(NGRP):
        ws = range(gi * GRP, min((gi + 1) * GRP, WPC))
        for h in range(2):
            Tg[gi, h] = sum(int(Twh[w, h]) for w in ws)

    with tile.TileContext(nc) as tc:
        nc.gpsimd.load_library(mlp)
        with (
            tc.tile_pool(name="const", bufs=1) as cpool,
            tc.tile_pool(name="own", bufs=1) as opool,
            tc.tile_pool(name="evac", bufs=4) as epool,
            tc.tile_pool(name="att", bufs=4) as apool,
            tc.tile_pool(name="msg", bufs=10) as mpool,
            tc.tile_pool(name="sel", bufs=5) as spool,
            tc.tile_pool(name="part", bufs=WPC) as ppool,
            tc.tile_pool(name="p256", bufs=4, space="PSUM") as p256,
            tc.tile_pool(name="p128", bufs=3, space="PSUM") as p128,
            tc.tile_pool(name="ptr", bufs=1, space="PSUM") as ptr,
            tc.tile_pool(name="dram", bufs=1, space="DRAM") as dram,
        ):
            # ---- constants to SBUF
            w1_s = cpool.tile([128, 2, HID], bf)
            w2_s = cpool.tile([128, 2, OUTC], bf)
            iota_s = cpool.tile([128, 128], bf)
            ident_s = cpool.tile([128, 128], bf)
            dc1_s = cpool.tile([128, WPC], f32)
            dc2_s = cpool.tile([128, WPC], f32)
            idx_s = cpool.tile([128, TT * 8], mybir.dt.int16)
            slots_s = cpool.tile([128, TT], bf)
            ownx_s = opool.tile([128, WPC, INC], bf)    # own xd rows per window
            own2_s = opool.tile([128, WPC, OUTC], bf)   # own table2 rows
            for k in range(2):
                nc.sync.dma_start(w1_s[:, k, :], w1_d[k])
                nc.sync.dma_start(w2_s[:, k, :], w2_d[k])
            nc.sync.dma_start(iota_s[:], iota_d[:])
            nc.sync.dma_start(ident_s[:], ident_d[:])
            nc.sync.dma_start(dc1_s[:], dc1_d[:])
            nc.sync.dma_start(dc2_s[:], dc2_d[:])
            nc.sync.dma_start(idx_s[:], idx_d[:])
            nc.sync.dma_start(slots_s[:], slots_d[:])
            nc.scalar.dma_start(
                ownx_s[:], ownx_d[:].rearrange("p (w c) -> p w c", w=WPC))

            ag2a_in = dram.tile([LSPL, OUTC], bf)
            ag2b_in = dram.tile([RPAD - LSPL, OUTC], bf)
            tb2a = dram.tile([NA, OUTC], bf)
            tb2b = dram.tile([NB, OUTC], bf)

            # ---- edge aggregation unit: gathers + S build for one (group, sub)
            def gather_unit(gi, h, tbl, width, qctr):
                T = int(Tg[gi, h])
                if T == 0:
                    return None, None
                ws = list(range(gi * GRP, min((gi + 1) * GRP, WPC)))
                b = int(base[ws[0], h])
                m_s = mpool.tile([128, T, width], bf, tag="msg")
                nc.gpsimd.dma_gather(
                    m_s[:], tbl[:, :], idx_s[:, b * 8 : (b + T) * 8],
                    T * 128, T * 128, width,
                    single_packet=SINGLE_PACKET, queue_num=qctr[0] % 4)
                qctr[0] += 1
                S_s = spool.tile([128, T, 128], bf, tag="sel")
                nc.vector.tensor_tensor(
                    out=S_s[:],
                    in0=slots_s[:, b : b + T, None].to_broadcast([128, T, 128]),
                    in1=iota_s[:, None, :].to_broadcast([128, T, 128]),
                    op=mybir.AluOpType.is_equal)
                return m_s, S_s

            def win_mms(w, h, ps, m_s, S_s, first, last):
                gw0 = (w // GRP) * GRP
                b = int(base[gw0, h])
                n = int(Twh[w, h])
                for t in range(n):
                    tt = int(base[w, h]) - b + t
                    nc.tensor.matmul(ps[:], lhsT=S_s[:, tt, :], rhs=m_s[:, tt, :],
                                     start=(first and t == 0),
                                     stop=(last and t == n - 1))

            qctr = [0]

            # ---- layer-1 aggregation (aggregate-first; B-side groups first)
            def l1_group(gi):
                ws = list(range(gi * GRP, min((gi + 1) * GRP, WPC)))
                units = {}
                for h in range(2):
                    units[h] = gather_unit(gi, h, xda_d if h == 0 else xdb_d,
                                           INC, qctr)
                pss = {}
                for w in ws:
                    ps = p256.tile([128, INC], f32, tag="p256")
                    pss[w] = ps
                    started = False
                    for h in range(2):
                        m_s, S_s = units[h]
                        if m_s is None or Twh[w, h] == 0:
                            continue
                        win_mms(w, h, ps, m_s, S_s, not started, False)
                        started = True
                    # self-loop contribution: own xd rows
                    nc.tensor.matmul(ps[:], lhsT=ident_s[:], rhs=ownx_s[:, w, :],
                                     start=not started, stop=True)
                for w in ws:
                    ps = pss[w]
                    # raw aggregate -> bf16 -> transpose -> @W1 -> relu*dinv^2
                    a_s = epool.tile([128, INC], bf, tag="a")
                    nc.scalar.activation(a_s[:], ps[:],
                                         mybir.ActivationFunctionType.Copy)
                    at_s = apool.tile([128, 2, 128], bf, tag="at")
                    for k in range(2):
                        pt = ptr.tile([128, 128], bf, tag="pt")
                        nc.tensor.transpose(pt[:], a_s[:, k * 128 : (k + 1) * 128],
                                            ident_s[:])
                        nc.vector.tensor_copy(at_s[:, k, :], pt[:])
                    psH = p256.tile([128, HID], f32, tag="p256")
                    for k in range(2):
                        nc.tensor.matmul(psH[:], lhsT=at_s[:, k, :],
                                         rhs=w1_s[:, k, :],
                                         start=(k == 0), stop=(k == 1))
                    g_s = epool.tile([128, HID], bf, tag="g")
                    nc.scalar.activation(g_s[:], psH[:],
                                         mybir.ActivationFunctionType.Relu,
                                         scale=dc2_s[:, w : w + 1])
                    # g~ -> transpose -> @W2 -> own table2 rows
                    gt_s = apool.tile([128, 2, 128], bf, tag="at")
                    for k in range(2):
                        pt = ptr.tile([128, 128], bf, tag="pt")
                        nc.tensor.transpose(pt[:], g_s[:, k * 128 : (k + 1) * 128],
                                            ident_s[:])
                        nc.vector.tensor_copy(gt_s[:, k, :], pt[:])
                    ps2 = p128.tile([128, OUTC], f32, tag="p128")
                    for k in range(2):
                        nc.tensor.matmul(ps2[:], lhsT=gt_s[:, k, :],
                                         rhs=w2_s[:, k, :],
                                         start=(k == 0), stop=(k == 1))
                    nc.vector.tensor_copy(own2_s[:, w, :], ps2[:])
                    if w < WA:
                        nc.sync.dma_start(ag2a_in[w * 128 : (w + 1) * 128, :],
                                          own2_s[:, w, :])
                    else:
                        nc.sync.dma_start(ag2b_in[(w - WA) * 128 : (w - WA + 1) * 128, :],
                                          own2_s[:, w, :])

            # ---- layer-2 stage-1 unit: self + sub-B messages -> partial
            partials = {}

            def p6b_unit(gi):
                ws = list(range(gi * GRP, min((gi + 1) * GRP, WPC)))
                m_s, S_s = gather_unit(gi, 1, tb2b, OUTC, qctr)
                for w in ws:
                    ps = p128.tile([128, OUTC], f32, tag="p128")
                    started = False
                    if m_s is not None and Twh[w, 1] > 0:
                        win_mms(w, 1, ps, m_s, S_s, True, False)
                        started = True
                    nc.tensor.matmul(ps[:], lhsT=ident_s[:], rhs=own2_s[:, w, :],
                                     start=not started, stop=True)
                    pp = ppool.tile([128, OUTC], bf, tag="partial")
                    nc.scalar.activation(pp[:], ps[:],
                                         mybir.ActivationFunctionType.Copy)
                    partials[w] = pp

            with nc.named_scope("p3_l1b"):
                for gi in GORDER[: NGRP - NGA]:
                    l1_group(gi)
            # AG2 for sub-table B launches while L1 still works on A-side groups
            with nc.named_scope("ag2b"):
                nc.gpsimd.collective_compute(
                    "AllGather", mybir.AluOpType.bypass,
                    replica_groups=[list(range(NCORES))],
                    ins=[ag2b_in.opt()], outs=[tb2b.opt()])
            # A-side L1 groups with a few p6b units interleaved late (after
            # ag2b has surely completed); the rest of p6b is issued after the
            # ag2a trigger so it fills that collective's latency.
            pending = list(GORDER[: NGRP - NGA])
            with nc.named_scope("p3_l1a_mix"):
                for j, gi in enumerate(GORDER[NGRP - NGA :]):
                    l1_group(gi)
                    pending.append(gi)
                    if j >= 6 and pending:
                        p6b_unit(pending.pop(0))
            with nc.named_scope("ag2a"):
                nc.gpsimd.collective_compute(
                    "AllGather", mybir.AluOpType.bypass,
                    replica_groups=[list(range(NCORES))],
                    ins=[ag2a_in.opt()], outs=[tb2a.opt()])
            with nc.named_scope("p6_b_drain"):
                for gi in pending:
                    p6b_unit(gi)

            with nc.named_scope("p6_a"):
                # stage 2: partial + sub-A messages -> output (accumulated on PE)
                for gi in range(NGRP):
                    ws = list(range(gi * GRP, min((gi + 1) * GRP, WPC)))
                    m_s, S_s = gather_unit(gi, 0, tb2a, OUTC, qctr)
                    for w in ws:
                        has_msg = m_s is not None and Twh[w, 0] > 0
                        ps = p128.tile([128, OUTC], f32, tag="p128")
                        nc.tensor.matmul(ps[:], lhsT=ident_s[:], rhs=partials[w][:],
                                         start=True, stop=not has_msg)
                        if has_msg:
                            win_mms(w, 0, ps, m_s, S_s, False, True)
                        o_s = epool.tile([128, OUTC], f32, tag="o")
                        nc.scalar.activation(o_s[:], ps[:],
                                             mybir.ActivationFunctionType.Copy,
                                             scale=dc1_s[:, w : w + 1])
                        nc.sync.dma_start(out_d[w * 128 : (w + 1) * 128, :], o_s[:])

    nc.compile()
    return nc


def kernel(x, edge_index, W1, b1, W2, b2):
    x = np.asarray(x, np.float32)
    W1 = np.asarray(W1, np.float32)
    W2 = np.asarray(W2, np.float32)
    assert not np.any(np.asarray(b1)) and not np.any(np.asarray(b2)), \
        "kernel assumes zero biases (as in the reference setup)"

    idx16, slots, Twh, base, TT, dcol1, dcol2, dinv = _preprocess(np.asarray(edge_index))
    nc = _build(TT, Twh, base)

    iota = np.broadcast_to(np.arange(128, dtype=np.float32), (128, 128)).astype(ml_dtypes.bfloat16)
    ident = np.eye(128, dtype=np.float32).astype(ml_dtypes.bfloat16)
    w1_in = np.ascontiguousarray(W1.reshape(2, 128, HID)).astype(ml_dtypes.bfloat16)
    w2_in = np.ascontiguousarray(W2.reshape(2, 128, OUTC)).astype(ml_dtypes.bfloat16)
    xda, xdb, ownx = _xd_tables(x, dinv)
    slots_bf = slots.astype(ml_dtypes.bfloat16)

    in_maps = []
    for c in range(NCORES):
        in_maps.append({
            "xda": xda, "xdb": xdb, "ownx": ownx[c],
            "w1": w1_in, "w2": w2_in, "iota": iota, "ident": ident,
            "dcol1": dcol1[c], "dcol2": dcol2[c],
            "idx": idx16[c], "slots": slots_bf[c],
        })

    trace = bool(int(os.environ.get("GCN_KERNEL_TRACE", "0")))
    try:
        res = run_bass_kernel_spmd(nc, in_maps, core_ids=list(range(NCORES)), trace=trace)
    except Exception:
        # rare transient NRT exec failure: retry once on a fresh dispatch
        time_mod = __import__("time"); time_mod.sleep(2.0)
        res = run_bass_kernel_spmd(nc, in_maps, core_ids=list(range(NCORES)), trace=False)
    kernel.last_results = res
    if trace:
        print(f"HW exec time: {res.exec_time_ns} ns")
        kernel.last_exec_time_ns = res.exec_time_ns

    out = np.concatenate([res.results[c]["out"][:RPC] for c in range(NCORES)], axis=0)
    return out.astype(np.float32)


# revision 12
# speedup vs baseline: 1.1627x; 1.1472x over previous
"""GCN encoder (2-layer GCNConv, PyG-style) on 8 Trainium2 NeuronCores.

Sharding: nodes row-sharded 6250/core; edges partitioned by destination-node
owner; per-core segment-sum over 128-dst-slot windows via selection-matrix
matmuls.

v2: aggregate-first layer 1. Since segment_sum commutes with @W1, the layer-1
gather table is just dinv.*x (bf16, prepared on host, uploaded as input) —
no on-device table build, no replicated GEMM, and gathers start at t=0.
Per window, after the raw-feature aggregation:
  A[dst,256]  = sum_e xd[src_e] + xd[dst]          (S-matmuls + identity matmul)
  g~          = dinv^2 .* relu(A @ W1)             (transpose, GEMM, relu-scale)
  table2 rows = g~ @ W2                            (transpose, GEMM)
Layer 2 stays transform-first (OUTC < HID): table2 is all-gathered, split
into two collectives (sub-tables A/B) that overlap with remaining L1 work.

norm = dinv[src]*dinv[dst] folding (b1 == b2 == 0):
  xd    = dinv .* x
  g~    = dinv^2 .* relu(segsum(xd[src]) @ W1) = dinv .* h
  out   = dinv .* segsum((g~ @ W2)[src])

Self-loop messages never go through the gather path: their contribution to a
window's segment-sum is the core's own xd / table2 rows, added with one
identity matmul per window from SBUF-resident copies.

Sub-tables (for int16 gather indices and collective splitting): local row
l < 3200 (windows 0-24) -> sub A (8*3200 = 25600 rows); l >= 3200
(windows 25-48) -> sub B (8*3072 = 24576 rows). Both < 2**15.
"""

import os
import numpy as np
import ml_dtypes

import concourse.bacc as bacc
import concourse.tile as tile
from concourse import bass, mybir
from concourse.bass_utils import run_bass_kernel_spmd
from concourse.library_config import mlp

N = 50000
INC, HID, OUTC = 256, 256, 128
NCORES = 8
RPC = N // NCORES            # 6250 rows per core
WPC = (RPC + 127) // 128     # 49 windows per core
RPAD = WPC * 128             # 6272
LSPL = 3200                  # sub-table split on local row (windows 0..24 | 25..48)
NA = NCORES * LSPL           # 25600 rows in sub-table A
NB = NCORES * (RPAD - LSPL)  # 24576 rows in sub-table B
WA = LSPL // 128             # 25 windows in A
GRP = 2                      # windows per supergather group
NGRP = (WPC + GRP - 1) // GRP
# L1 processes B-side groups first so AG2(B) can launch early.
NGA = WA // GRP
GORDER = list(range(NGA, NGRP)) + list(range(0, NGA))
# AllGather piece boundaries (local rows) within each sub-table; pieces are
# triggered progressively as their windows complete so collective latency
# hides behind layer-1 work. Tables are laid out piece-major:
# row(piece p, rank r, local l) = 8*P[p] + r*(P[p+1]-P[p]) + (l-P[p]).
PA = (0, 768, 1536, 2304, 3200)       # sub A local rows [0, LSPL)
PB = (0, 768, 1536, 2304, 3072)       # sub B local rows [0, RPAD-LSPL)

SINGLE_PACKET = bool(int(os.environ.get("GCN_SINGLE_PACKET", "0")))


def _preprocess(edge_index):
    """Edge partitioning / ordering and normalization constants (host, index-only)."""
    src = np.asarray(edge_index[0], np.int64)
    dst = np.asarray(edge_index[1], np.int64)

    # degrees include the self-loops the reference adds
    deg = (np.bincount(dst, minlength=N) + 1).astype(np.float64)
    dinv = (1.0 / np.sqrt(deg)).astype(np.float32)

    owner = dst // RPC
    dstl = dst - owner * RPC
    win = dstl >> 7
    slot = dstl & 127
    srho = src // RPC
    srl = src - srho * RPC
    sub = (srl >= LSPL).astype(np.int64)

    def _piece_gl(local, P):
        P = np.asarray(P)
        p = np.searchsorted(P, local, side="right") - 1
        return 8 * P[p] + srho * (P[p + 1] - P[p]) + (local - P[p])

    gl = np.where(sub == 0, _piece_gl(np.minimum(srl, LSPL - 1), PA),
                  _piece_gl(np.maximum(srl - LSPL, 0), PB)).astype(np.int32)

    # sort by (bucket, src row): ascending addresses within each bucket make
    # the gather's HBM access pattern row-buffer friendly
    order = np.lexsort((gl, (owner * WPC + win) * 2 + sub))
    key_s = ((owner * WPC + win) * 2 + sub)[order]
    gl_s = gl[order]
    slot_s = slot[order].astype(np.int32)

    nbuck = NCORES * WPC * 2
    counts = np.bincount(key_s, minlength=nbuck).reshape(NCORES, WPC, 2)
    starts_flat = np.concatenate([[0], np.cumsum(counts.reshape(-1))])

    # tiles per (window, sub): max over cores so one SPMD program fits all
    Twh = (counts.max(axis=0) + 127) // 128     # [WPC, 2]
    TT = int(Twh.sum())
    # stream order: group -> sub -> window in group -> tiles
    base = np.zeros((WPC, 2), np.int64)
    pos = 0
    for gi in range(NGRP):
        ws = range(gi * GRP, min((gi + 1) * GRP, WPC))
        for h in range(2):
            for w in ws:
                base[w, h] = pos
                pos += Twh[w, h]
    assert pos == TT

    idx_seq = np.zeros((NCORES, TT * 128), np.int32)
    slot_seq = np.full((NCORES, TT * 128), 128, np.int32)  # 128 = dropped sentinel
    for c in range(NCORES):
        for w in range(WPC):
            for h in range(2):
                n = counts[c, w, h]
                if n == 0:
                    continue
                s0 = starts_flat[(c * WPC + w) * 2 + h]
                p0 = base[w, h] * 128
                idx_seq[c, p0 : p0 + n] = gl_s[s0 : s0 + n]
                slot_seq[c, p0 : p0 + n] = slot_s[s0 : s0 + n]

    # wrapped int16 gather-index layout: element j at [j%16, j//16], replicated x8
    idx16 = np.empty((NCORES, 128, TT * 8), np.int16)
    slots = np.empty((NCORES, 128, TT), np.float32)
    for c in range(NCORES):
        a = idx_seq[c].astype(np.int16).reshape(-1, 16).T
        idx16[c] = np.tile(a, (8, 1))
        slots[c] = slot_seq[c].astype(np.float32).reshape(TT, 128).T

    # per-core per-window dinv columns for own rows
    dcol1 = np.zeros((NCORES, 128, WPC), np.float32)
    for c in range(NCORES):
        d = np.zeros(RPAD, np.float32)
        d[:RPC] = dinv[c * RPC : (c + 1) * RPC]
        dcol1[c] = d.reshape(WPC, 128).T
    dcol2 = dcol1 * dcol1

    return idx16, slots, Twh, base, TT, dcol1, dcol2, dinv


def _xd_tables(x, dinv):
    """dinv.*x rows in [A | B] rank-major padded order, bf16; plus per-core
    own-row blocks in partition-major [128, WPC*256] layout."""
    xd = (x * dinv[:, None]).astype(np.float32)
    xda = np.zeros((NA, INC), np.float32)
    xdb = np.zeros((NB, INC), np.float32)
    for rho in range(NCORES):
        xs = np.zeros((RPAD, INC), np.float32)
        xs[:RPC] = xd[rho * RPC : (rho + 1) * RPC]   # [6272, 256] padded
        for p in range(4):
            lo, hi = PA[p], PA[p + 1]
            xda[8 * lo + rho * (hi - lo) : 8 * lo + (rho + 1) * (hi - lo)] = xs[lo:hi]
            lo, hi = PB[p], PB[p + 1]
            xdb[8 * lo + rho * (hi - lo) : 8 * lo + (rho + 1) * (hi - lo)] = \
                xs[LSPL + lo : LSPL + hi]
    ownx = np.zeros((NCORES, 128, WPC, INC), np.float32)
    for c in range(NCORES):
        blk = np.zeros((RPAD, INC), np.float32)
        blk[:RPC] = xd[c * RPC : (c + 1) * RPC]
        ownx[c] = blk.reshape(WPC, 128, INC).transpose(1, 0, 2)
    return (xda.astype(ml_dtypes.bfloat16), xdb.astype(ml_dtypes.bfloat16),
            ownx.reshape(NCORES, 128, WPC * INC).astype(ml_dtypes.bfloat16))


def _build(TT, Twh, base):
    nc = bacc.Bacc("TRN2", num_devices=NCORES, num_swdge_queues=4)
    f32 = mybir.dt.float32
    bf = mybir.dt.bfloat16

    xda_d = nc.dram_tensor("xda", [NA, INC], bf, kind="ExternalInput")
    xdb_d = nc.dram_tensor("xdb", [NB, INC], bf, kind="ExternalInput")
    ownx_d = nc.dram_tensor("ownx", [128, WPC * INC], bf, kind="ExternalInput")
    w1_d = nc.dram_tensor("w1", [2, 128, HID], bf, kind="ExternalInput")
    w2_d = nc.dram_tensor("w2", [2, 128, OUTC], bf, kind="ExternalInput")
    iota_d = nc.dram_tensor("iota", [128, 128], bf, kind="ExternalInput")
    ident_d = nc.dram_tensor("ident", [128, 128], bf, kind="ExternalInput")
    dc1_d = nc.dram_tensor("dcol1", [128, WPC], f32, kind="ExternalInput")
    dc2_d = nc.dram_tensor("dcol2", [128, WPC], f32, kind="ExternalInput")
    idx_d = nc.dram_tensor("idx", [128, TT * 8], mybir.dt.int16, kind="ExternalInput")
    slots_d = nc.dram_tensor("slots", [128, TT], bf, kind="ExternalInput")
    out_d = nc.dram_tensor("out", [RPAD, OUTC], f32, kind="ExternalOutput")

    # tiles per supergather (group, sub)
    Tg = np.zeros((NGRP, 2), np.int64)
    for gi in range(NGRP):
        ws = range(gi * GRP, min((gi + 1) * GRP, WPC))
        for h in range(2):
            Tg[gi, h] = sum(int(Twh[w, h]) for w in ws)

    with tile.TileContext(nc) as tc:
        nc.gpsimd.load_library(mlp)
        with (
            tc.tile_pool(name="const", bufs=1) as cpool,
            tc.tile_pool(name="own", bufs=1) as opool,
            tc.tile_pool(name="evac", bufs=4) as epool,
            tc.tile_pool(name="att", bufs=4) as apool,
            tc.tile_pool(name="msg", bufs=10) as mpool,
            tc.tile_pool(name="sel", bufs=5) as spool,
            tc.tile_pool(name="part", bufs=WPC) as ppool,
            tc.tile_pool(name="p256", bufs=4, space="PSUM") as p256,
            tc.tile_pool(name="p128", bufs=3, space="PSUM") as p128,
            tc.tile_pool(name="ptr", bufs=1, space="PSUM") as ptr,
            tc.tile_pool(name="dram", bufs=1, space="DRAM") as dram,
        ):
            # ---- constants to SBUF
            w1_s = cpool.tile([128, 2, HID], bf)
            w2_s = cpool.tile([128, 2, OUTC], bf)
            iota_s = cpool.tile([128, 128], bf)
            ident_s = cpool.tile([128, 128], bf)
            dc1_s = cpool.tile([128, WPC], f32)
            dc2_s = cpool.tile([128, WPC], f32)
            idx_s = cpool.tile([128, TT * 8], mybir.dt.int16)
            slots_s = cpool.tile([128, TT], bf)
            ownx_s = opool.tile([128, WPC, INC], bf)    # own xd rows per window
            own2_s = opool.tile([128, WPC, OUTC], bf)   # own table2 rows
            for k in range(2):
                nc.sync.dma_start(w1_s[:, k, :], w1_d[k])
                nc.sync.dma_start(w2_s[:, k, :], w2_d[k])
            nc.sync.dma_start(iota_s[:], iota_d[:])
            nc.sync.dma_start(ident_s[:], ident_d[:])
            nc.sync.dma_start(dc1_s[:], dc1_d[:])
            nc.sync.dma_start(dc2_s[:], dc2_d[:])
            nc.sync.dma_start(idx_s[:], idx_d[:])
            nc.sync.dma_start(slots_s[:], slots_d[:])
            nc.scalar.dma_start(
                ownx_s[:], ownx_d[:].rearrange("p (w c) -> p w c", w=WPC))

            ag2a_in = dram.tile([LSPL, OUTC], bf)
            ag2b_in = dram.tile([RPAD - LSPL, OUTC], bf)
            tb2a = dram.tile([NA, OUTC], bf)
            tb2b = dram.tile([NB, OUTC], bf)

            # ---- edge aggregation unit: gathers + S build for one (group, sub)
            def gather_unit(gi, h, tbl, width, qctr):
                T = int(Tg[gi, h])
                if T == 0:
                    return None, None
                ws = list(range(gi * GRP, min((gi + 1) * GRP, WPC)))
                b = int(base[ws[0], h])
                m_s = mpool.tile([128, T, width], bf, tag="msg")
                nc.gpsimd.dma_gather(
                    m_s[:], tbl[:, :], idx_s[:, b * 8 : (b + T) * 8],
                    T * 128, T * 128, width,
                    single_packet=SINGLE_PACKET, queue_num=qctr[0] % 4)
                qctr[0] += 1
                S_s = spool.tile([128, T, 128], bf, tag="sel")
                nc.vector.tensor_tensor(
                    out=S_s[:],
                    in0=slots_s[:, b : b + T, None].to_broadcast([128, T, 128]),
                    in1=iota_s[:, None, :].to_broadcast([128, T, 128]),
                    op=mybir.AluOpType.is_equal)
                return m_s, S_s

            def win_mms(w, h, ps, m_s, S_s, first, last):
                gw0 = (w // GRP) * GRP
                b = int(base[gw0, h])
                n = int(Twh[w, h])
                for t in range(n):
                    tt = int(base[w, h]) - b + t
                    nc.tensor.matmul(ps[:], lhsT=S_s[:, tt, :], rhs=m_s[:, tt, :],
                                     start=(first and t == 0),
                                     stop=(last and t == n - 1))

            qctr = [0]

            # ---- layer-1 aggregation (aggregate-first; B-side groups first)
            def l1_group(gi):
                ws = list(range(gi * GRP, min((gi + 1) * GRP, WPC)))
                units = {}
                for h in range(2):
                    units[h] = gather_unit(gi, h, xda_d if h == 0 else xdb_d,
                                           INC, qctr)
                pss = {}
                for w in ws:
                    ps = p256.tile([128, INC], f32, tag="p256")
                    pss[w] = ps
                    started = False
                    for h in range(2):
                        m_s, S_s = units[h]
                        if m_s is None or Twh[w, h] == 0:
                            continue
                        win_mms(w, h, ps, m_s, S_s, not started, False)
                        started = True
                    # self-loop contribution: own xd rows
                    nc.tensor.matmul(ps[:], lhsT=ident_s[:], rhs=ownx_s[:, w, :],
                                     start=not started, stop=True)
                for w in ws:
                    ps = pss[w]
                    # raw aggregate -> bf16 -> transpose -> @W1 -> relu*dinv^2
                    a_s = epool.tile([128, INC], bf, tag="a")
                    nc.scalar.activation(a_s[:], ps[:],
                                         mybir.ActivationFunctionType.Copy)
                    at_s = apool.tile([128, 2, 128], bf, tag="at")
                    for k in range(2):
                        pt = ptr.tile([128, 128], bf, tag="pt")
                        nc.tensor.transpose(pt[:], a_s[:, k * 128 : (k + 1) * 128],
                                            ident_s[:])
                        nc.vector.tensor_copy(at_s[:, k, :], pt[:])
                    psH = p256.tile([128, HID], f32, tag="p256")
                    for k in range(2):
                        nc.tensor.matmul(psH[:], lhsT=at_s[:, k, :],
                                         rhs=w1_s[:, k, :],
                                         start=(k == 0), stop=(k == 1))
                    g_s = epool.tile([128, HID], bf, tag="g")
                    nc.scalar.activation(g_s[:], psH[:],
                                         mybir.ActivationFunctionType.Relu,
                                         scale=dc2_s[:, w : w + 1])
                    # g~ -> transpose -> @W2 -> own table2 rows
                    gt_s = apool.tile([128, 2, 128], bf, tag="at")
                    for k in range(2):
                        pt = ptr.tile([128, 128], bf, tag="pt")
                        nc.tensor.transpose(pt[:], g_s[:, k * 128 : (k + 1) * 128],
                                            ident_s[:])
                        nc.vector.tensor_copy(gt_s[:, k, :], pt[:])
                    ps2 = p128.tile([128, OUTC], f32, tag="p128")
                    for k in range(2):
                        nc.tensor.matmul(ps2[:], lhsT=gt_s[:, k, :],
                                         rhs=w2_s[:, k, :],
                                         start=(k == 0), stop=(k == 1))
                    nc.vector.tensor_copy(own2_s[:, w, :], ps2[:])
                    if w < WA:
                        nc.sync.dma_start(ag2a_in[w * 128 : (w + 1) * 128, :],
                                          own2_s[:, w, :])
                    else:
                        nc.sync.dma_start(ag2b_in[(w - WA) * 128 : (w - WA + 1) * 128, :],
                                          own2_s[:, w, :])

            # ---- layer-2 stage-1 unit: self + sub-B messages -> partial
            partials = {}

            def p6b_unit(gi):
                ws = list(range(gi * GRP, min((gi + 1) * GRP, WPC)))
                m_s, S_s = gather_unit(gi, 1, tb2b, OUTC, qctr)
                for w in ws:
                    ps = p128.tile([128, OUTC], f32, tag="p128")
                    started = False
                    if m_s is not None and Twh[w, 1] > 0:
                        win_mms(w, 1, ps, m_s, S_s, True, False)
                        started = True
                    nc.tensor.matmul(ps[:], lhsT=ident_s[:], rhs=own2_s[:, w, :],
                                     start=not started, stop=True)
                    pp = ppool.tile([128, OUTC], bf, tag="partial")
                    nc.scalar.activation(pp[:], ps[:],
                                         mybir.ActivationFunctionType.Copy)
                    partials[w] = pp

            def ag_piece(which, p):
                lo, hi = (PA[p], PA[p + 1]) if which == "a" else (PB[p], PB[p + 1])
                inp = ag2a_in if which == "a" else ag2b_in
                outp = tb2a if which == "a" else tb2b
                with nc.named_scope(f"ag2{which}{p}"):
                    nc.gpsimd.collective_compute(
                        "AllGather", mybir.AluOpType.bypass,
                        replica_groups=[list(range(NCORES))],
                        ins=[inp[lo:hi, :].opt()],
                        outs=[outp[8 * lo : 8 * hi, :].opt()])

            # B-side groups (windows 24..48); AG pieces fire as windows finish
            with nc.named_scope("p3_l1b"):
                for k, gi in enumerate(GORDER[: NGRP - NGA]):
                    l1_group(gi)
                    if k == 3:
                        ag_piece("b", 0)
                    elif k == 6:
                        ag_piece("b", 1)
                    elif k == 9:
                        ag_piece("b", 2)
                ag_piece("b", 3)
            # A-side groups (windows 0..23)
            with nc.named_scope("p3_l1a"):
                for j, gi in enumerate(GORDER[NGRP - NGA :]):
                    l1_group(gi)
                    if j == 2:
                        ag_piece("a", 0)
                    elif j == 5:
                        ag_piece("a", 1)
                    elif j == 8:
                        ag_piece("a", 2)
                ag_piece("a", 3)
            # layer-2 stage 1 fills the final AG piece's latency
            with nc.named_scope("p6_b"):
                for gi in GORDER:
                    p6b_unit(gi)

            with nc.named_scope("p6_a"):
                # stage 2: partial + sub-A messages -> output (accumulated on PE)
                for gi in range(NGRP):
                    ws = list(range(gi * GRP, min((gi + 1) * GRP, WPC)))
                    m_s, S_s = gather_unit(gi, 0, tb2a, OUTC, qctr)
                    for w in ws:
                        has_msg = m_s is not None and Twh[w, 0] > 0
                        ps = p128.tile([128, OUTC], f32, tag="p128")
                        nc.tensor.matmul(ps[:], lhsT=ident_s[:], rhs=partials[w][:],
                                         start=True, stop=not has_msg)
                        if has_msg:
                            win_mms(w, 0, ps, m_s, S_s, False, True)
                        o_s = epool.tile([128, OUTC], f32, tag="o")
                        nc.scalar.activation(o_s[:], ps[:],
                                             mybir.ActivationFunctionType.Copy,
                                             scale=dc1_s[:, w : w + 1])
                        nc.sync.dma_start(out_d[w * 128 : (w + 1) * 128, :], o_s[:])

    nc.compile()
    return nc


def kernel(x, edge_index, W1, b1, W2, b2):
    x = np.asarray(x, np.float32)
    W1 = np.asarray(W1, np.float32)
    W2 = np.asarray(W2, np.float32)
    assert not np.any(np.asarray(b1)) and not np.any(np.asarray(b2)), \
        "kernel assumes zero biases (as in the reference setup)"

    idx16, slots, Twh, base, TT, dcol1, dcol2, dinv = _preprocess(np.asarray(edge_index))
    nc = _build(TT, Twh, base)

    iota = np.broadcast_to(np.arange(128, dtype=np.float32), (128, 128)).astype(ml_dtypes.bfloat16)
    ident = np.eye(128, dtype=np.float32).astype(ml_dtypes.bfloat16)
    w1_in = np.ascontiguousarray(W1.reshape(2, 128, HID)).astype(ml_dtypes.bfloat16)
    w2_in = np.ascontiguousarray(W2.reshape(2, 128, OUTC)).astype(ml_dtypes.bfloat16)
    xda, xdb, ownx = _xd_tables(x, dinv)
    slots_bf = slots.astype(ml_dtypes.bfloat16)

    in_maps = []
    for c in range(NCORES):
        in_maps.append({
            "xda": xda, "xdb": xdb, "ownx": ownx[c],
            "w1": w1_in, "w2": w2_in, "iota": iota, "ident": ident,
            "dcol1": dcol1[c], "dcol2": dcol2[c],
            "idx": idx16[c], "slots": slots_bf[c],
        })

    trace = bool(int(os.environ.get("GCN_KERNEL_TRACE", "0")))
    try:
        res = run_bass_kernel_spmd(nc, in_maps, core_ids=list(range(NCORES)), trace=trace)
    except Exception:
        # rare transient NRT exec failure: retry once on a fresh dispatch
        time_mod = __import__("time"); time_mod.sleep(2.0)
        res = run_bass_kernel_spmd(nc, in_maps, core_ids=list(range(NCORES)), trace=False)
    kernel.last_results = res
    if trace:
        print(f"HW exec time: {res.exec_time_ns} ns")
        kernel.last_exec_time_ns = res.exec_time_ns

    out = np.concatenate([res.results[c]["out"][:RPC] for c in range(NCORES)], axis=0)
    return out.astype(np.float32)


# revision 17
# speedup vs baseline: 1.2550x; 1.0794x over previous
"""GCN encoder (2-layer GCNConv, PyG-style) on 8 Trainium2 NeuronCores.

Sharding: nodes row-sharded 6250/core; edges partitioned by destination-node
owner; per-core segment-sum over 128-dst-slot windows via selection-matrix
matmuls.

v2: aggregate-first layer 1. Since segment_sum commutes with @W1, the layer-1
gather table is just dinv.*x (bf16, prepared on host, uploaded as input) —
no on-device table build, no replicated GEMM, and gathers start at t=0.
Per window, after the raw-feature aggregation:
  A[dst,256]  = sum_e xd[src_e] + xd[dst]          (S-matmuls + identity matmul)
  g~          = dinv^2 .* relu(A @ W1)             (transpose, GEMM, relu-scale)
  table2 rows = g~ @ W2                            (transpose, GEMM)
Layer 2 stays transform-first (OUTC < HID): table2 is all-gathered, split
into two collectives (sub-tables A/B) that overlap with remaining L1 work.

norm = dinv[src]*dinv[dst] folding (b1 == b2 == 0):
  xd    = dinv .* x
  g~    = dinv^2 .* relu(segsum(xd[src]) @ W1) = dinv .* h
  out   = dinv .* segsum((g~ @ W2)[src])

Self-loop messages never go through the gather path: their contribution to a
window's segment-sum is the core's own xd / table2 rows, added with one
identity matmul per window from SBUF-resident copies.

Sub-tables (for int16 gather indices and collective splitting): local row
l < 3200 (windows 0-24) -> sub A (8*3200 = 25600 rows); l >= 3200
(windows 25-48) -> sub B (8*3072 = 24576 rows). Both < 2**15.
"""

import os
import numpy as np
import ml_dtypes

import concourse.bacc as bacc
import concourse.tile as tile
from concourse import bass, mybir
from concourse.bass_utils import run_bass_kernel_spmd
from concourse.library_config import mlp

N = 50000
INC, HID, OUTC = 256, 256, 128
NCORES = 8
RPC = N // NCORES            # 6250 rows per core
WPC = (RPC + 127) // 128     # 49 windows per core
RPAD = WPC * 128             # 6272
LSPL = 2176                  # sub-table split on local row (windows 0..16 | 17..48);
                             # B is maxed at 32768 rows = the int16 gather-index limit
                             # so the tail phase (sub A, gathered last) is smallest
NA = NCORES * LSPL           # 25600 rows in sub-table A
NB = NCORES * (RPAD - LSPL)  # 24576 rows in sub-table B
WA = LSPL // 128             # 25 windows in A
GRP = 2                      # windows per supergather group
NGRP = (WPC + GRP - 1) // GRP
# L1 processes B-side groups first so AG2(B) can launch early.
NGA = WA // GRP
GORDER = list(range(NGA, NGRP)) + list(range(0, NGA))
# AllGather piece boundaries (local rows) within each sub-table; pieces are
# triggered progressively as their windows complete so collective latency
# hides behind layer-1 work. Tables are laid out piece-major:
# row(piece p, rank r, local l) = 8*P[p] + r*(P[p+1]-P[p]) + (l-P[p]).
PA = (0, 512, 1024, 1536, 2176)       # sub A local rows [0, LSPL)
PB = (0, 1024, 2048, 3072, 4096)      # sub B local rows [0, RPAD-LSPL)

SINGLE_PACKET = bool(int(os.environ.get("GCN_SINGLE_PACKET", "0")))


def _preprocess(edge_index):
    """Edge partitioning / ordering and normalization constants (host, index-only)."""
    src = np.asarray(edge_index[0], np.int64)
    dst = np.asarray(edge_index[1], np.int64)

    # degrees include the self-loops the reference adds
    deg = (np.bincount(dst, minlength=N) + 1).astype(np.float64)
    dinv = (1.0 / np.sqrt(deg)).astype(np.float32)

    owner = dst // RPC
    dstl = dst - owner * RPC
    win = dstl >> 7
    slot = dstl & 127
    srho = src // RPC
    srl = src - srho * RPC
    sub = (srl >= LSPL).astype(np.int64)

    def _piece_gl(local, P):
        P = np.asarray(P)
        p = np.searchsorted(P, local, side="right") - 1
        return 8 * P[p] + srho * (P[p + 1] - P[p]) + (local - P[p])

    gl = np.where(sub == 0, _piece_gl(np.minimum(srl, LSPL - 1), PA),
                  _piece_gl(np.maximum(srl - LSPL, 0), PB)).astype(np.int32)
    assert gl.max() < 32768, "gather indices must fit int16"

    # sort by (bucket, src row): ascending addresses within each bucket make
    # the gather's HBM access pattern row-buffer friendly
    order = np.lexsort((gl, (owner * WPC + win) * 2 + sub))
    key_s = ((owner * WPC + win) * 2 + sub)[order]
    gl_s = gl[order]
    slot_s = slot[order].astype(np.int32)

    nbuck = NCORES * WPC * 2
    counts = np.bincount(key_s, minlength=nbuck).reshape(NCORES, WPC, 2)
    starts_flat = np.concatenate([[0], np.cumsum(counts.reshape(-1))])

    # tiles per (window, sub): max over cores so one SPMD program fits all
    Twh = (counts.max(axis=0) + 127) // 128     # [WPC, 2]
    TT = int(Twh.sum())
    # stream order: group -> sub -> window in group -> tiles
    base = np.zeros((WPC, 2), np.int64)
    pos = 0
    for gi in range(NGRP):
        ws = range(gi * GRP, min((gi + 1) * GRP, WPC))
        for h in range(2):
            for w in ws:
                base[w, h] = pos
                pos += Twh[w, h]
    assert pos == TT

    idx_seq = np.zeros((NCORES, TT * 128), np.int32)
    slot_seq = np.full((NCORES, TT * 128), 128, np.int32)  # 128 = dropped sentinel
    for c in range(NCORES):
        for w in range(WPC):
            for h in range(2):
                n = counts[c, w, h]
                if n == 0:
                    continue
                s0 = starts_flat[(c * WPC + w) * 2 + h]
                p0 = base[w, h] * 128
                idx_seq[c, p0 : p0 + n] = gl_s[s0 : s0 + n]
                slot_seq[c, p0 : p0 + n] = slot_s[s0 : s0 + n]

    # wrapped int16 gather-index layout: element j at [j%16, j//16], replicated x8
    idx16 = np.empty((NCORES, 128, TT * 8), np.int16)
    slots = np.empty((NCORES, 128, TT), np.float32)
    for c in range(NCORES):
        a = idx_seq[c].astype(np.int16).reshape(-1, 16).T
        idx16[c] = np.tile(a, (8, 1))
        slots[c] = slot_seq[c].astype(np.float32).reshape(TT, 128).T

    # per-core per-window dinv columns for own rows
    dcol1 = np.zeros((NCORES, 128, WPC), np.float32)
    for c in range(NCORES):
        d = np.zeros(RPAD, np.float32)
        d[:RPC] = dinv[c * RPC : (c + 1) * RPC]
        dcol1[c] = d.reshape(WPC, 128).T
    dcol2 = dcol1 * dcol1

    return idx16, slots, Twh, base, TT, dcol1, dcol2, dinv


def _xd_tables(x, dinv):
    """dinv.*x rows in [A | B] rank-major padded order, bf16; plus per-core
    own-row blocks in partition-major [128, WPC*256] layout."""
    xd = (x * dinv[:, None]).astype(np.float32)
    xda = np.zeros((NA, INC), np.float32)
    xdb = np.zeros((NB, INC), np.float32)
    for rho in range(NCORES):
        xs = np.zeros((RPAD, INC), np.float32)
        xs[:RPC] = xd[rho * RPC : (rho + 1) * RPC]   # [6272, 256] padded
        for p in range(4):
            lo, hi = PA[p], PA[p + 1]
            xda[8 * lo + rho * (hi - lo) : 8 * lo + (rho + 1) * (hi - lo)] = xs[lo:hi]
            lo, hi = PB[p], PB[p + 1]
            xdb[8 * lo + rho * (hi - lo) : 8 * lo + (rho + 1) * (hi - lo)] = \
                xs[LSPL + lo : LSPL + hi]
    ownx = np.zeros((NCORES, 128, WPC, INC), np.float32)
    for c in range(NCORES):
        blk = np.zeros((RPAD, INC), np.float32)
        blk[:RPC] = xd[c * RPC : (c + 1) * RPC]
        ownx[c] = blk.reshape(WPC, 128, INC).transpose(1, 0, 2)
    return (xda.astype(ml_dtypes.bfloat16), xdb.astype(ml_dtypes.bfloat16),
            ownx.reshape(NCORES, 128, WPC * INC).astype(ml_dtypes.bfloat16))


def _build(TT, Twh, base):
    nc = bacc.Bacc("TRN2", num_devices=NCORES, num_swdge_queues=4)
    f32 = mybir.dt.float32
    bf = mybir.dt.bfloat16

    xda_d = nc.dram_tensor("xda", [NA, INC], bf, kind="ExternalInput")
    xdb_d = nc.dram_tensor("xdb", [NB, INC], bf, kind="ExternalInput")
    ownx_d = nc.dram_tensor("ownx", [128, WPC * INC], bf, kind="ExternalInput")
    w1_d = nc.dram_tensor("w1", [2, 128, HID], bf, kind="ExternalInput")
    w2_d = nc.dram_tensor("w2", [2, 128, OUTC], bf, kind="ExternalInput")
    iota_d = nc.dram_tensor("iota", [128, 128], bf, kind="ExternalInput")
    ident_d = nc.dram_tensor("ident", [128, 128], bf, kind="ExternalInput")
    dc1_d = nc.dram_tensor("dcol1", [128, WPC], f32, kind="ExternalInput")
    dc2_d = nc.dram_tensor("dcol2", [128, WPC], f32, kind="ExternalInput")
    idx_d = nc.dram_tensor("idx", [128, TT * 8], mybir.dt.int16, kind="ExternalInput")
    slots_d = nc.dram_tensor("slots", [128, TT], bf, kind="ExternalInput")
    out_d = nc.dram_tensor("out", [RPAD, OUTC], f32, kind="ExternalOutput")

    # tiles per supergather (group, sub)
    Tg = np.zeros((NGRP, 2), np.int64)
    for gi in range(NGRP):
        ws = range(gi * GRP, min((gi + 1) * GRP, WPC))
        for h in range(2):
            Tg[gi, h] = sum(int(Twh[w, h]) for w in ws)

    with tile.TileContext(nc) as tc:
        nc.gpsimd.load_library(mlp)
        with (
            tc.tile_pool(name="const", bufs=1) as cpool,
            tc.tile_pool(name="own", bufs=1) as opool,
            tc.tile_pool(name="evac", bufs=4) as epool,
            tc.tile_pool(name="att", bufs=4) as apool,
            tc.tile_pool(name="msg", bufs=9) as mpool,
            tc.tile_pool(name="sel", bufs=5) as spool,
            tc.tile_pool(name="part", bufs=WPC) as ppool,
            tc.tile_pool(name="p256", bufs=4, space="PSUM") as p256,
            tc.tile_pool(name="p128", bufs=3, space="PSUM") as p128,
            tc.tile_pool(name="ptr", bufs=1, space="PSUM") as ptr,
            tc.tile_pool(name="dram", bufs=1, space="DRAM") as dram,
        ):
            # ---- constants to SBUF
            w1_s = cpool.tile([128, 2, HID], bf)
            w2_s = cpool.tile([128, 2, OUTC], bf)
            iota_s = cpool.tile([128, 128], bf)
            ident_s = cpool.tile([128, 128], bf)
            dc1_s = cpool.tile([128, WPC], f32)
            dc2_s = cpool.tile([128, WPC], f32)
            idx_s = cpool.tile([128, TT * 8], mybir.dt.int16)
            slots_s = cpool.tile([128, TT], bf)
            ownx_s = opool.tile([128, WPC, INC], bf)    # own xd rows per window
            own2_s = opool.tile([128, WPC, OUTC], bf)   # own table2 rows
            for k in range(2):
                nc.sync.dma_start(w1_s[:, k, :], w1_d[k])
                nc.sync.dma_start(w2_s[:, k, :], w2_d[k])
            nc.sync.dma_start(iota_s[:], iota_d[:])
            nc.sync.dma_start(ident_s[:], ident_d[:])
            nc.sync.dma_start(dc1_s[:], dc1_d[:])
            nc.sync.dma_start(dc2_s[:], dc2_d[:])
            nc.sync.dma_start(idx_s[:], idx_d[:])
            nc.sync.dma_start(slots_s[:], slots_d[:])
            nc.scalar.dma_start(
                ownx_s[:], ownx_d[:].rearrange("p (w c) -> p w c", w=WPC))

            ag2a_in = dram.tile([LSPL, OUTC], bf)
            ag2b_in = dram.tile([RPAD - LSPL, OUTC], bf)
            tb2a = dram.tile([NA, OUTC], bf)
            tb2b = dram.tile([NB, OUTC], bf)

            # ---- edge aggregation unit: gathers + S build for one (group, sub)
            def gather_unit(gi, h, tbl, width, qctr):
                T = int(Tg[gi, h])
                if T == 0:
                    return None, None
                ws = list(range(gi * GRP, min((gi + 1) * GRP, WPC)))
                b = int(base[ws[0], h])
                m_s = mpool.tile([128, T, width], bf, tag="msg")
                nc.gpsimd.dma_gather(
                    m_s[:], tbl[:, :], idx_s[:, b * 8 : (b + T) * 8],
                    T * 128, T * 128, width,
                    single_packet=SINGLE_PACKET, queue_num=qctr[0] % 4)
                qctr[0] += 1
                S_s = spool.tile([128, T, 128], bf, tag="sel")
                nc.vector.tensor_tensor(
                    out=S_s[:],
                    in0=slots_s[:, b : b + T, None].to_broadcast([128, T, 128]),
                    in1=iota_s[:, None, :].to_broadcast([128, T, 128]),
                    op=mybir.AluOpType.is_equal)
                return m_s, S_s

            def win_mms(w, h, ps, m_s, S_s, first, last):
                gw0 = (w // GRP) * GRP
                b = int(base[gw0, h])
                n = int(Twh[w, h])
                for t in range(n):
                    tt = int(base[w, h]) - b + t
                    nc.tensor.matmul(ps[:], lhsT=S_s[:, tt, :], rhs=m_s[:, tt, :],
                                     start=(first and t == 0),
                                     stop=(last and t == n - 1))

            qctr = [0]

            # ---- layer-1 aggregation (aggregate-first; B-side groups first)
            def l1_group(gi):
                ws = list(range(gi * GRP, min((gi + 1) * GRP, WPC)))
                units = {}
                for h in range(2):
                    units[h] = gather_unit(gi, h, xda_d if h == 0 else xdb_d,
                                           INC, qctr)
                pss = {}
                for w in ws:
                    ps = p256.tile([128, INC], f32, tag="p256")
                    pss[w] = ps
                    started = False
                    for h in range(2):
                        m_s, S_s = units[h]
                        if m_s is None or Twh[w, h] == 0:
                            continue
                        win_mms(w, h, ps, m_s, S_s, not started, False)
                        started = True
                    # self-loop contribution: own xd rows
                    nc.tensor.matmul(ps[:], lhsT=ident_s[:], rhs=ownx_s[:, w, :],
                                     start=not started, stop=True)
                for w in ws:
                    ps = pss[w]
                    # raw aggregate -> bf16 -> transpose -> @W1 -> relu*dinv^2
                    a_s = epool.tile([128, INC], bf, tag="a")
                    nc.scalar.activation(a_s[:], ps[:],
                                         mybir.ActivationFunctionType.Copy)
                    at_s = apool.tile([128, 2, 128], bf, tag="at")
                    for k in range(2):
                        pt = ptr.tile([128, 128], bf, tag="pt")
                        nc.tensor.transpose(pt[:], a_s[:, k * 128 : (k + 1) * 128],
                                            ident_s[:])
                        nc.vector.tensor_copy(at_s[:, k, :], pt[:])
                    psH = p256.tile([128, HID], f32, tag="p256")
                    for k in range(2):
                        nc.tensor.matmul(psH[:], lhsT=at_s[:, k, :],
                                         rhs=w1_s[:, k, :],
                                         start=(k == 0), stop=(k == 1))
                    g_s = epool.tile([128, HID], bf, tag="g")
                    nc.scalar.activation(g_s[:], psH[:],
                                         mybir.ActivationFunctionType.Relu,
                                         scale=dc2_s[:, w : w + 1])
                    # g~ -> transpose -> @W2 -> own table2 rows
                    gt_s = apool.tile([128, 2, 128], bf, tag="at")
                    for k in range(2):
                        pt = ptr.tile([128, 128], bf, tag="pt")
                        nc.tensor.transpose(pt[:], g_s[:, k * 128 : (k + 1) * 128],
                                            ident_s[:])
                        nc.vector.tensor_copy(gt_s[:, k, :], pt[:])
                    ps2 = p128.tile([128, OUTC], f32, tag="p128")
                    for k in range(2):
                        nc.tensor.matmul(ps2[:], lhsT=gt_s[:, k, :],
                                         rhs=w2_s[:, k, :],
                                         start=(k == 0), stop=(k == 1))
                    nc.vector.tensor_copy(own2_s[:, w, :], ps2[:])
                    if w < WA:
                        nc.sync.dma_start(ag2a_in[w * 128 : (w + 1) * 128, :],
                                          own2_s[:, w, :])
                    else:
                        nc.sync.dma_start(ag2b_in[(w - WA) * 128 : (w - WA + 1) * 128, :],
                                          own2_s[:, w, :])

            # ---- layer-2 stage-1 unit: self + sub-B messages -> partial
            partials = {}

            def p6b_unit(gi):
                ws = list(range(gi * GRP, min((gi + 1) * GRP, WPC)))
                m_s, S_s = gather_unit(gi, 1, tb2b, OUTC, qctr)
                for w in ws:
                    ps = p128.tile([128, OUTC], f32, tag="p128")
                    started = False
                    if m_s is not None and Twh[w, 1] > 0:
                        win_mms(w, 1, ps, m_s, S_s, True, False)
                        started = True
                    nc.tensor.matmul(ps[:], lhsT=ident_s[:], rhs=own2_s[:, w, :],
                                     start=not started, stop=True)
                    pp = ppool.tile([128, OUTC], bf, tag="partial")
                    nc.scalar.activation(pp[:], ps[:],
                                         mybir.ActivationFunctionType.Copy)
                    partials[w] = pp

            def ag_piece(which, p):
                lo, hi = (PA[p], PA[p + 1]) if which == "a" else (PB[p], PB[p + 1])
                inp = ag2a_in if which == "a" else ag2b_in
                outp = tb2a if which == "a" else tb2b
                with nc.named_scope(f"ag2{which}{p}"):
                    nc.gpsimd.collective_compute(
                        "AllGather", mybir.AluOpType.bypass,
                        replica_groups=[list(range(NCORES))],
                        ins=[inp[lo:hi, :].opt()],
                        outs=[outp[8 * lo : 8 * hi, :].opt()])

            # AG piece triggers are delayed ~2 groups past the group that
            # completes their input windows: the trigger's dependency wait is
            # then already satisfied, so it never stalls the in-order gather
            # stream on the Pool sequencer.
            # B-side groups (windows 16..48); B pieces cover windows
            # 17-24 / 25-32 / 33-40 / 41-48, done at k = 4 / 8 / 12 / 16(end)
            with nc.named_scope("p3_l1b"):
                for k, gi in enumerate(GORDER[: NGRP - NGA]):
                    l1_group(gi)
                    if k == 6:
                        ag_piece("b", 0)
                    elif k == 10:
                        ag_piece("b", 1)
                    elif k == 14:
                        ag_piece("b", 2)
            # A-side groups (windows 0..15); A pieces cover windows
            # 0-3 / 4-7 / 8-11 / 12-16, done at j = 1 / 3 / 5 / 7(end)
            with nc.named_scope("p3_l1a"):
                for j, gi in enumerate(GORDER[NGRP - NGA :]):
                    l1_group(gi)
                    if j == 1:
                        ag_piece("b", 3)
                    elif j == 3:
                        ag_piece("a", 0)
                    elif j == 5:
                        ag_piece("a", 1)
                    elif j == 7:
                        ag_piece("a", 2)
            # layer-2 stage 1; the last A piece fires once the final L1
            # windows' pipelines have surely drained
            with nc.named_scope("p6_b"):
                for n, gi in enumerate(GORDER):
                    p6b_unit(gi)
                    if n == 1:
                        ag_piece("a", 3)

            with nc.named_scope("p6_a"):
                # stage 2: partial + sub-A messages -> output (accumulated on PE)
                for gi in range(NGRP):
                    ws = list(range(gi * GRP, min((gi + 1) * GRP, WPC)))
                    m_s, S_s = gather_unit(gi, 0, tb2a, OUTC, qctr)
                    for w in ws:
                        has_msg = m_s is not None and Twh[w, 0] > 0
                        ps = p128.tile([128, OUTC], f32, tag="p128")
                        nc.tensor.matmul(ps[:], lhsT=ident_s[:], rhs=partials[w][:],
                                         start=True, stop=not has_msg)
                        if has_msg:
                            win_mms(w, 0, ps, m_s, S_s, False, True)
                        o_s = epool.tile([128, OUTC], f32, tag="o")
                        nc.scalar.activation(o_s[:], ps[:],
                                             mybir.ActivationFunctionType.Copy,
                                             scale=dc1_s[:, w : w + 1])
                        nc.sync.dma_start(out_d[w * 128 : (w + 1) * 128, :], o_s[:])

    nc.compile()
    return nc


def kernel(x, edge_index, W1, b1, W2, b2):
    x = np.asarray(x, np.float32)
    W1 = np.asarray(W1, np.float32)
    W2 = np.asarray(W2, np.float32)
    assert not np.any(np.asarray(b1)) and not np.any(np.asarray(b2)), \
        "kernel assumes zero biases (as in the reference setup)"

    idx16, slots, Twh, base, TT, dcol1, dcol2, dinv = _preprocess(np.asarray(edge_index))
    nc = _build(TT, Twh, base)

    iota = np.broadcast_to(np.arange(128, dtype=np.float32), (128, 128)).astype(ml_dtypes.bfloat16)
    ident = np.eye(128, dtype=np.float32).astype(ml_dtypes.bfloat16)
    w1_in = np.ascontiguousarray(W1.reshape(2, 128, HID)).astype(ml_dtypes.bfloat16)
    w2_in = np.ascontiguousarray(W2.reshape(2, 128, OUTC)).astype(ml_dtypes.bfloat16)
    xda, xdb, ownx = _xd_tables(x, dinv)
    slots_bf = slots.astype(ml_dtypes.bfloat16)

    in_maps = []
    for c in range(NCORES):
        in_maps.append({
            "xda": xda, "xdb": xdb, "ownx": ownx[c],
            "w1": w1_in, "w2": w2_in, "iota": iota, "ident": ident,
            "dcol1": dcol1[c], "dcol2": dcol2[c],
            "idx": idx16[c], "slots": slots_bf[c],
        })

    trace = bool(int(os.environ.get("GCN_KERNEL_TRACE", "0")))
    try:
        res = run_bass_kernel_spmd(nc, in_maps, core_ids=list(range(NCORES)), trace=trace)
    except Exception:
        # rare transient NRT exec failure: retry once on a fresh dispatch
        time_mod = __import__("time"); time_mod.sleep(2.0)
        res = run_bass_kernel_spmd(nc, in_maps, core_ids=list(range(NCORES)), trace=False)
    kernel.last_results = res
    if trace:
        print(f"HW exec time: {res.exec_time_ns} ns")
        kernel.last_exec_time_ns = res.exec_time_ns

    out = np.concatenate([res.results[c]["out"][:RPC] for c in range(NCORES)], axis=0)
    return out.astype(np.float32)


# revision 18
# speedup vs baseline: 1.3002x; 1.0359x over previous
"""GCN encoder (2-layer GCNConv, PyG-style) on 8 Trainium2 NeuronCores.

Sharding: nodes row-sharded 6250/core; edges partitioned by destination-node
owner; per-core segment-sum over 128-dst-slot windows via selection-matrix
matmuls.

Aggregate-first layer 1: since segment_sum commutes with @W1, the layer-1
gather table is just dinv.*x (bf16, prepared on host, uploaded as input) —
no on-device table build, and gathers start at t=0. Per window, after the
raw-feature aggregation:
  A[dst,256]  = sum_e xd[src_e] + xd[dst]          (S-matmuls + identity matmul)
  g~          = dinv^2 .* relu(A @ W1)             (transpose, GEMM, relu-scale)
  table2 rows = g~ @ W2                            (transpose, GEMM)
Layer 2 stays transform-first (OUTC < HID): table2 is all-gathered in 8
progressive pieces (4 per sub-table) that overlap layer-1 work.

norm = dinv[src]*dinv[dst] folding (b1 == b2 == 0):
  xd    = dinv .* x
  g~    = dinv^2 .* relu(segsum(xd[src]) @ W1) = dinv .* h
  out   = dinv .* segsum((g~ @ W2)[src])

Self-loop messages never go through the gather path: their contribution to a
window's segment-sum is the core's own xd / table2 rows, added with one
identity matmul per window from SBUF-resident copies.

Gathers are issued per (window, sub-table) on 4 rotating SWDGE queues; the
measured bottleneck is a fixed per-descriptor cadence (~60ns/desc/engine), so
smaller, more numerous gathers maximize in-flight concurrency. Edges within a
bucket are sorted by source row for HBM row-buffer locality.

Sub-tables (int16 gather-index limit 32768 rows): local row l < 2176
(windows 0-16) -> sub A; l >= 2176 (windows 17-48) -> sub B (32768 rows
exactly). B is processed first in layer 1 and maximal so the final tail
phase (sub A of layer 2) is smallest.
"""

import os
import numpy as np
import ml_dtypes

import concourse.bacc as bacc
import concourse.tile as tile
from concourse import bass, mybir
from concourse.bass_utils import run_bass_kernel_spmd
from concourse.library_config import mlp

N = 50000
INC, HID, OUTC = 256, 256, 128
NCORES = 8
RPC = N // NCORES            # 6250 rows per core
WPC = (RPC + 127) // 128     # 49 windows per core
RPAD = WPC * 128             # 6272
LSPL = 2176                  # sub-table split on local row (windows 0..16 | 17..48)
NA = NCORES * LSPL           # 17408 rows in sub-table A
NB = NCORES * (RPAD - LSPL)  # 32768 rows in sub-table B
WA = LSPL // 128             # 17 windows in A
# layer-1 window order: B-side windows first (their table2 rows feed the
# earlier AllGather pieces), w16 leads so ag2a piece 3's input is ready early
WORDER = list(range(WA - 1, WPC)) + list(range(0, WA - 1))
# AllGather piece boundaries (local rows) within each sub-table; pieces are
# triggered progressively as their windows complete. Tables are piece-major:
# row(piece p, rank r, local l) = 8*P[p] + r*(P[p+1]-P[p]) + (l-P[p]).
PA = (0, 512, 1024, 1536, 2176)       # windows 0-3 | 4-7 | 8-11 | 12-16
PB = (0, 1024, 2048, 3072, 4096)      # windows 17-24 | 25-32 | 33-40 | 41-48


def _preprocess(edge_index):
    """Edge partitioning / ordering and normalization constants (host, index-only)."""
    src = np.asarray(edge_index[0], np.int64)
    dst = np.asarray(edge_index[1], np.int64)

    # degrees include the self-loops the reference adds
    deg = (np.bincount(dst, minlength=N) + 1).astype(np.float64)
    dinv = (1.0 / np.sqrt(deg)).astype(np.float32)

    owner = dst // RPC
    dstl = dst - owner * RPC
    win = dstl >> 7
    slot = dstl & 127
    srho = src // RPC
    srl = src - srho * RPC
    sub = (srl >= LSPL).astype(np.int64)

    def _piece_gl(local, P):
        P = np.asarray(P)
        p = np.searchsorted(P, local, side="right") - 1
        return 8 * P[p] + srho * (P[p + 1] - P[p]) + (local - P[p])

    gl = np.where(sub == 0, _piece_gl(np.minimum(srl, LSPL - 1), PA),
                  _piece_gl(np.maximum(srl - LSPL, 0), PB)).astype(np.int32)
    assert gl.max() < 32768, "gather indices must fit int16"

    # sort by (bucket, src row): ascending addresses within each bucket make
    # the gather's HBM access pattern row-buffer friendly
    order = np.lexsort((gl, (owner * WPC + win) * 2 + sub))
    key_s = ((owner * WPC + win) * 2 + sub)[order]
    gl_s = gl[order]
    slot_s = slot[order].astype(np.int32)

    nbuck = NCORES * WPC * 2
    counts = np.bincount(key_s, minlength=nbuck).reshape(NCORES, WPC, 2)
    starts_flat = np.concatenate([[0], np.cumsum(counts.reshape(-1))])

    # tiles per (window, sub): max over cores so one SPMD program fits all
    Twh = (counts.max(axis=0) + 127) // 128     # [WPC, 2]
    TT = int(Twh.sum())
    # stream order: layer-1 window order -> sub -> tiles
    base = np.zeros((WPC, 2), np.int64)
    pos = 0
    for w in WORDER:
        for h in range(2):
            base[w, h] = pos
            pos += Twh[w, h]
    assert pos == TT

    idx_seq = np.zeros((NCORES, TT * 128), np.int32)
    slot_seq = np.full((NCORES, TT * 128), 128, np.int32)  # 128 = dropped sentinel
    for c in range(NCORES):
        for w in range(WPC):
            for h in range(2):
                n = counts[c, w, h]
                if n == 0:
                    continue
                s0 = starts_flat[(c * WPC + w) * 2 + h]
                p0 = base[w, h] * 128
                idx_seq[c, p0 : p0 + n] = gl_s[s0 : s0 + n]
                slot_seq[c, p0 : p0 + n] = slot_s[s0 : s0 + n]

    # wrapped int16 gather-index layout: element j at [j%16, j//16], replicated x8
    idx16 = np.empty((NCORES, 128, TT * 8), np.int16)
    slots = np.empty((NCORES, 128, TT), np.float32)
    for c in range(NCORES):
        a = idx_seq[c].astype(np.int16).reshape(-1, 16).T
        idx16[c] = np.tile(a, (8, 1))
        slots[c] = slot_seq[c].astype(np.float32).reshape(TT, 128).T

    # per-core per-window dinv columns for own rows
    dcol1 = np.zeros((NCORES, 128, WPC), np.float32)
    for c in range(NCORES):
        d = np.zeros(RPAD, np.float32)
        d[:RPC] = dinv[c * RPC : (c + 1) * RPC]
        dcol1[c] = d.reshape(WPC, 128).T
    dcol2 = dcol1 * dcol1

    return idx16, slots, Twh, base, TT, dcol1, dcol2, dinv


def _xd_tables(x, dinv):
    """dinv.*x rows in piece-major [A | B] order, bf16; plus per-core
    own-row blocks in partition-major [128, WPC*256] layout."""
    xd = (x * dinv[:, None]).astype(np.float32)
    xda = np.zeros((NA, INC), np.float32)
    xdb = np.zeros((NB, INC), np.float32)
    for rho in range(NCORES):
        xs = np.zeros((RPAD, INC), np.float32)
        xs[:RPC] = xd[rho * RPC : (rho + 1) * RPC]   # [6272, 256] padded
        for p in range(4):
            lo, hi = PA[p], PA[p + 1]
            xda[8 * lo + rho * (hi - lo) : 8 * lo + (rho + 1) * (hi - lo)] = xs[lo:hi]
            lo, hi = PB[p], PB[p + 1]
            xdb[8 * lo + rho * (hi - lo) : 8 * lo + (rho + 1) * (hi - lo)] = \
                xs[LSPL + lo : LSPL + hi]
    ownx = np.zeros((NCORES, 128, WPC, INC), np.float32)
    for c in range(NCORES):
        blk = np.zeros((RPAD, INC), np.float32)
        blk[:RPC] = xd[c * RPC : (c + 1) * RPC]
        ownx[c] = blk.reshape(WPC, 128, INC).transpose(1, 0, 2)
    return (xda.astype(ml_dtypes.bfloat16), xdb.astype(ml_dtypes.bfloat16),
            ownx.reshape(NCORES, 128, WPC * INC).astype(ml_dtypes.bfloat16))


def _build(TT, Twh, base):
    nc = bacc.Bacc("TRN2", num_devices=NCORES, num_swdge_queues=4)
    f32 = mybir.dt.float32
    bf = mybir.dt.bfloat16

    xda_d = nc.dram_tensor("xda", [NA, INC], bf, kind="ExternalInput")
    xdb_d = nc.dram_tensor("xdb", [NB, INC], bf, kind="ExternalInput")
    ownx_d = nc.dram_tensor("ownx", [128, WPC * INC], bf, kind="ExternalInput")
    w1_d = nc.dram_tensor("w1", [2, 128, HID], bf, kind="ExternalInput")
    w2_d = nc.dram_tensor("w2", [2, 128, OUTC], bf, kind="ExternalInput")
    iota_d = nc.dram_tensor("iota", [128, 128], bf, kind="ExternalInput")
    ident_d = nc.dram_tensor("ident", [128, 128], bf, kind="ExternalInput")
    dc1_d = nc.dram_tensor("dcol1", [128, WPC], f32, kind="ExternalInput")
    dc2_d = nc.dram_tensor("dcol2", [128, WPC], f32, kind="ExternalInput")
    idx_d = nc.dram_tensor("idx", [128, TT * 8], mybir.dt.int16, kind="ExternalInput")
    slots_d = nc.dram_tensor("slots", [128, TT], bf, kind="ExternalInput")
    out_d = nc.dram_tensor("out", [RPAD, OUTC], f32, kind="ExternalOutput")

    with tile.TileContext(nc) as tc:
        nc.gpsimd.load_library(mlp)
        with (
            tc.tile_pool(name="const", bufs=1) as cpool,
            tc.tile_pool(name="own", bufs=1) as opool,
            tc.tile_pool(name="evac", bufs=4) as epool,
            tc.tile_pool(name="att", bufs=4) as apool,
            tc.tile_pool(name="msg", bufs=14) as mpool,
            tc.tile_pool(name="sel", bufs=8) as spool,
            tc.tile_pool(name="part", bufs=WPC) as ppool,
            tc.tile_pool(name="p256", bufs=4, space="PSUM") as p256,
            tc.tile_pool(name="p128", bufs=3, space="PSUM") as p128,
            tc.tile_pool(name="ptr", bufs=1, space="PSUM") as ptr,
            tc.tile_pool(name="dram", bufs=1, space="DRAM") as dram,
        ):
            # ---- constants to SBUF; idx loaded in two slices so the first
            # gathers don't wait on the full 1.8MB index transfer
            w1_s = cpool.tile([128, 2, HID], bf)
            w2_s = cpool.tile([128, 2, OUTC], bf)
            iota_s = cpool.tile([128, 128], bf)
            ident_s = cpool.tile([128, 128], bf)
            dc1_s = cpool.tile([128, WPC], f32)
            dc2_s = cpool.tile([128, WPC], f32)
            idx_s = cpool.tile([128, TT * 8], mybir.dt.int16)
            slots_s = cpool.tile([128, TT], bf)
            ownx_s = opool.tile([128, WPC, INC], bf)    # own xd rows per window
            own2_s = opool.tile([128, WPC, OUTC], bf)   # own table2 rows
            ISPL = min(2048, TT * 8)
            nc.sync.dma_start(idx_s[:, :ISPL], idx_d[:, :ISPL])
            nc.sync.dma_start(slots_s[:], slots_d[:])
            nc.sync.dma_start(iota_s[:], iota_d[:])
            for k in range(2):
                nc.sync.dma_start(w1_s[:, k, :], w1_d[k])
                nc.sync.dma_start(w2_s[:, k, :], w2_d[k])
            nc.sync.dma_start(ident_s[:], ident_d[:])
            nc.sync.dma_start(dc1_s[:], dc1_d[:])
            nc.sync.dma_start(dc2_s[:], dc2_d[:])
            if ISPL < TT * 8:
                nc.sync.dma_start(idx_s[:, ISPL:], idx_d[:, ISPL:])
            nc.scalar.dma_start(
                ownx_s[:], ownx_d[:].rearrange("p (w c) -> p w c", w=WPC))

            ag2a_in = dram.tile([LSPL, OUTC], bf)
            ag2b_in = dram.tile([RPAD - LSPL, OUTC], bf)
            tb2a = dram.tile([NA, OUTC], bf)
            tb2b = dram.tile([NB, OUTC], bf)

            qctr = [0]

            # ---- one gather + selection-matrix build for (window, sub)
            def gather_win(w, h, tbl, width):
                T = int(Twh[w, h])
                if T == 0:
                    return None, None
                b = int(base[w, h])
                m_s = mpool.tile([128, T, width], bf, tag="msg")
                nc.gpsimd.dma_gather(
                    m_s[:], tbl[:, :], idx_s[:, b * 8 : (b + T) * 8],
                    T * 128, T * 128, width,
                    single_packet=False, queue_num=qctr[0] % 4)
                qctr[0] += 1
                S_s = spool.tile([128, T, 128], bf, tag="sel")
                nc.vector.tensor_tensor(
                    out=S_s[:],
                    in0=slots_s[:, b : b + T, None].to_broadcast([128, T, 128]),
                    in1=iota_s[:, None, :].to_broadcast([128, T, 128]),
                    op=mybir.AluOpType.is_equal)
                return m_s, S_s

            # ---- layer-1 window: gather both subs, aggregate raw features,
            # then W1 -> relu -> W2 to produce this window's table2 rows
            def l1_window(w):
                units = [gather_win(w, h, xda_d if h == 0 else xdb_d, INC)
                         for h in range(2)]
                ps = p256.tile([128, INC], f32, tag="p256")
                started = False
                for h in range(2):
                    m_s, S_s = units[h]
                    if m_s is None:
                        continue
                    T = int(Twh[w, h])
                    for t in range(T):
                        nc.tensor.matmul(ps[:], lhsT=S_s[:, t, :], rhs=m_s[:, t, :],
                                         start=(not started and t == 0), stop=False)
                        started = True
                nc.tensor.matmul(ps[:], lhsT=ident_s[:], rhs=ownx_s[:, w, :],
                                 start=not started, stop=True)
                a_s = epool.tile([128, INC], bf, tag="a")
                nc.scalar.activation(a_s[:], ps[:],
                                     mybir.ActivationFunctionType.Copy)
                at_s = apool.tile([128, 2, 128], bf, tag="at")
                for k in range(2):
                    pt = ptr.tile([128, 128], bf, tag="pt")
                    nc.tensor.transpose(pt[:], a_s[:, k * 128 : (k + 1) * 128],
                                        ident_s[:])
                    nc.vector.tensor_copy(at_s[:, k, :], pt[:])
                psH = p256.tile([128, HID], f32, tag="p256")
                for k in range(2):
                    nc.tensor.matmul(psH[:], lhsT=at_s[:, k, :], rhs=w1_s[:, k, :],
                                     start=(k == 0), stop=(k == 1))
                g_s = epool.tile([128, HID], bf, tag="g")
                nc.scalar.activation(g_s[:], psH[:],
                                     mybir.ActivationFunctionType.Relu,
                                     scale=dc2_s[:, w : w + 1])
                gt_s = apool.tile([128, 2, 128], bf, tag="at")
                for k in range(2):
                    pt = ptr.tile([128, 128], bf, tag="pt")
                    nc.tensor.transpose(pt[:], g_s[:, k * 128 : (k + 1) * 128],
                                        ident_s[:])
                    nc.vector.tensor_copy(gt_s[:, k, :], pt[:])
                ps2 = p128.tile([128, OUTC], f32, tag="p128")
                for k in range(2):
                    nc.tensor.matmul(ps2[:], lhsT=gt_s[:, k, :], rhs=w2_s[:, k, :],
                                     start=(k == 0), stop=(k == 1))
                nc.vector.tensor_copy(own2_s[:, w, :], ps2[:])
                if w < WA:
                    nc.sync.dma_start(ag2a_in[w * 128 : (w + 1) * 128, :],
                                      own2_s[:, w, :])
                else:
                    lw = w - WA
                    nc.sync.dma_start(ag2b_in[lw * 128 : (lw + 1) * 128, :],
                                      own2_s[:, w, :])

            partials = {}

            # ---- layer-2 stage 1: self + sub-B messages -> partial
            def p6b_window(w):
                m_s, S_s = gather_win(w, 1, tb2b, OUTC)
                ps = p128.tile([128, OUTC], f32, tag="p128")
                started = False
                if m_s is not None:
                    for t in range(int(Twh[w, 1])):
                        nc.tensor.matmul(ps[:], lhsT=S_s[:, t, :], rhs=m_s[:, t, :],
                                         start=(t == 0), stop=False)
                    started = True
                nc.tensor.matmul(ps[:], lhsT=ident_s[:], rhs=own2_s[:, w, :],
                                 start=not started, stop=True)
                pp = ppool.tile([128, OUTC], bf, tag="partial")
                nc.scalar.activation(pp[:], ps[:],
                                     mybir.ActivationFunctionType.Copy)
                partials[w] = pp

            # ---- layer-2 stage 2: partial + sub-A messages -> output
            def p6a_window(w):
                m_s, S_s = gather_win(w, 0, tb2a, OUTC)
                ps = p128.tile([128, OUTC], f32, tag="p128")
                nc.tensor.matmul(ps[:], lhsT=ident_s[:], rhs=partials[w][:],
                                 start=True, stop=m_s is None)
                if m_s is not None:
                    T = int(Twh[w, 0])
                    for t in range(T):
                        nc.tensor.matmul(ps[:], lhsT=S_s[:, t, :], rhs=m_s[:, t, :],
                                         start=False, stop=(t == T - 1))
                o_s = epool.tile([128, OUTC], f32, tag="o")
                nc.scalar.activation(o_s[:], ps[:],
                                     mybir.ActivationFunctionType.Copy,
                                     scale=dc1_s[:, w : w + 1])
                nc.sync.dma_start(out_d[w * 128 : (w + 1) * 128, :], o_s[:])

            def ag_piece(which, p):
                lo, hi = (PA[p], PA[p + 1]) if which == "a" else (PB[p], PB[p + 1])
                inp = ag2a_in if which == "a" else ag2b_in
                outp = tb2a if which == "a" else tb2b
                with nc.named_scope(f"ag2{which}{p}"):
                    nc.gpsimd.collective_compute(
                        "AllGather", mybir.AluOpType.bypass,
                        replica_groups=[list(range(NCORES))],
                        ins=[inp[lo:hi, :].opt()],
                        outs=[outp[8 * lo : 8 * hi, :].opt()])

            # AG piece triggers are delayed ~4 windows past the window that
            # completes their input so their dependency wait is already
            # satisfied and never stalls the in-order gather stream.
            with nc.named_scope("p3_l1"):
                for w in WORDER:
                    l1_window(w)
                    if w == 28:
                        ag_piece("b", 0)      # windows 17-24, done at w24
                    elif w == 36:
                        ag_piece("b", 1)      # windows 25-32
                    elif w == 44:
                        ag_piece("b", 2)      # windows 33-40
                    elif w == 2:
                        ag_piece("b", 3)      # windows 41-48, done at B end
                    elif w == 7:
                        ag_piece("a", 0)      # windows 0-3
                    elif w == 11:
                        ag_piece("a", 1)      # windows 4-7
                    elif w == 15:
                        ag_piece("a", 2)      # windows 8-11
            with nc.named_scope("p6_b"):
                for n, w in enumerate(WORDER):
                    p6b_window(w)
                    if n == 3:
                        ag_piece("a", 3)      # windows 12-16, done at A end
            with nc.named_scope("p6_a"):
                for w in WORDER:
                    p6a_window(w)

    nc.compile()
    return nc


def kernel(x, edge_index, W1, b1, W2, b2):
    x = np.asarray(x, np.float32)
    W1 = np.asarray(W1, np.float32)
    W2 = np.asarray(W2, np.float32)
    assert not np.any(np.asarray(b1)) and not np.any(np.asarray(b2)), \
        "kernel assumes zero biases (as in the reference setup)"

    idx16, slots, Twh, base, TT, dcol1, dcol2, dinv = _preprocess(np.asarray(edge_index))
    nc = _build(TT, Twh, base)

    iota = np.broadcast_to(np.arange(128, dtype=np.float32), (128, 128)).astype(ml_dtypes.bfloat16)
    ident = np.eye(128, dtype=np.float32).astype(ml_dtypes.bfloat16)
    w1_in = np.ascontiguousarray(W1.reshape(2, 128, HID)).astype(ml_dtypes.bfloat16)
    w2_in = np.ascontiguousarray(W2.reshape(2, 128, OUTC)).astype(ml_dtypes.bfloat16)
    xda, xdb, ownx = _xd_tables(x, dinv)
    slots_bf = slots.astype(ml_dtypes.bfloat16)

    in_maps = []
    for c in range(NCORES):
        in_maps.append({
            "xda": xda, "xdb": xdb, "ownx": ownx[c],
            "w1": w1_in, "w2": w2_in, "iota": iota, "ident": ident,
            "dcol1": dcol1[c], "dcol2": dcol2[c],
            "idx": idx16[c], "slots": slots_bf[c],
        })

    trace = bool(int(os.environ.get("GCN_KERNEL_TRACE", "0")))
    try:
        res = run_bass_kernel_spmd(nc, in_maps, core_ids=list(range(NCORES)), trace=trace)
    except Exception:
        # rare transient NRT exec failure: retry once on a fresh dispatch
        time_mod = __import__("time"); time_mod.sleep(2.0)
        res = run_bass_kernel_spmd(nc, in_maps, core_ids=list(range(NCORES)), trace=False)
    kernel.last_results = res
    if trace:
        print(f"HW exec time: {res.exec_time_ns} ns")
        kernel.last_exec_time_ns = res.exec_time_ns

    out = np.concatenate([res.results[c]["out"][:RPC] for c in range(NCORES)], axis=0)
    return out.astype(np.float32)


# revision 26
# speedup vs baseline: 1.7250x; 1.3267x over previous
"""GCN encoder (2-layer GCNConv, PyG-style) on 8 Trainium2 NeuronCores.

Sharding: nodes row-sharded 6250/core; edges partitioned by destination-node
owner; per-core segment-sum over 128-dst-slot windows via selection-matrix
matmuls.

Aggregate-first layer 1: since segment_sum commutes with @W1, the layer-1
gather table is just dinv.*x (bf16, prepared on host, uploaded as input) —
no on-device table build, and gathers start at t=0. Per window, after the
raw-feature aggregation:
  A[dst,256]  = sum_e xd[src_e] + xd[dst]          (S-matmuls + identity matmul)
  g~          = dinv^2 .* relu(A @ W1)             (transpose, GEMM, relu-scale)
  table2 rows = g~ @ W2                            (transpose, GEMM)
Layer 2 stays transform-first (OUTC < HID): table2 is all-gathered in 8
progressive pieces (4 per sub-table) that overlap layer-1 work.

norm = dinv[src]*dinv[dst] folding (b1 == b2 == 0):
  xd    = dinv .* x
  g~    = dinv^2 .* relu(segsum(xd[src]) @ W1) = dinv .* h
  out   = dinv .* segsum((g~ @ W2)[src])

Self-loop messages never go through the gather path: their contribution to a
window's segment-sum is the core's own xd / table2 rows, added with one
identity matmul per window from SBUF-resident copies.

Gathers are issued per (window, sub-table) on 4 rotating SWDGE queues; the
measured bottleneck is a fixed per-descriptor cadence (~60ns/desc/engine), so
smaller, more numerous gathers maximize in-flight concurrency. Edges within a
bucket are sorted by source row for HBM row-buffer locality.

Sub-tables (int16 gather-index limit 32768 rows): local row l < 2176
(windows 0-16) -> sub A; l >= 2176 (windows 17-48) -> sub B (32768 rows
exactly). B is processed first in layer 1 and maximal so the final tail
phase (sub A of layer 2) is smallest.
"""

import os
import numpy as np
import ml_dtypes

import concourse.bacc as bacc
import concourse.tile as tile
from concourse import bass, mybir
from concourse.bass_utils import run_bass_kernel_spmd
from concourse.library_config import mlp

N = 50000
INC, HID, OUTC = 256, 256, 128
NCORES = 8
RPC = N // NCORES            # 6250 rows per core
WPC = (RPC + 127) // 128     # 49 windows per core
RPAD = WPC * 128             # 6272
LSPL = 2176                  # sub-table split on local row (windows 0..16 | 17..48)
NA = NCORES * LSPL           # 17408 rows in sub-table A
NB = NCORES * (RPAD - LSPL)  # 32768 rows in sub-table B
WA = LSPL // 128             # 17 windows in A
# layer-1 window order: B-side windows first (their table2 rows feed the
# earlier AllGather pieces), w16 leads so ag2a piece 3's input is ready early
WORDER = list(range(WA - 1, WPC)) + list(range(0, WA - 1))
# AllGather piece boundaries (local rows) within each sub-table; pieces are
# triggered progressively as their windows complete. Tables are piece-major:
# row(piece p, rank r, local l) = 8*P[p] + r*(P[p+1]-P[p]) + (l-P[p]).
PA = (0, 512, 1024, 1536, 2176)       # windows 0-3 | 4-7 | 8-11 | 12-16
PB = (0, 1024, 2048, 3072, 4096)      # windows 17-24 | 25-32 | 33-40 | 41-48


def _preprocess(edge_index):
    """Edge partitioning / ordering and normalization constants (host, index-only)."""
    src = np.asarray(edge_index[0], np.int64)
    dst = np.asarray(edge_index[1], np.int64)

    # degrees include the self-loops the reference adds
    deg = (np.bincount(dst, minlength=N) + 1).astype(np.float64)
    dinv = (1.0 / np.sqrt(deg)).astype(np.float32)

    owner = dst // RPC
    dstl = dst - owner * RPC
    win = dstl >> 7
    slot = dstl & 127
    srho = src // RPC
    srl = src - srho * RPC
    sub = (srl >= LSPL).astype(np.int64)

    def _piece_gl(local, P):
        P = np.asarray(P)
        p = np.searchsorted(P, local, side="right") - 1
        return 8 * P[p] + srho * (P[p + 1] - P[p]) + (local - P[p])

    gl = np.where(sub == 0, _piece_gl(np.minimum(srl, LSPL - 1), PA),
                  _piece_gl(np.maximum(srl - LSPL, 0), PB)).astype(np.int32)
    assert gl.max() < 32768, "gather indices must fit int16"

    # sort by (bucket, src row): ascending addresses within each bucket make
    # the gather's HBM access pattern row-buffer friendly
    order = np.lexsort((gl, (owner * WPC + win) * 2 + sub))
    key_s = ((owner * WPC + win) * 2 + sub)[order]
    gl_s = gl[order]
    slot_s = slot[order].astype(np.int32)

    nbuck = NCORES * WPC * 2
    counts = np.bincount(key_s, minlength=nbuck).reshape(NCORES, WPC, 2)
    starts_flat = np.concatenate([[0], np.cumsum(counts.reshape(-1))])

    # tiles per (window, sub): max over cores so one SPMD program fits all
    Twh = (counts.max(axis=0) + 127) // 128     # [WPC, 2]
    TT = int(Twh.sum())
    # stream order: layer-1 window order -> sub -> tiles
    base = np.zeros((WPC, 2), np.int64)
    pos = 0
    for w in WORDER:
        for h in range(2):
            base[w, h] = pos
            pos += Twh[w, h]
    assert pos == TT

    # indices beyond each core's actual bucket count are -1: together with a
    # runtime per-bucket count register, the gather skips the padded tail
    # entirely (the padding is ~12% of all descriptors). Skipped message rows
    # hold stale-but-finite data and are zeroed out by the S sentinel.
    idx_seq = np.full((NCORES, TT * 128), -1, np.int32)
    slot_seq = np.full((NCORES, TT * 128), 128, np.int32)  # 128 = dropped sentinel
    bcnt = np.ones((NCORES, WPC, 2), np.int32)
    for c in range(NCORES):
        for w in range(WPC):
            for h in range(2):
                n = counts[c, w, h]
                if Twh[w, h] == 0:
                    continue
                p0 = base[w, h] * 128
                if n == 0:
                    idx_seq[c, p0] = 0  # dummy valid index, dropped by S
                    continue
                s0 = starts_flat[(c * WPC + w) * 2 + h]
                idx_seq[c, p0 : p0 + n] = gl_s[s0 : s0 + n]
                slot_seq[c, p0 : p0 + n] = slot_s[s0 : s0 + n]
                bcnt[c, w, h] = n

    # wrapped int16 gather-index layout: element j at [j%16, j//16], replicated x8
    idx16 = np.empty((NCORES, 128, TT * 8), np.int16)
    slots = np.empty((NCORES, 128, TT), np.float32)
    for c in range(NCORES):
        a = idx_seq[c].astype(np.int16).reshape(-1, 16).T
        idx16[c] = np.tile(a, (8, 1))
        slots[c] = slot_seq[c].astype(np.float32).reshape(TT, 128).T

    # per-core per-window dinv columns for own rows
    dcol1 = np.zeros((NCORES, 128, WPC), np.float32)
    for c in range(NCORES):
        d = np.zeros(RPAD, np.float32)
        d[:RPC] = dinv[c * RPC : (c + 1) * RPC]
        dcol1[c] = d.reshape(WPC, 128).T
    dcol2 = dcol1 * dcol1

    # per-core bucket counts in stream order, replicated across partitions
    # for the gpsimd count-register loads
    bseq = np.empty((NCORES, WPC * 2), np.int32)
    i = 0
    for w in WORDER:
        for h in range(2):
            bseq[:, i] = bcnt[:, w, h]
            i += 1
    bseq = np.broadcast_to(bseq[:, None, :], (NCORES, 128, WPC * 2)).copy()

    return idx16, slots, Twh, base, TT, dcol1, dcol2, dinv, bseq


def _xd_tables(x, dinv):
    """dinv.*x rows in piece-major [A | B] order, bf16; plus per-core
    own-row blocks in partition-major [128, WPC*256] layout."""
    xd = (x * dinv[:, None]).astype(np.float32)
    xda = np.zeros((NA, INC), np.float32)
    xdb = np.zeros((NB, INC), np.float32)
    for rho in range(NCORES):
        xs = np.zeros((RPAD, INC), np.float32)
        xs[:RPC] = xd[rho * RPC : (rho + 1) * RPC]   # [6272, 256] padded
        for p in range(4):
            lo, hi = PA[p], PA[p + 1]
            xda[8 * lo + rho * (hi - lo) : 8 * lo + (rho + 1) * (hi - lo)] = xs[lo:hi]
            lo, hi = PB[p], PB[p + 1]
            xdb[8 * lo + rho * (hi - lo) : 8 * lo + (rho + 1) * (hi - lo)] = \
                xs[LSPL + lo : LSPL + hi]
    ownx = np.zeros((NCORES, 128, WPC, INC), np.float32)
    for c in range(NCORES):
        blk = np.zeros((RPAD, INC), np.float32)
        blk[:RPC] = xd[c * RPC : (c + 1) * RPC]
        ownx[c] = blk.reshape(WPC, 128, INC).transpose(1, 0, 2)
    return (xda.astype(ml_dtypes.bfloat16), xdb.astype(ml_dtypes.bfloat16),
            ownx.reshape(NCORES, 128, WPC * INC).astype(ml_dtypes.bfloat16))


def _build(TT, Twh, base):
    nc = bacc.Bacc("TRN2", num_devices=NCORES, num_swdge_queues=4)
    f32 = mybir.dt.float32
    bf = mybir.dt.bfloat16

    xda_d = nc.dram_tensor("xda", [NA, INC], bf, kind="ExternalInput")
    xdb_d = nc.dram_tensor("xdb", [NB, INC], bf, kind="ExternalInput")
    ownx_d = nc.dram_tensor("ownx", [128, WPC * INC], bf, kind="ExternalInput")
    w1_d = nc.dram_tensor("w1", [2, 128, HID], bf, kind="ExternalInput")
    w2_d = nc.dram_tensor("w2", [2, 128, OUTC], bf, kind="ExternalInput")
    iota_d = nc.dram_tensor("iota", [128, 128], bf, kind="ExternalInput")
    ident_d = nc.dram_tensor("ident", [128, 128], bf, kind="ExternalInput")
    dc1_d = nc.dram_tensor("dcol1", [128, WPC], f32, kind="ExternalInput")
    dc2_d = nc.dram_tensor("dcol2", [128, WPC], f32, kind="ExternalInput")
    idx_d = nc.dram_tensor("idx", [128, TT * 8], mybir.dt.int16, kind="ExternalInput")
    slots_d = nc.dram_tensor("slots", [128, TT], bf, kind="ExternalInput")
    bcnt_d = nc.dram_tensor("bcnt", [128, WPC * 2], mybir.dt.int32,
                            kind="ExternalInput")
    out_d = nc.dram_tensor("out", [RPAD, OUTC], f32, kind="ExternalOutput")

    BKT = {}
    for _w in WORDER:
        for _h in range(2):
            BKT[(_w, _h)] = len(BKT)

    with tile.TileContext(nc) as tc:
        nc.gpsimd.load_library(mlp)
        with (
            tc.tile_pool(name="const", bufs=1) as cpool,
            tc.tile_pool(name="own", bufs=1) as opool,
            tc.tile_pool(name="evac", bufs=4) as epool,
            tc.tile_pool(name="att", bufs=4) as apool,
            tc.tile_pool(name="msg", bufs=14) as mpool,
            tc.tile_pool(name="sel", bufs=8) as spool,
            tc.tile_pool(name="part", bufs=WPC) as ppool,
            tc.tile_pool(name="p256", bufs=4, space="PSUM") as p256,
            tc.tile_pool(name="p128", bufs=3, space="PSUM") as p128,
            tc.tile_pool(name="ptr", bufs=1, space="PSUM") as ptr,
            tc.tile_pool(name="dram", bufs=1, space="DRAM") as dram,
        ):
            # ---- constants to SBUF; idx loaded in two slices so the first
            # gathers don't wait on the full 1.8MB index transfer
            w1_s = cpool.tile([128, 2, HID], bf)
            w2_s = cpool.tile([128, 2, OUTC], bf)
            iota_s = cpool.tile([128, 128], bf)
            ident_s = cpool.tile([128, 128], bf)
            dc1_s = cpool.tile([128, WPC], f32)
            dc2_s = cpool.tile([128, WPC], f32)
            idx_s = cpool.tile([128, TT * 8], mybir.dt.int16)
            slots_s = cpool.tile([128, TT], bf)
            bcnt_s = cpool.tile([128, WPC * 2], mybir.dt.int32)
            ownx_s = opool.tile([128, WPC, INC], bf)    # own xd rows per window
            own2_s = opool.tile([128, WPC, OUTC], bf)   # own table2 rows
            ISPL = min(2048, TT * 8)
            nc.sync.dma_start(idx_s[:, :ISPL], idx_d[:, :ISPL])
            nc.sync.dma_start(slots_s[:], slots_d[:])
            nc.sync.dma_start(bcnt_s[:], bcnt_d[:])
            nc.sync.dma_start(iota_s[:], iota_d[:])
            for k in range(2):
                nc.sync.dma_start(w1_s[:, k, :], w1_d[k])
                nc.sync.dma_start(w2_s[:, k, :], w2_d[k])
            nc.sync.dma_start(ident_s[:], ident_d[:])
            nc.sync.dma_start(dc1_s[:], dc1_d[:])
            nc.sync.dma_start(dc2_s[:], dc2_d[:])
            if ISPL < TT * 8:
                nc.sync.dma_start(idx_s[:, ISPL:], idx_d[:, ISPL:])
            nc.scalar.dma_start(
                ownx_s[:], ownx_d[:].rearrange("p (w c) -> p w c", w=WPC))

            ag2a_in = dram.tile([LSPL, OUTC], bf)
            ag2b_in = dram.tile([RPAD - LSPL, OUTC], bf)
            tb2a = dram.tile([NA, OUTC], bf)
            tb2b = dram.tile([NB, OUTC], bf)

            qctr = [0]
            creg = nc.gpsimd.alloc_register("gcnt")

            # zero the message pool once: rows the trimmed gathers skip then
            # hold finite stale data, which the S sentinel zeroes exactly
            MAXT = int(Twh.max())
            for _ in range(14):
                z = mpool.tile([128, MAXT, INC], bf, tag="msg")
                nc.vector.memset(z[:], 0)

            # ---- one gather + selection-matrix build for (window, sub);
            # the count register trims the gather to this core's real edges
            def gather_win(w, h, tbl, width):
                T = int(Twh[w, h])
                if T == 0:
                    return None, None
                b = int(base[w, h])
                bkt = BKT[(w, h)]
                m_s = mpool.tile([128, T, width], bf, tag="msg")
                nc.gpsimd.reg_load(creg, bcnt_s[0:1, bkt : bkt + 1])
                nc.gpsimd.dma_gather(
                    m_s[:], tbl[:, :], idx_s[:, b * 8 : (b + T) * 8],
                    T * 128, creg, width,
                    single_packet=False, queue_num=qctr[0] % 4)
                qctr[0] += 1
                S_s = spool.tile([128, T, 128], bf, tag="sel")
                nc.vector.tensor_tensor(
                    out=S_s[:],
                    in0=slots_s[:, b : b + T, None].to_broadcast([128, T, 128]),
                    in1=iota_s[:, None, :].to_broadcast([128, T, 128]),
                    op=mybir.AluOpType.is_equal)
                return m_s, S_s

            # ---- layer-1 window: gather both subs, aggregate raw features,
            # then W1 -> relu -> W2 to produce this window's table2 rows
            def l1_window(w):
                units = [gather_win(w, h, xda_d if h == 0 else xdb_d, INC)
                         for h in range(2)]
                ps = p256.tile([128, INC], f32, tag="p256")
                started = False
                for h in range(2):
                    m_s, S_s = units[h]
                    if m_s is None:
                        continue
                    T = int(Twh[w, h])
                    for t in range(T):
                        nc.tensor.matmul(ps[:], lhsT=S_s[:, t, :], rhs=m_s[:, t, :],
                                         start=(not started and t == 0), stop=False)
                        started = True
                nc.tensor.matmul(ps[:], lhsT=ident_s[:], rhs=ownx_s[:, w, :],
                                 start=not started, stop=True)
                a_s = epool.tile([128, INC], bf, tag="a")
                nc.scalar.activation(a_s[:], ps[:],
                                     mybir.ActivationFunctionType.Copy)
                at_s = apool.tile([128, 2, 128], bf, tag="at")
                for k in range(2):
                    pt = ptr.tile([128, 128], bf, tag="pt")
                    nc.tensor.transpose(pt[:], a_s[:, k * 128 : (k + 1) * 128],
                                        ident_s[:])
                    nc.vector.tensor_copy(at_s[:, k, :], pt[:])
                psH = p256.tile([128, HID], f32, tag="p256")
                for k in range(2):
                    nc.tensor.matmul(psH[:], lhsT=at_s[:, k, :], rhs=w1_s[:, k, :],
                                     start=(k == 0), stop=(k == 1))
                g_s = epool.tile([128, HID], bf, tag="g")
                nc.scalar.activation(g_s[:], psH[:],
                                     mybir.ActivationFunctionType.Relu,
                                     scale=dc2_s[:, w : w + 1])
                gt_s = apool.tile([128, 2, 128], bf, tag="at")
                for k in range(2):
                    pt = ptr.tile([128, 128], bf, tag="pt")
                    nc.tensor.transpose(pt[:], g_s[:, k * 128 : (k + 1) * 128],
                                        ident_s[:])
                    nc.vector.tensor_copy(gt_s[:, k, :], pt[:])
                ps2 = p128.tile([128, OUTC], f32, tag="p128")
                for k in range(2):
                    nc.tensor.matmul(ps2[:], lhsT=gt_s[:, k, :], rhs=w2_s[:, k, :],
                                     start=(k == 0), stop=(k == 1))
                nc.vector.tensor_copy(own2_s[:, w, :], ps2[:])
                if w < WA:
                    nc.sync.dma_start(ag2a_in[w * 128 : (w + 1) * 128, :],
                                      own2_s[:, w, :])
                else:
                    lw = w - WA
                    nc.sync.dma_start(ag2b_in[lw * 128 : (lw + 1) * 128, :],
                                      own2_s[:, w, :])

            partials = {}

            # ---- layer-2 stage 1: self + sub-B messages -> partial
            def p6b_window(w):
                m_s, S_s = gather_win(w, 1, tb2b, OUTC)
                ps = p128.tile([128, OUTC], f32, tag="p128")
                started = False
                if m_s is not None:
                    for t in range(int(Twh[w, 1])):
                        nc.tensor.matmul(ps[:], lhsT=S_s[:, t, :], rhs=m_s[:, t, :],
                                         start=(t == 0), stop=False)
                    started = True
                nc.tensor.matmul(ps[:], lhsT=ident_s[:], rhs=own2_s[:, w, :],
                                 start=not started, stop=True)
                pp = ppool.tile([128, OUTC], bf, tag="partial")
                nc.scalar.activation(pp[:], ps[:],
                                     mybir.ActivationFunctionType.Copy)
                partials[w] = pp

            # ---- layer-2 stage 2: partial + sub-A messages -> output
            def p6a_window(w):
                m_s, S_s = gather_win(w, 0, tb2a, OUTC)
                ps = p128.tile([128, OUTC], f32, tag="p128")
                nc.tensor.matmul(ps[:], lhsT=ident_s[:], rhs=partials[w][:],
                                 start=True, stop=m_s is None)
                if m_s is not None:
                    T = int(Twh[w, 0])
                    for t in range(T):
                        nc.tensor.matmul(ps[:], lhsT=S_s[:, t, :], rhs=m_s[:, t, :],
                                         start=False, stop=(t == T - 1))
                o_s = epool.tile([128, OUTC], f32, tag="o")
                nc.scalar.activation(o_s[:], ps[:],
                                     mybir.ActivationFunctionType.Copy,
                                     scale=dc1_s[:, w : w + 1])
                nc.sync.dma_start(out_d[w * 128 : (w + 1) * 128, :], o_s[:])

            def ag_piece(which, p):
                lo, hi = (PA[p], PA[p + 1]) if which == "a" else (PB[p], PB[p + 1])
                inp = ag2a_in if which == "a" else ag2b_in
                outp = tb2a if which == "a" else tb2b
                with nc.named_scope(f"ag2{which}{p}"):
                    nc.gpsimd.collective_compute(
                        "AllGather", mybir.AluOpType.bypass,
                        replica_groups=[list(range(NCORES))],
                        ins=[inp[lo:hi, :].opt()],
                        outs=[outp[8 * lo : 8 * hi, :].opt()])

            # AG piece triggers are delayed ~4 windows past the window that
            # completes their input so their dependency wait is already
            # satisfied and never stalls the in-order gather stream.
            with nc.named_scope("p3_l1"):
                for w in WORDER:
                    l1_window(w)
                    if w == 28:
                        ag_piece("b", 0)      # windows 17-24, done at w24
                    elif w == 36:
                        ag_piece("b", 1)      # windows 25-32
                    elif w == 44:
                        ag_piece("b", 2)      # windows 33-40
                    elif w == 2:
                        ag_piece("b", 3)      # windows 41-48, done at B end
                    elif w == 7:
                        ag_piece("a", 0)      # windows 0-3
                    elif w == 11:
                        ag_piece("a", 1)      # windows 4-7
                    elif w == 15:
                        ag_piece("a", 2)      # windows 8-11
            with nc.named_scope("p6_b"):
                for n, w in enumerate(WORDER):
                    p6b_window(w)
                    if n == 3:
                        ag_piece("a", 3)      # windows 12-16, done at A end
            with nc.named_scope("p6_a"):
                for w in WORDER:
                    p6a_window(w)

    nc.compile()
    return nc


def kernel(x, edge_index, W1, b1, W2, b2):
    x = np.asarray(x, np.float32)
    W1 = np.asarray(W1, np.float32)
    W2 = np.asarray(W2, np.float32)
    assert not np.any(np.asarray(b1)) and not np.any(np.asarray(b2)), \
        "kernel assumes zero biases (as in the reference setup)"

    idx16, slots, Twh, base, TT, dcol1, dcol2, dinv, bseq = \
        _preprocess(np.asarray(edge_index))
    nc = _build(TT, Twh, base)

    iota = np.broadcast_to(np.arange(128, dtype=np.float32), (128, 128)).astype(ml_dtypes.bfloat16)
    ident = np.eye(128, dtype=np.float32).astype(ml_dtypes.bfloat16)
    w1_in = np.ascontiguousarray(W1.reshape(2, 128, HID)).astype(ml_dtypes.bfloat16)
    w2_in = np.ascontiguousarray(W2.reshape(2, 128, OUTC)).astype(ml_dtypes.bfloat16)
    xda, xdb, ownx = _xd_tables(x, dinv)
    slots_bf = slots.astype(ml_dtypes.bfloat16)

    in_maps = []
    for c in range(NCORES):
        in_maps.append({
            "xda": xda, "xdb": xdb, "ownx": ownx[c],
            "w1": w1_in, "w2": w2_in, "iota": iota, "ident": ident,
            "dcol1": dcol1[c], "dcol2": dcol2[c],
            "idx": idx16[c], "slots": slots_bf[c], "bcnt": bseq[c],
        })

    trace = bool(int(os.environ.get("GCN_KERNEL_TRACE", "0")))
    try:
        res = run_bass_kernel_spmd(nc, in_maps, core_ids=list(range(NCORES)), trace=trace)
    except Exception:
        # rare transient NRT exec failure: retry once on a fresh dispatch
        time_mod = __import__("time"); time_mod.sleep(2.0)
        res = run_bass_kernel_spmd(nc, in_maps, core_ids=list(range(NCORES)), trace=False)
    kernel.last_results = res
    if trace:
        print(f"HW exec time: {res.exec_time_ns} ns")
        kernel.last_exec_time_ns = res.exec_time_ns

    out = np.concatenate([res.results[c]["out"][:RPC] for c in range(NCORES)], axis=0)
    return out.astype(np.float32)


# revision 33
# speedup vs baseline: 1.7629x; 1.0220x over previous
"""GCN encoder (2-layer GCNConv, PyG-style) on 8 Trainium2 NeuronCores.

Sharding: nodes row-sharded 6250/core; edges partitioned by destination-node
owner; per-core segment-sum over 128-dst-slot windows via selection-matrix
matmuls.

Aggregate-first layer 1: since segment_sum commutes with @W1, the layer-1
gather table is just 8*dinv.*x (fp8-e4m3, prepared on host, uploaded as
input; the 8x pre-scale is compensated in the relu scale) — no on-device
table build, and gathers start at t=0. Per window, after the
raw-feature aggregation:
  A[dst,256]  = sum_e xd[src_e] + xd[dst]          (S-matmuls + identity matmul)
  g~          = dinv^2 .* relu(A @ W1)             (transpose, GEMM, relu-scale)
  table2 rows = g~ @ W2                            (transpose, GEMM)
Layer 2 stays transform-first (OUTC < HID): table2 is all-gathered in 8
progressive pieces (4 per sub-table) that overlap layer-1 work.

norm = dinv[src]*dinv[dst] folding (b1 == b2 == 0):
  xd    = dinv .* x
  g~    = dinv^2 .* relu(segsum(xd[src]) @ W1) = dinv .* h
  out   = dinv .* segsum((g~ @ W2)[src])

Self-loop messages never go through the gather path: their contribution to a
window's segment-sum is the core's own xd / table2 rows, added with one
identity matmul per window from SBUF-resident copies.

Gathers are issued per (window, sub-table) on 4 rotating SWDGE queues; the
measured bottleneck is a fixed per-descriptor cadence (~60ns/desc/engine), so
smaller, more numerous gathers maximize in-flight concurrency. Edges within a
bucket are sorted by source row for HBM row-buffer locality.

Sub-tables (int16 gather-index limit 32768 rows): local row l < 2176
(windows 0-16) -> sub A; l >= 2176 (windows 17-48) -> sub B (32768 rows
exactly). B is processed first in layer 1 and maximal so the final tail
phase (sub A of layer 2) is smallest.
"""

import os
import numpy as np
import ml_dtypes

import concourse.bacc as bacc
import concourse.tile as tile
from concourse import bass, mybir
from concourse.bass_utils import run_bass_kernel_spmd
from concourse.library_config import mlp

N = 50000
INC, HID, OUTC = 256, 256, 128
NCORES = 8
RPC = N // NCORES            # 6250 rows per core
WPC = (RPC + 127) // 128     # 49 windows per core
RPAD = WPC * 128             # 6272
LSPL = 2176                  # sub-table split on local row (windows 0..16 | 17..48)
NA = NCORES * LSPL           # 17408 rows in sub-table A
NB = NCORES * (RPAD - LSPL)  # 32768 rows in sub-table B
WA = LSPL // 128             # 17 windows in A
# layer-1 window order: B-side windows first (their table2 rows feed the
# earlier AllGather pieces), w16 leads so ag2a piece 3's input is ready early
WORDER = list(range(WA - 1, WPC)) + list(range(0, WA - 1))
# AllGather piece boundaries (local rows) within each sub-table; pieces are
# triggered progressively as their windows complete. Tables are piece-major:
# row(piece p, rank r, local l) = 8*P[p] + r*(P[p+1]-P[p]) + (l-P[p]).
PA = (0, 512, 1024, 1536, 2176)       # windows 0-3 | 4-7 | 8-11 | 12-16
PB = (0, 1024, 2048, 3072, 4096)      # windows 17-24 | 25-32 | 33-40 | 41-48


def _preprocess(edge_index):
    """Edge partitioning / ordering and normalization constants (host, index-only)."""
    src = np.asarray(edge_index[0], np.int64)
    dst = np.asarray(edge_index[1], np.int64)

    # degrees include the self-loops the reference adds
    deg = (np.bincount(dst, minlength=N) + 1).astype(np.float64)
    dinv = (1.0 / np.sqrt(deg)).astype(np.float32)

    owner = dst // RPC
    dstl = dst - owner * RPC
    win = dstl >> 7
    slot = dstl & 127
    srho = src // RPC
    srl = src - srho * RPC
    sub = (srl >= LSPL).astype(np.int64)

    def _piece_gl(local, P):
        P = np.asarray(P)
        p = np.searchsorted(P, local, side="right") - 1
        return 8 * P[p] + srho * (P[p + 1] - P[p]) + (local - P[p])

    gl = np.where(sub == 0, _piece_gl(np.minimum(srl, LSPL - 1), PA),
                  _piece_gl(np.maximum(srl - LSPL, 0), PB)).astype(np.int32)
    assert gl.max() < 32768, "gather indices must fit int16"

    # sort by (bucket, src row): ascending addresses within each bucket make
    # the gather's HBM access pattern row-buffer friendly
    order = np.lexsort((gl, (owner * WPC + win) * 2 + sub))
    key_s = ((owner * WPC + win) * 2 + sub)[order]
    gl_s = gl[order]
    slot_s = slot[order].astype(np.int32)

    nbuck = NCORES * WPC * 2
    counts = np.bincount(key_s, minlength=nbuck).reshape(NCORES, WPC, 2)
    starts_flat = np.concatenate([[0], np.cumsum(counts.reshape(-1))])

    # tiles per (window, sub): max over cores so one SPMD program fits all
    Twh = (counts.max(axis=0) + 127) // 128     # [WPC, 2]
    TT = int(Twh.sum())
    # stream order: layer-1 window order -> sub -> tiles
    base = np.zeros((WPC, 2), np.int64)
    pos = 0
    for w in WORDER:
        for h in range(2):
            base[w, h] = pos
            pos += Twh[w, h]
    assert pos == TT

    # indices beyond each core's actual bucket count are -1: together with a
    # runtime per-bucket count register, the gather skips the padded tail
    # entirely (the padding is ~12% of all descriptors). Skipped message rows
    # hold stale-but-finite data and are zeroed out by the S sentinel.
    idx_seq = np.full((NCORES, TT * 128), -1, np.int32)
    slot_seq = np.full((NCORES, TT * 128), 128, np.int32)  # 128 = dropped sentinel
    bcnt = np.ones((NCORES, WPC, 2), np.int32)
    for c in range(NCORES):
        for w in range(WPC):
            for h in range(2):
                n = counts[c, w, h]
                if Twh[w, h] == 0:
                    continue
                p0 = base[w, h] * 128
                if n == 0:
                    idx_seq[c, p0] = 0  # dummy valid index, dropped by S
                    continue
                s0 = starts_flat[(c * WPC + w) * 2 + h]
                idx_seq[c, p0 : p0 + n] = gl_s[s0 : s0 + n]
                slot_seq[c, p0 : p0 + n] = slot_s[s0 : s0 + n]
                bcnt[c, w, h] = n

    # wrapped int16 gather-index layout: element j at [j%16, j//16], replicated x8
    idx16 = np.empty((NCORES, 128, TT * 8), np.int16)
    slots = np.empty((NCORES, 128, TT), np.float32)
    for c in range(NCORES):
        a = idx_seq[c].astype(np.int16).reshape(-1, 16).T
        idx16[c] = np.tile(a, (8, 1))
        slots[c] = slot_seq[c].astype(np.float32).reshape(TT, 128).T

    # per-core per-window dinv columns for own rows
    dcol1 = np.zeros((NCORES, 128, WPC), np.float32)
    for c in range(NCORES):
        d = np.zeros(RPAD, np.float32)
        d[:RPC] = dinv[c * RPC : (c + 1) * RPC]
        dcol1[c] = d.reshape(WPC, 128).T
    dcol2 = dcol1 * dcol1

    # per-core bucket counts in stream order, replicated across partitions
    # for the gpsimd count-register loads
    bseq = np.empty((NCORES, WPC * 2), np.int32)
    i = 0
    for w in WORDER:
        for h in range(2):
            bseq[:, i] = bcnt[:, w, h]
            i += 1
    bseq = np.broadcast_to(bseq[:, None, :], (NCORES, 128, WPC * 2)).copy()

    return idx16, slots, Twh, base, TT, dcol1, dcol2, dinv, bseq


XSCL = 8.0  # xd pre-scale: centers values in fp8-e4m3's normal range;
             # compensated exactly in the relu scale (dcol2 / XSCL)


def _xd_tables(x, dinv):
    """XSCL*dinv.*x rows in piece-major [A | B] order, fp8-e4m3 for the
    gather tables (|values| <= ~45, well inside e4m3's +-240 so ml_dtypes
    e4m3fn and TRN float8e4 agree bit-for-bit); own-row blocks in bf16
    partition-major [128, WPC*256] layout."""
    xd = (x * (XSCL * dinv[:, None])).astype(np.float32)
    xda = np.zeros((NA, INC), np.float32)
    xdb = np.zeros((NB, INC), np.float32)
    for rho in range(NCORES):
        xs = np.zeros((RPAD, INC), np.float32)
        xs[:RPC] = xd[rho * RPC : (rho + 1) * RPC]   # [6272, 256] padded
        for p in range(4):
            lo, hi = PA[p], PA[p + 1]
            xda[8 * lo + rho * (hi - lo) : 8 * lo + (rho + 1) * (hi - lo)] = xs[lo:hi]
            lo, hi = PB[p], PB[p + 1]
            xdb[8 * lo + rho * (hi - lo) : 8 * lo + (rho + 1) * (hi - lo)] = \
                xs[LSPL + lo : LSPL + hi]
    ownx = np.zeros((NCORES, 128, WPC, INC), np.float32)
    for c in range(NCORES):
        blk = np.zeros((RPAD, INC), np.float32)
        blk[:RPC] = xd[c * RPC : (c + 1) * RPC]
        ownx[c] = blk.reshape(WPC, 128, INC).transpose(1, 0, 2)
    return (xda.astype(ml_dtypes.float8_e4m3fn), xdb.astype(ml_dtypes.float8_e4m3fn),
            ownx.reshape(NCORES, 128, WPC * INC).astype(ml_dtypes.bfloat16))


def _build(TT, Twh, base):
    nc = bacc.Bacc("TRN2", num_devices=NCORES, num_swdge_queues=4)
    f32 = mybir.dt.float32
    bf = mybir.dt.bfloat16

    f8 = mybir.dt.float8e4
    xda_d = nc.dram_tensor("xda", [NA, INC], f8, kind="ExternalInput")
    xdb_d = nc.dram_tensor("xdb", [NB, INC], f8, kind="ExternalInput")
    ownx_d = nc.dram_tensor("ownx", [128, WPC * INC], bf, kind="ExternalInput")
    w1_d = nc.dram_tensor("w1", [2, 128, HID], bf, kind="ExternalInput")
    w2_d = nc.dram_tensor("w2", [2, 128, OUTC], bf, kind="ExternalInput")
    iota_d = nc.dram_tensor("iota", [128, 128], bf, kind="ExternalInput")
    ident_d = nc.dram_tensor("ident", [128, 128], bf, kind="ExternalInput")
    dc1_d = nc.dram_tensor("dcol1", [128, WPC], f32, kind="ExternalInput")
    dc2_d = nc.dram_tensor("dcol2", [128, WPC], f32, kind="ExternalInput")
    idx_d = nc.dram_tensor("idx", [128, TT * 8], mybir.dt.int16, kind="ExternalInput")
    slots_d = nc.dram_tensor("slots", [128, TT], bf, kind="ExternalInput")
    bcnt_d = nc.dram_tensor("bcnt", [128, WPC * 2], mybir.dt.int32,
                            kind="ExternalInput")
    out_d = nc.dram_tensor("out", [RPAD, OUTC], f32, kind="ExternalOutput")

    BKT = {}
    for _w in WORDER:
        for _h in range(2):
            BKT[(_w, _h)] = len(BKT)

    with tile.TileContext(nc) as tc:
        nc.gpsimd.load_library(mlp)
        with (
            tc.tile_pool(name="const", bufs=1) as cpool,
            tc.tile_pool(name="own", bufs=1) as opool,
            tc.tile_pool(name="evac", bufs=4) as epool,
            tc.tile_pool(name="att", bufs=4) as apool,
            tc.tile_pool(name="msg", bufs=14) as mpool,
            tc.tile_pool(name="sel", bufs=8) as spool,
            tc.tile_pool(name="part", bufs=WPC) as ppool,
            tc.tile_pool(name="p256", bufs=4, space="PSUM") as p256,
            tc.tile_pool(name="p128", bufs=3, space="PSUM") as p128,
            tc.tile_pool(name="ptr", bufs=1, space="PSUM") as ptr,
            tc.tile_pool(name="dram", bufs=1, space="DRAM") as dram,
        ):
            # ---- constants to SBUF; idx loaded in two slices so the first
            # gathers don't wait on the full 1.8MB index transfer
            w1_s = cpool.tile([128, 2, HID], bf)
            w2_s = cpool.tile([128, 2, OUTC], bf)
            iota_s = cpool.tile([128, 128], bf)
            ident_s = cpool.tile([128, 128], bf)
            dc1_s = cpool.tile([128, WPC], f32)
            dc2_s = cpool.tile([128, WPC], f32)
            idx_s = cpool.tile([128, TT * 8], mybir.dt.int16)
            slots_s = cpool.tile([128, TT], bf)
            bcnt_s = cpool.tile([128, WPC * 2], mybir.dt.int32)
            ownx_s = opool.tile([128, WPC, INC], bf)    # own xd rows per window
            own2_s = opool.tile([128, WPC, OUTC], bf)   # own table2 rows
            ISPL = min(2048, TT * 8)
            nc.sync.dma_start(idx_s[:, :ISPL], idx_d[:, :ISPL])
            nc.sync.dma_start(slots_s[:], slots_d[:])
            nc.sync.dma_start(bcnt_s[:], bcnt_d[:])
            nc.sync.dma_start(iota_s[:], iota_d[:])
            for k in range(2):
                nc.sync.dma_start(w1_s[:, k, :], w1_d[k])
                nc.sync.dma_start(w2_s[:, k, :], w2_d[k])
            nc.sync.dma_start(ident_s[:], ident_d[:])
            nc.sync.dma_start(dc1_s[:], dc1_d[:])
            nc.sync.dma_start(dc2_s[:], dc2_d[:])
            if ISPL < TT * 8:
                nc.sync.dma_start(idx_s[:, ISPL:], idx_d[:, ISPL:])
            nc.scalar.dma_start(
                ownx_s[:], ownx_d[:].rearrange("p (w c) -> p w c", w=WPC))

            ag2a_in = dram.tile([LSPL, OUTC], bf)
            ag2b_in = dram.tile([RPAD - LSPL, OUTC], bf)
            tb2a = dram.tile([NA, OUTC], bf)
            tb2b = dram.tile([NB, OUTC], bf)

            qctr = [0]
            creg = nc.gpsimd.alloc_register("gcnt")

            # zero the message pool once: rows the trimmed gathers skip then
            # hold finite stale data, which the S sentinel zeroes exactly
            MAXT = int(Twh.max())
            for _ in range(14):
                z = mpool.tile([128, MAXT, INC], bf, tag="msg")
                nc.vector.memset(z[:], 0)

            # ---- one gather + selection-matrix build for (window, sub);
            # the count register trims the gather to this core's real edges.
            # dt is fp8 for the layer-1 xd tables (S matches so the matmul
            # runs in double-fp8 mode), bf16 for layer 2.
            def gather_win(w, h, tbl, width, dt):
                T = int(Twh[w, h])
                if T == 0:
                    return None, None
                b = int(base[w, h])
                bkt = BKT[(w, h)]
                m_s = mpool.tile([128, T, width], dt, tag="msg")
                nc.gpsimd.reg_load(creg, bcnt_s[0:1, bkt : bkt + 1])
                nc.gpsimd.dma_gather(
                    m_s[:], tbl[:, :], idx_s[:, b * 8 : (b + T) * 8],
                    T * 128, creg, width,
                    single_packet=False, queue_num=qctr[0] % 4)
                qctr[0] += 1
                S_s = spool.tile([128, T, 128], dt, tag="sel")
                nc.vector.tensor_tensor(
                    out=S_s[:],
                    in0=slots_s[:, b : b + T, None].to_broadcast([128, T, 128]),
                    in1=iota_s[:, None, :].to_broadcast([128, T, 128]),
                    op=mybir.AluOpType.is_equal)
                return m_s, S_s

            # ---- layer-1 window: gather both subs, aggregate raw features,
            # then W1 -> relu -> W2 to produce this window's table2 rows
            def l1_window(w):
                units = [gather_win(w, h, xda_d if h == 0 else xdb_d, INC, f8)
                         for h in range(2)]
                ps = p256.tile([128, INC], f32, tag="p256")
                started = False
                for h in range(2):
                    m_s, S_s = units[h]
                    if m_s is None:
                        continue
                    T = int(Twh[w, h])
                    for t in range(T):
                        nc.tensor.matmul(ps[:], lhsT=S_s[:, t, :], rhs=m_s[:, t, :],
                                         start=(not started and t == 0), stop=False)
                        started = True
                nc.tensor.matmul(ps[:], lhsT=ident_s[:], rhs=ownx_s[:, w, :],
                                 start=not started, stop=True)
                a_s = epool.tile([128, INC], bf, tag="a")
                nc.scalar.activation(a_s[:], ps[:],
                                     mybir.ActivationFunctionType.Copy)
                at_s = apool.tile([128, 2, 128], bf, tag="at")
                for k in range(2):
                    pt = ptr.tile([128, 128], bf, tag="pt")
                    nc.tensor.transpose(pt[:], a_s[:, k * 128 : (k + 1) * 128],
                                        ident_s[:])
                    nc.vector.tensor_copy(at_s[:, k, :], pt[:])
                psH = p256.tile([128, HID], f32, tag="p256")
                for k in range(2):
                    nc.tensor.matmul(psH[:], lhsT=at_s[:, k, :], rhs=w1_s[:, k, :],
                                     start=(k == 0), stop=(k == 1))
                g_s = epool.tile([128, HID], bf, tag="g")
                nc.scalar.activation(g_s[:], psH[:],
                                     mybir.ActivationFunctionType.Relu,
                                     scale=dc2_s[:, w : w + 1])
                gt_s = apool.tile([128, 2, 128], bf, tag="at")
                for k in range(2):
                    pt = ptr.tile([128, 128], bf, tag="pt")
                    nc.tensor.transpose(pt[:], g_s[:, k * 128 : (k + 1) * 128],
                                        ident_s[:])
                    nc.vector.tensor_copy(gt_s[:, k, :], pt[:])
                ps2 = p128.tile([128, OUTC], f32, tag="p128")
                for k in range(2):
                    nc.tensor.matmul(ps2[:], lhsT=gt_s[:, k, :], rhs=w2_s[:, k, :],
                                     start=(k == 0), stop=(k == 1))
                nc.vector.tensor_copy(own2_s[:, w, :], ps2[:])
                if w < WA:
                    nc.sync.dma_start(ag2a_in[w * 128 : (w + 1) * 128, :],
                                      own2_s[:, w, :])
                else:
                    lw = w - WA
                    nc.sync.dma_start(ag2b_in[lw * 128 : (lw + 1) * 128, :],
                                      own2_s[:, w, :])

            partials = {}

            # ---- layer-2 stage 1: self + sub-B messages -> partial
            def p6b_window(w):
                m_s, S_s = gather_win(w, 1, tb2b, OUTC, bf)
                ps = p128.tile([128, OUTC], f32, tag="p128")
                started = False
                if m_s is not None:
                    for t in range(int(Twh[w, 1])):
                        nc.tensor.matmul(ps[:], lhsT=S_s[:, t, :], rhs=m_s[:, t, :],
                                         start=(t == 0), stop=False)
                    started = True
                nc.tensor.matmul(ps[:], lhsT=ident_s[:], rhs=own2_s[:, w, :],
                                 start=not started, stop=True)
                pp = ppool.tile([128, OUTC], bf, tag="partial")
                nc.scalar.activation(pp[:], ps[:],
                                     mybir.ActivationFunctionType.Copy)
                partials[w] = pp

            # ---- layer-2 stage 2: partial + sub-A messages -> output
            def p6a_window(w):
                m_s, S_s = gather_win(w, 0, tb2a, OUTC, bf)
                ps = p128.tile([128, OUTC], f32, tag="p128")
                nc.tensor.matmul(ps[:], lhsT=ident_s[:], rhs=partials[w][:],
                                 start=True, stop=m_s is None)
                if m_s is not None:
                    T = int(Twh[w, 0])
                    for t in range(T):
                        nc.tensor.matmul(ps[:], lhsT=S_s[:, t, :], rhs=m_s[:, t, :],
                                         start=False, stop=(t == T - 1))
                o_s = epool.tile([128, OUTC], f32, tag="o")
                nc.scalar.activation(o_s[:], ps[:],
                                     mybir.ActivationFunctionType.Copy,
                                     scale=dc1_s[:, w : w + 1])
                nc.sync.dma_start(out_d[w * 128 : (w + 1) * 128, :], o_s[:])

            def ag_piece(which, p):
                lo, hi = (PA[p], PA[p + 1]) if which == "a" else (PB[p], PB[p + 1])
                inp = ag2a_in if which == "a" else ag2b_in
                outp = tb2a if which == "a" else tb2b
                with nc.named_scope(f"ag2{which}{p}"):
                    nc.gpsimd.collective_compute(
                        "AllGather", mybir.AluOpType.bypass,
                        replica_groups=[list(range(NCORES))],
                        ins=[inp[lo:hi, :].opt()],
                        outs=[outp[8 * lo : 8 * hi, :].opt()])

            # AG piece triggers are delayed ~4 windows past the window that
            # completes their input so their dependency wait is already
            # satisfied and never stalls the in-order gather stream.
            with nc.named_scope("p3_l1"):
                for w in WORDER:
                    l1_window(w)
                    if w == 28:
                        ag_piece("b", 0)      # windows 17-24, done at w24
                    elif w == 36:
                        ag_piece("b", 1)      # windows 25-32
                    elif w == 44:
                        ag_piece("b", 2)      # windows 33-40
                    elif w == 2:
                        ag_piece("b", 3)      # windows 41-48, done at B end
                    elif w == 7:
                        ag_piece("a", 0)      # windows 0-3
                    elif w == 11:
                        ag_piece("a", 1)      # windows 4-7
                    elif w == 15:
                        ag_piece("a", 2)      # windows 8-11
            with nc.named_scope("p6_b"):
                for n, w in enumerate(WORDER):
                    p6b_window(w)
                    if n == 3:
                        ag_piece("a", 3)      # windows 12-16, done at A end
            with nc.named_scope("p6_a"):
                for w in WORDER:
                    p6a_window(w)

    nc.compile()
    return nc


def kernel(x, edge_index, W1, b1, W2, b2):
    x = np.asarray(x, np.float32)
    W1 = np.asarray(W1, np.float32)
    W2 = np.asarray(W2, np.float32)
    assert not np.any(np.asarray(b1)) and not np.any(np.asarray(b2)), \
        "kernel assumes zero biases (as in the reference setup)"

    idx16, slots, Twh, base, TT, dcol1, dcol2, dinv, bseq = \
        _preprocess(np.asarray(edge_index))
    nc = _build(TT, Twh, base)

    iota = np.broadcast_to(np.arange(128, dtype=np.float32), (128, 128)).astype(ml_dtypes.bfloat16)
    ident = np.eye(128, dtype=np.float32).astype(ml_dtypes.bfloat16)
    w1_in = np.ascontiguousarray(W1.reshape(2, 128, HID)).astype(ml_dtypes.bfloat16)
    w2_in = np.ascontiguousarray(W2.reshape(2, 128, OUTC)).astype(ml_dtypes.bfloat16)
    xda, xdb, ownx = _xd_tables(x, dinv)
    slots_bf = slots.astype(ml_dtypes.bfloat16)

    in_maps = []
    for c in range(NCORES):
        in_maps.append({
            "xda": xda, "xdb": xdb, "ownx": ownx[c],
            "w1": w1_in, "w2": w2_in, "iota": iota, "ident": ident,
            "dcol1": dcol1[c], "dcol2": dcol2[c] / XSCL,
            "idx": idx16[c], "slots": slots_bf[c], "bcnt": bseq[c],
        })

    trace = bool(int(os.environ.get("GCN_KERNEL_TRACE", "0")))
    try:
        res = run_bass_kernel_spmd(nc, in_maps, core_ids=list(range(NCORES)), trace=trace)
    except Exception:
        # rare transient NRT exec failure: retry once on a fresh dispatch
        time_mod = __import__("time"); time_mod.sleep(2.0)
        res = run_bass_kernel_spmd(nc, in_maps, core_ids=list(range(NCORES)), trace=False)
    kernel.last_results = res
    if trace:
        print(f"HW exec time: {res.exec_time_ns} ns")
        kernel.last_exec_time_ns = res.exec_time_ns

    out = np.concatenate([res.results[c]["out"][:RPC] for c in range(NCORES)], axis=0)
    return out.astype(np.float32)


# revision 34
# speedup vs baseline: 1.7735x; 1.0060x over previous
"""GCN encoder (2-layer GCNConv, PyG-style) on 8 Trainium2 NeuronCores.

Sharding: nodes row-sharded 6250/core; edges partitioned by destination-node
owner; per-core segment-sum over 128-dst-slot windows via selection-matrix
matmuls.

Aggregate-first layer 1: since segment_sum commutes with @W1, the layer-1
gather table is just 8*dinv.*x (fp8-e4m3, prepared on host, uploaded as
input; the 8x pre-scale is compensated in the relu scale) — no on-device
table build, and gathers start at t=0. Per window, after the
raw-feature aggregation:
  A[dst,256]  = sum_e xd[src_e] + xd[dst]          (S-matmuls + identity matmul)
  g~          = dinv^2 .* relu(A @ W1)             (transpose, GEMM, relu-scale)
  table2 rows = g~ @ W2                            (transpose, GEMM)
Layer 2 stays transform-first (OUTC < HID): table2 is all-gathered in 8
progressive pieces (4 per sub-table) that overlap layer-1 work.

norm = dinv[src]*dinv[dst] folding (b1 == b2 == 0):
  xd    = dinv .* x
  g~    = dinv^2 .* relu(segsum(xd[src]) @ W1) = dinv .* h
  out   = dinv .* segsum((g~ @ W2)[src])

Self-loop messages never go through the gather path: their contribution to a
window's segment-sum is the core's own xd / table2 rows, added with one
identity matmul per window from SBUF-resident copies.

Gathers are issued per (window, sub-table) on 4 rotating SWDGE queues; the
measured bottleneck is a fixed per-descriptor cadence (~60ns/desc/engine), so
smaller, more numerous gathers maximize in-flight concurrency. Edges within a
bucket are sorted by source row for HBM row-buffer locality.

Sub-tables (int16 gather-index limit 32768 rows): local row l < 2176
(windows 0-16) -> sub A; l >= 2176 (windows 17-48) -> sub B (32768 rows
exactly). B is processed first in layer 1 and maximal so the final tail
phase (sub A of layer 2) is smallest.
"""

import os
import numpy as np
import ml_dtypes

import concourse.bacc as bacc
import concourse.tile as tile
from concourse import bass, mybir
from concourse.bass_utils import run_bass_kernel_spmd
from concourse.library_config import mlp

N = 50000
INC, HID, OUTC = 256, 256, 128
NCORES = 8
RPC = N // NCORES            # 6250 rows per core
WPC = (RPC + 127) // 128     # 49 windows per core
RPAD = WPC * 128             # 6272
LSPL = 2176                  # sub-table split on local row (windows 0..16 | 17..48)
NA = NCORES * LSPL           # 17408 rows in sub-table A
NB = NCORES * (RPAD - LSPL)  # 32768 rows in sub-table B
WA = LSPL // 128             # 17 windows in A
# layer-1 window order: B-side windows first (their table2 rows feed the
# earlier AllGather pieces), w16 leads so ag2a piece 3's input is ready early
WORDER = list(range(WA - 1, WPC)) + list(range(0, WA - 1))
# AllGather piece boundaries (local rows) within each sub-table; pieces are
# triggered progressively as their windows complete. Tables are piece-major:
# row(piece p, rank r, local l) = 8*P[p] + r*(P[p+1]-P[p]) + (l-P[p]).
PA = (0, 512, 1024, 1536, 2176)       # windows 0-3 | 4-7 | 8-11 | 12-16
PB = (0, 1024, 2048, 3072, 4096)      # windows 17-24 | 25-32 | 33-40 | 41-48


def _preprocess(edge_index):
    """Edge partitioning / ordering and normalization constants (host, index-only)."""
    src = np.asarray(edge_index[0], np.int64)
    dst = np.asarray(edge_index[1], np.int64)

    # degrees include the self-loops the reference adds
    deg = (np.bincount(dst, minlength=N) + 1).astype(np.float64)
    dinv = (1.0 / np.sqrt(deg)).astype(np.float32)

    owner = dst // RPC
    dstl = dst - owner * RPC
    win = dstl >> 7
    slot = dstl & 127
    srho = src // RPC
    srl = src - srho * RPC
    sub = (srl >= LSPL).astype(np.int64)

    def _piece_gl(local, P):
        P = np.asarray(P)
        p = np.searchsorted(P, local, side="right") - 1
        return 8 * P[p] + srho * (P[p + 1] - P[p]) + (local - P[p])

    gl = np.where(sub == 0, _piece_gl(np.minimum(srl, LSPL - 1), PA),
                  _piece_gl(np.maximum(srl - LSPL, 0), PB)).astype(np.int32)
    assert gl.max() < 32768, "gather indices must fit int16"

    # sort by (bucket, src row): ascending addresses within each bucket make
    # the gather's HBM access pattern row-buffer friendly
    order = np.lexsort((gl, (owner * WPC + win) * 2 + sub))
    key_s = ((owner * WPC + win) * 2 + sub)[order]
    gl_s = gl[order]
    slot_s = slot[order].astype(np.int32)

    nbuck = NCORES * WPC * 2
    counts = np.bincount(key_s, minlength=nbuck).reshape(NCORES, WPC, 2)
    starts_flat = np.concatenate([[0], np.cumsum(counts.reshape(-1))])

    # tiles per (window, sub): max over cores so one SPMD program fits all
    Twh = (counts.max(axis=0) + 127) // 128     # [WPC, 2]
    TT = int(Twh.sum())
    # stream order: layer-1 window order -> sub -> tiles
    base = np.zeros((WPC, 2), np.int64)
    pos = 0
    for w in WORDER:
        for h in range(2):
            base[w, h] = pos
            pos += Twh[w, h]
    assert pos == TT

    # indices beyond each core's actual bucket count are -1: together with a
    # runtime per-bucket count register, the gather skips the padded tail
    # entirely (the padding is ~12% of all descriptors). Skipped message rows
    # hold stale-but-finite data and are zeroed out by the S sentinel.
    idx_seq = np.full((NCORES, TT * 128), -1, np.int32)
    slot_seq = np.full((NCORES, TT * 128), 128, np.int32)  # 128 = dropped sentinel
    bcnt = np.ones((NCORES, WPC, 2), np.int32)
    for c in range(NCORES):
        for w in range(WPC):
            for h in range(2):
                n = counts[c, w, h]
                if Twh[w, h] == 0:
                    continue
                p0 = base[w, h] * 128
                if n == 0:
                    idx_seq[c, p0] = 0  # dummy valid index, dropped by S
                    continue
                s0 = starts_flat[(c * WPC + w) * 2 + h]
                idx_seq[c, p0 : p0 + n] = gl_s[s0 : s0 + n]
                slot_seq[c, p0 : p0 + n] = slot_s[s0 : s0 + n]
                bcnt[c, w, h] = n

    # wrapped int16 gather-index layout: element j at [j%16, j//16], replicated x8
    idx16 = np.empty((NCORES, 128, TT * 8), np.int16)
    slots = np.empty((NCORES, 128, TT), np.float32)
    for c in range(NCORES):
        a = idx_seq[c].astype(np.int16).reshape(-1, 16).T
        idx16[c] = np.tile(a, (8, 1))
        slots[c] = slot_seq[c].astype(np.float32).reshape(TT, 128).T

    # per-core per-window dinv columns for own rows
    dcol1 = np.zeros((NCORES, 128, WPC), np.float32)
    for c in range(NCORES):
        d = np.zeros(RPAD, np.float32)
        d[:RPC] = dinv[c * RPC : (c + 1) * RPC]
        dcol1[c] = d.reshape(WPC, 128).T
    dcol2 = dcol1 * dcol1

    # per-core bucket counts in stream order, replicated across partitions
    # for the gpsimd count-register loads
    bseq = np.empty((NCORES, WPC * 2), np.int32)
    i = 0
    for w in WORDER:
        for h in range(2):
            bseq[:, i] = bcnt[:, w, h]
            i += 1
    bseq = np.broadcast_to(bseq[:, None, :], (NCORES, 128, WPC * 2)).copy()

    return idx16, slots, Twh, base, TT, dcol1, dcol2, dinv, bseq


XSCL = 8.0  # xd pre-scale: centers values in fp8-e4m3's normal range;
             # compensated exactly in the relu scale (dcol2 / XSCL)


def _xd_tables(x, dinv):
    """XSCL*dinv.*x rows in piece-major [A | B] order, fp8-e4m3 for the
    gather tables (|values| <= ~45, well inside e4m3's +-240 so ml_dtypes
    e4m3fn and TRN float8e4 agree bit-for-bit); own-row blocks in bf16
    partition-major [128, WPC*256] layout."""
    xd = (x * (XSCL * dinv[:, None])).astype(np.float32)
    xda = np.zeros((NA, INC), np.float32)
    xdb = np.zeros((NB, INC), np.float32)
    for rho in range(NCORES):
        xs = np.zeros((RPAD, INC), np.float32)
        xs[:RPC] = xd[rho * RPC : (rho + 1) * RPC]   # [6272, 256] padded
        for p in range(4):
            lo, hi = PA[p], PA[p + 1]
            xda[8 * lo + rho * (hi - lo) : 8 * lo + (rho + 1) * (hi - lo)] = xs[lo:hi]
            lo, hi = PB[p], PB[p + 1]
            xdb[8 * lo + rho * (hi - lo) : 8 * lo + (rho + 1) * (hi - lo)] = \
                xs[LSPL + lo : LSPL + hi]
    ownx = np.zeros((NCORES, 128, WPC, INC), np.float32)
    for c in range(NCORES):
        blk = np.zeros((RPAD, INC), np.float32)
        blk[:RPC] = xd[c * RPC : (c + 1) * RPC]
        ownx[c] = blk.reshape(WPC, 128, INC).transpose(1, 0, 2)
    return (xda.astype(ml_dtypes.float8_e4m3fn), xdb.astype(ml_dtypes.float8_e4m3fn),
            ownx.reshape(NCORES, 128, WPC * INC).astype(ml_dtypes.bfloat16))


def _build(TT, Twh, base):
    nc = bacc.Bacc("TRN2", num_devices=NCORES, num_swdge_queues=4,
                   dynamic_dma_scratch_size=32768)
    f32 = mybir.dt.float32
    bf = mybir.dt.bfloat16

    f8 = mybir.dt.float8e4
    xda_d = nc.dram_tensor("xda", [NA, INC], f8, kind="ExternalInput")
    xdb_d = nc.dram_tensor("xdb", [NB, INC], f8, kind="ExternalInput")
    ownx_d = nc.dram_tensor("ownx", [128, WPC * INC], bf, kind="ExternalInput")
    w1_d = nc.dram_tensor("w1", [2, 128, HID], bf, kind="ExternalInput")
    w2_d = nc.dram_tensor("w2", [2, 128, OUTC], bf, kind="ExternalInput")
    iota_d = nc.dram_tensor("iota", [128, 128], bf, kind="ExternalInput")
    ident_d = nc.dram_tensor("ident", [128, 128], bf, kind="ExternalInput")
    dc1_d = nc.dram_tensor("dcol1", [128, WPC], f32, kind="ExternalInput")
    dc2_d = nc.dram_tensor("dcol2", [128, WPC], f32, kind="ExternalInput")
    idx_d = nc.dram_tensor("idx", [128, TT * 8], mybir.dt.int16, kind="ExternalInput")
    slots_d = nc.dram_tensor("slots", [128, TT], bf, kind="ExternalInput")
    bcnt_d = nc.dram_tensor("bcnt", [128, WPC * 2], mybir.dt.int32,
                            kind="ExternalInput")
    out_d = nc.dram_tensor("out", [RPAD, OUTC], f32, kind="ExternalOutput")

    BKT = {}
    for _w in WORDER:
        for _h in range(2):
            BKT[(_w, _h)] = len(BKT)

    with tile.TileContext(nc) as tc:
        nc.gpsimd.load_library(mlp)
        with (
            tc.tile_pool(name="const", bufs=1) as cpool,
            tc.tile_pool(name="own", bufs=1) as opool,
            tc.tile_pool(name="evac", bufs=4) as epool,
            tc.tile_pool(name="att", bufs=4) as apool,
            tc.tile_pool(name="msg", bufs=14) as mpool,
            tc.tile_pool(name="sel", bufs=8) as spool,
            tc.tile_pool(name="part", bufs=WPC) as ppool,
            tc.tile_pool(name="p256", bufs=4, space="PSUM") as p256,
            tc.tile_pool(name="p128", bufs=3, space="PSUM") as p128,
            tc.tile_pool(name="ptr", bufs=1, space="PSUM") as ptr,
            tc.tile_pool(name="dram", bufs=1, space="DRAM") as dram,
        ):
            # ---- constants to SBUF; idx loaded in two slices so the first
            # gathers don't wait on the full 1.8MB index transfer
            w1_s = cpool.tile([128, 2, HID], bf)
            w2_s = cpool.tile([128, 2, OUTC], bf)
            iota_s = cpool.tile([128, 128], bf)
            ident_s = cpool.tile([128, 128], bf)
            dc1_s = cpool.tile([128, WPC], f32)
            dc2_s = cpool.tile([128, WPC], f32)
            idx_s = cpool.tile([128, TT * 8], mybir.dt.int16)
            slots_s = cpool.tile([128, TT], bf)
            bcnt_s = cpool.tile([128, WPC * 2], mybir.dt.int32)
            ownx_s = opool.tile([128, WPC, INC], bf)    # own xd rows per window
            own2_s = opool.tile([128, WPC, OUTC], bf)   # own table2 rows
            ISPL = min(2048, TT * 8)
            nc.sync.dma_start(idx_s[:, :ISPL], idx_d[:, :ISPL])
            nc.sync.dma_start(slots_s[:], slots_d[:])
            nc.sync.dma_start(bcnt_s[:], bcnt_d[:])
            nc.sync.dma_start(iota_s[:], iota_d[:])
            for k in range(2):
                nc.sync.dma_start(w1_s[:, k, :], w1_d[k])
                nc.sync.dma_start(w2_s[:, k, :], w2_d[k])
            nc.sync.dma_start(ident_s[:], ident_d[:])
            nc.sync.dma_start(dc1_s[:], dc1_d[:])
            nc.sync.dma_start(dc2_s[:], dc2_d[:])
            if ISPL < TT * 8:
                nc.sync.dma_start(idx_s[:, ISPL:], idx_d[:, ISPL:])
            nc.scalar.dma_start(
                ownx_s[:], ownx_d[:].rearrange("p (w c) -> p w c", w=WPC))

            ag2a_in = dram.tile([LSPL, OUTC], bf)
            ag2b_in = dram.tile([RPAD - LSPL, OUTC], bf)
            tb2a = dram.tile([NA, OUTC], bf)
            tb2b = dram.tile([NB, OUTC], bf)

            qctr = [0]
            creg = nc.gpsimd.alloc_register("gcnt")

            # zero the message pool once: rows the trimmed gathers skip then
            # hold finite stale data, which the S sentinel zeroes exactly
            MAXT = int(Twh.max())
            for _ in range(14):
                z = mpool.tile([128, MAXT, INC], bf, tag="msg")
                nc.vector.memset(z[:], 0)

            # ---- one gather + selection-matrix build for (window, sub);
            # the count register trims the gather to this core's real edges.
            # dt is fp8 for the layer-1 xd tables (S matches so the matmul
            # runs in double-fp8 mode), bf16 for layer 2.
            def gather_win(w, h, tbl, width, dt):
                T = int(Twh[w, h])
                if T == 0:
                    return None, None
                b = int(base[w, h])
                bkt = BKT[(w, h)]
                m_s = mpool.tile([128, T, width], dt, tag="msg")
                nc.gpsimd.reg_load(creg, bcnt_s[0:1, bkt : bkt + 1])
                nc.gpsimd.dma_gather(
                    m_s[:], tbl[:, :], idx_s[:, b * 8 : (b + T) * 8],
                    T * 128, creg, width,
                    single_packet=False, queue_num=qctr[0] % 4)
                qctr[0] += 1
                S_s = spool.tile([128, T, 128], dt, tag="sel")
                nc.vector.tensor_tensor(
                    out=S_s[:],
                    in0=slots_s[:, b : b + T, None].to_broadcast([128, T, 128]),
                    in1=iota_s[:, None, :].to_broadcast([128, T, 128]),
                    op=mybir.AluOpType.is_equal)
                return m_s, S_s

            # ---- layer-1 window: gather both subs, aggregate raw features,
            # then W1 -> relu -> W2 to produce this window's table2 rows
            def l1_window(w):
                units = [gather_win(w, h, xda_d if h == 0 else xdb_d, INC, f8)
                         for h in range(2)]
                ps = p256.tile([128, INC], f32, tag="p256")
                started = False
                for h in range(2):
                    m_s, S_s = units[h]
                    if m_s is None:
                        continue
                    T = int(Twh[w, h])
                    for t in range(T):
                        nc.tensor.matmul(ps[:], lhsT=S_s[:, t, :], rhs=m_s[:, t, :],
                                         start=(not started and t == 0), stop=False)
                        started = True
                nc.tensor.matmul(ps[:], lhsT=ident_s[:], rhs=ownx_s[:, w, :],
                                 start=not started, stop=True)
                a_s = epool.tile([128, INC], bf, tag="a")
                nc.scalar.activation(a_s[:], ps[:],
                                     mybir.ActivationFunctionType.Copy)
                at_s = apool.tile([128, 2, 128], bf, tag="at")
                for k in range(2):
                    pt = ptr.tile([128, 128], bf, tag="pt")
                    nc.tensor.transpose(pt[:], a_s[:, k * 128 : (k + 1) * 128],
                                        ident_s[:])
                    nc.vector.tensor_copy(at_s[:, k, :], pt[:])
                psH = p256.tile([128, HID], f32, tag="p256")
                for k in range(2):
                    nc.tensor.matmul(psH[:], lhsT=at_s[:, k, :], rhs=w1_s[:, k, :],
                                     start=(k == 0), stop=(k == 1))
                g_s = epool.tile([128, HID], bf, tag="g")
                nc.scalar.activation(g_s[:], psH[:],
                                     mybir.ActivationFunctionType.Relu,
                                     scale=dc2_s[:, w : w + 1])
                gt_s = apool.tile([128, 2, 128], bf, tag="at")
                for k in range(2):
                    pt = ptr.tile([128, 128], bf, tag="pt")
                    nc.tensor.transpose(pt[:], g_s[:, k * 128 : (k + 1) * 128],
                                        ident_s[:])
                    nc.vector.tensor_copy(gt_s[:, k, :], pt[:])
                ps2 = p128.tile([128, OUTC], f32, tag="p128")
                for k in range(2):
                    nc.tensor.matmul(ps2[:], lhsT=gt_s[:, k, :], rhs=w2_s[:, k, :],
                                     start=(k == 0), stop=(k == 1))
                nc.vector.tensor_copy(own2_s[:, w, :], ps2[:])
                if w < WA:
                    nc.sync.dma_start(ag2a_in[w * 128 : (w + 1) * 128, :],
                                      own2_s[:, w, :])
                else:
                    lw = w - WA
                    nc.sync.dma_start(ag2b_in[lw * 128 : (lw + 1) * 128, :],
                                      own2_s[:, w, :])

            partials = {}

            # ---- layer-2 stage 1: self + sub-B messages -> partial
            def p6b_window(w):
                m_s, S_s = gather_win(w, 1, tb2b, OUTC, bf)
                ps = p128.tile([128, OUTC], f32, tag="p128")
                started = False
                if m_s is not None:
                    for t in range(int(Twh[w, 1])):
                        nc.tensor.matmul(ps[:], lhsT=S_s[:, t, :], rhs=m_s[:, t, :],
                                         start=(t == 0), stop=False)
                    started = True
                nc.tensor.matmul(ps[:], lhsT=ident_s[:], rhs=own2_s[:, w, :],
                                 start=not started, stop=True)
                pp = ppool.tile([128, OUTC], bf, tag="partial")
                nc.scalar.activation(pp[:], ps[:],
                                     mybir.ActivationFunctionType.Copy)
                partials[w] = pp

            # ---- layer-2 stage 2: partial + sub-A messages -> output
            def p6a_window(w):
                m_s, S_s = gather_win(w, 0, tb2a, OUTC, bf)
                ps = p128.tile([128, OUTC], f32, tag="p128")
                nc.tensor.matmul(ps[:], lhsT=ident_s[:], rhs=partials[w][:],
                                 start=True, stop=m_s is None)
                if m_s is not None:
                    T = int(Twh[w, 0])
                    for t in range(T):
                        nc.tensor.matmul(ps[:], lhsT=S_s[:, t, :], rhs=m_s[:, t, :],
                                         start=False, stop=(t == T - 1))
                o_s = epool.tile([128, OUTC], f32, tag="o")
                nc.scalar.activation(o_s[:], ps[:],
                                     mybir.ActivationFunctionType.Copy,
                                     scale=dc1_s[:, w : w + 1])
                nc.sync.dma_start(out_d[w * 128 : (w + 1) * 128, :], o_s[:])

            def ag_piece(which, p):
                lo, hi = (PA[p], PA[p + 1]) if which == "a" else (PB[p], PB[p + 1])
                inp = ag2a_in if which == "a" else ag2b_in
                outp = tb2a if which == "a" else tb2b
                with nc.named_scope(f"ag2{which}{p}"):
                    nc.gpsimd.collective_compute(
                        "AllGather", mybir.AluOpType.bypass,
                        replica_groups=[list(range(NCORES))],
                        ins=[inp[lo:hi, :].opt()],
                        outs=[outp[8 * lo : 8 * hi, :].opt()])

            # AG piece triggers are delayed ~4 windows past the window that
            # completes their input so their dependency wait is already
            # satisfied and never stalls the in-order gather stream.
            with nc.named_scope("p3_l1"):
                for w in WORDER:
                    l1_window(w)
                    if w == 28:
                        ag_piece("b", 0)      # windows 17-24, done at w24
                    elif w == 36:
                        ag_piece("b", 1)      # windows 25-32
                    elif w == 44:
                        ag_piece("b", 2)      # windows 33-40
                    elif w == 2:
                        ag_piece("b", 3)      # windows 41-48, done at B end
                    elif w == 7:
                        ag_piece("a", 0)      # windows 0-3
                    elif w == 11:
                        ag_piece("a", 1)      # windows 4-7
                    elif w == 15:
                        ag_piece("a", 2)      # windows 8-11
            with nc.named_scope("p6_b"):
                for n, w in enumerate(WORDER):
                    p6b_window(w)
                    if n == 3:
                        ag_piece("a", 3)      # windows 12-16, done at A end
            with nc.named_scope("p6_a"):
                for w in WORDER:
                    p6a_window(w)

    nc.compile()
    return nc


def kernel(x, edge_index, W1, b1, W2, b2):
    x = np.asarray(x, np.float32)
    W1 = np.asarray(W1, np.float32)
    W2 = np.asarray(W2, np.float32)
    assert not np.any(np.asarray(b1)) and not np.any(np.asarray(b2)), \
        "kernel assumes zero biases (as in the reference setup)"

    idx16, slots, Twh, base, TT, dcol1, dcol2, dinv, bseq = \
        _preprocess(np.asarray(edge_index))
    nc = _build(TT, Twh, base)

    iota = np.broadcast_to(np.arange(128, dtype=np.float32), (128, 128)).astype(ml_dtypes.bfloat16)
    ident = np.eye(128, dtype=np.float32).astype(ml_dtypes.bfloat16)
    w1_in = np.ascontiguousarray(W1.reshape(2, 128, HID)).astype(ml_dtypes.bfloat16)
    w2_in = np.ascontiguousarray(W2.reshape(2, 128, OUTC)).astype(ml_dtypes.bfloat16)
    xda, xdb, ownx = _xd_tables(x, dinv)
    slots_bf = slots.astype(ml_dtypes.bfloat16)

    in_maps = []
    for c in range(NCORES):
        in_maps.append({
            "xda": xda, "xdb": xdb, "ownx": ownx[c],
            "w1": w1_in, "w2": w2_in, "iota": iota, "ident": ident,
            "dcol1": dcol1[c], "dcol2": dcol2[c] / XSCL,
            "idx": idx16[c], "slots": slots_bf[c], "bcnt": bseq[c],
        })

    trace = bool(int(os.environ.get("GCN_KERNEL_TRACE", "0")))
    try:
        res = run_bass_kernel_spmd(nc, in_maps, core_ids=list(range(NCORES)), trace=trace)
    except Exception:
        # rare transient NRT exec failure: retry once on a fresh dispatch
        time_mod = __import__("time"); time_mod.sleep(2.0)
        res = run_bass_kernel_spmd(nc, in_maps, core_ids=list(range(NCORES)), trace=False)
    kernel.last_results = res
    if trace:
        print(f"HW exec time: {res.exec_time_ns} ns")
        kernel.last_exec_time_ns = res.exec_time_ns

    out = np.concatenate([res.results[c]["out"][:RPC] for c in range(NCORES)], axis=0)
    return out.astype(np.float32)


# revision 38
# speedup vs baseline: 1.7784x; 1.0028x over previous
"""GCN encoder (2-layer GCNConv, PyG-style) on 8 Trainium2 NeuronCores.

Sharding: nodes row-sharded 6250/core; edges partitioned by destination-node
owner; per-core segment-sum over 128-dst-slot windows via selection-matrix
matmuls.

Aggregate-first layer 1: since segment_sum commutes with @W1, the layer-1
gather table is just 8*dinv.*x (fp8-e4m3, prepared on host, uploaded as
input; the 8x pre-scale is compensated in the relu scale) — no on-device
table build, and gathers start at t=0. Per window, after the
raw-feature aggregation:
  A[dst,256]  = sum_e xd[src_e] + xd[dst]          (S-matmuls + identity matmul)
  g~          = dinv^2 .* relu(A @ W1)             (transpose, GEMM, relu-scale)
  table2 rows = g~ @ W2                            (transpose, GEMM)
Layer 2 stays transform-first (OUTC < HID): table2 is all-gathered in 8
progressive pieces (4 per sub-table) that overlap layer-1 work.

norm = dinv[src]*dinv[dst] folding (b1 == b2 == 0):
  xd    = dinv .* x
  g~    = dinv^2 .* relu(segsum(xd[src]) @ W1) = dinv .* h
  out   = dinv .* segsum((g~ @ W2)[src])

Self-loop messages never go through the gather path: their contribution to a
window's segment-sum is the core's own xd / table2 rows, added with one
identity matmul per window from SBUF-resident copies.

Gathers are issued per (window, sub-table) on 4 rotating SWDGE queues; the
measured bottleneck is a fixed per-descriptor cadence (~60ns/desc/engine), so
smaller, more numerous gathers maximize in-flight concurrency. Edges within a
bucket are sorted by source row for HBM row-buffer locality.

Sub-tables (int16 gather-index limit 32768 rows): local row l < 2176
(windows 0-16) -> sub A; l >= 2176 (windows 17-48) -> sub B (32768 rows
exactly). B is processed first in layer 1 and maximal so the final tail
phase (sub A of layer 2) is smallest.
"""

import os
import numpy as np
import ml_dtypes

import concourse.bacc as bacc
import concourse.tile as tile
from concourse import bass, mybir
from concourse.bass_utils import run_bass_kernel_spmd
from concourse.library_config import mlp

N = 50000
INC, HID, OUTC = 256, 256, 128
NCORES = 8
RPC = N // NCORES            # 6250 rows per core
WPC = (RPC + 127) // 128     # 49 windows per core
RPAD = WPC * 128             # 6272
LSPL = 2176                  # sub-table split on local row (windows 0..16 | 17..48)
NA = NCORES * LSPL           # 17408 rows in sub-table A
NB = NCORES * (RPAD - LSPL)  # 32768 rows in sub-table B
WA = LSPL // 128             # 17 windows in A
# layer-1 window order: B-side windows first (their table2 rows feed the
# earlier AllGather pieces), w16 leads so ag2a piece 3's input is ready early
WORDER = list(range(WA - 1, WPC)) + list(range(0, WA - 1))
# AllGather piece boundaries (local rows) within each sub-table; pieces are
# triggered progressively as their windows complete. Tables are piece-major:
# row(piece p, rank r, local l) = 8*P[p] + r*(P[p+1]-P[p]) + (l-P[p]).
PA = (0, 512, 1024, 1536, 2176)       # windows 0-3 | 4-7 | 8-11 | 12-16
PB = (0, 1024, 2048, 3072, 4096)      # windows 17-24 | 25-32 | 33-40 | 41-48


def _preprocess(edge_index):
    """Edge partitioning / ordering and normalization constants (host, index-only)."""
    src = np.asarray(edge_index[0], np.int64)
    dst = np.asarray(edge_index[1], np.int64)

    # degrees include the self-loops the reference adds
    deg = (np.bincount(dst, minlength=N) + 1).astype(np.float64)
    dinv = (1.0 / np.sqrt(deg)).astype(np.float32)

    owner = dst // RPC
    dstl = dst - owner * RPC
    win = dstl >> 7
    slot = dstl & 127
    srho = src // RPC
    srl = src - srho * RPC
    sub = (srl >= LSPL).astype(np.int64)

    def _piece_gl(local, P):
        P = np.asarray(P)
        p = np.searchsorted(P, local, side="right") - 1
        return 8 * P[p] + srho * (P[p + 1] - P[p]) + (local - P[p])

    gl = np.where(sub == 0, _piece_gl(np.minimum(srl, LSPL - 1), PA),
                  _piece_gl(np.maximum(srl - LSPL, 0), PB)).astype(np.int32)
    assert gl.max() < 32768, "gather indices must fit int16"

    # sort by (bucket, src row): ascending addresses within each bucket make
    # the gather's HBM access pattern row-buffer friendly
    order = np.lexsort((gl, (owner * WPC + win) * 2 + sub))
    key_s = ((owner * WPC + win) * 2 + sub)[order]
    gl_s = gl[order]
    slot_s = slot[order].astype(np.int32)

    nbuck = NCORES * WPC * 2
    counts = np.bincount(key_s, minlength=nbuck).reshape(NCORES, WPC, 2)
    starts_flat = np.concatenate([[0], np.cumsum(counts.reshape(-1))])

    # tiles per (window, sub): max over cores so one SPMD program fits all
    Twh = (counts.max(axis=0) + 127) // 128     # [WPC, 2]
    TT = int(Twh.sum())
    # stream order: layer-1 window order -> sub -> tiles
    base = np.zeros((WPC, 2), np.int64)
    pos = 0
    for w in WORDER:
        for h in range(2):
            base[w, h] = pos
            pos += Twh[w, h]
    assert pos == TT

    # indices beyond each core's actual bucket count are -1: together with a
    # runtime per-bucket count register, the gather skips the padded tail
    # entirely (the padding is ~12% of all descriptors). Skipped message rows
    # hold stale-but-finite data and are zeroed out by the S sentinel.
    idx_seq = np.full((NCORES, TT * 128), -1, np.int32)
    slot_seq = np.full((NCORES, TT * 128), 128, np.int32)  # 128 = dropped sentinel
    bcnt = np.ones((NCORES, WPC, 2), np.int32)
    for c in range(NCORES):
        for w in range(WPC):
            for h in range(2):
                n = counts[c, w, h]
                if Twh[w, h] == 0:
                    continue
                p0 = base[w, h] * 128
                if n == 0:
                    idx_seq[c, p0] = 0  # dummy valid index, dropped by S
                    continue
                s0 = starts_flat[(c * WPC + w) * 2 + h]
                idx_seq[c, p0 : p0 + n] = gl_s[s0 : s0 + n]
                slot_seq[c, p0 : p0 + n] = slot_s[s0 : s0 + n]
                bcnt[c, w, h] = n

    # wrapped int16 gather-index layout: element j at [j%16, j//16], replicated x8
    idx16 = np.empty((NCORES, 128, TT * 8), np.int16)
    slots = np.empty((NCORES, 128, TT), np.float32)
    for c in range(NCORES):
        a = idx_seq[c].astype(np.int16).reshape(-1, 16).T
        idx16[c] = np.tile(a, (8, 1))
        slots[c] = slot_seq[c].astype(np.float32).reshape(TT, 128).T

    # per-core per-window dinv columns for own rows
    dcol1 = np.zeros((NCORES, 128, WPC), np.float32)
    for c in range(NCORES):
        d = np.zeros(RPAD, np.float32)
        d[:RPC] = dinv[c * RPC : (c + 1) * RPC]
        dcol1[c] = d.reshape(WPC, 128).T
    dcol2 = dcol1 * dcol1

    # per-core bucket counts in stream order, replicated across partitions
    # for the gpsimd count-register loads
    bseq = np.empty((NCORES, WPC * 2), np.int32)
    i = 0
    for w in WORDER:
        for h in range(2):
            bseq[:, i] = bcnt[:, w, h]
            i += 1
    bseq = np.broadcast_to(bseq[:, None, :], (NCORES, 128, WPC * 2)).copy()

    return idx16, slots, Twh, base, TT, dcol1, dcol2, dinv, bseq


XSCL = 8.0  # xd pre-scale: centers values in fp8-e4m3's normal range;
             # compensated exactly in the relu scale (dcol2 / XSCL)


def _xd_tables(x, dinv):
    """XSCL*dinv.*x rows in piece-major [A | B] order, fp8-e4m3 for the
    gather tables (|values| <= ~45, well inside e4m3's +-240 so ml_dtypes
    e4m3fn and TRN float8e4 agree bit-for-bit); own-row blocks in bf16
    partition-major [128, WPC*256] layout."""
    xd = (x * (XSCL * dinv[:, None])).astype(np.float32)
    xda = np.zeros((NA, INC), np.float32)
    xdb = np.zeros((NB, INC), np.float32)
    for rho in range(NCORES):
        xs = np.zeros((RPAD, INC), np.float32)
        xs[:RPC] = xd[rho * RPC : (rho + 1) * RPC]   # [6272, 256] padded
        for p in range(4):
            lo, hi = PA[p], PA[p + 1]
            xda[8 * lo + rho * (hi - lo) : 8 * lo + (rho + 1) * (hi - lo)] = xs[lo:hi]
            lo, hi = PB[p], PB[p + 1]
            xdb[8 * lo + rho * (hi - lo) : 8 * lo + (rho + 1) * (hi - lo)] = \
                xs[LSPL + lo : LSPL + hi]
    ownx = np.zeros((NCORES, 128, WPC, INC), np.float32)
    for c in range(NCORES):
        blk = np.zeros((RPAD, INC), np.float32)
        blk[:RPC] = xd[c * RPC : (c + 1) * RPC]
        ownx[c] = blk.reshape(WPC, 128, INC).transpose(1, 0, 2)
    return (xda.astype(ml_dtypes.float8_e4m3fn), xdb.astype(ml_dtypes.float8_e4m3fn),
            ownx.reshape(NCORES, 128, WPC * INC).astype(ml_dtypes.bfloat16))


def _build(TT, Twh, base):
    nc = bacc.Bacc("TRN2", num_devices=NCORES, num_swdge_queues=4,
                   dynamic_dma_scratch_size=32768)
    f32 = mybir.dt.float32
    bf = mybir.dt.bfloat16

    f8 = mybir.dt.float8e4
    xda_d = nc.dram_tensor("xda", [NA, INC], f8, kind="ExternalInput")
    xdb_d = nc.dram_tensor("xdb", [NB, INC], f8, kind="ExternalInput")
    ownx_d = nc.dram_tensor("ownx", [128, WPC * INC], bf, kind="ExternalInput")
    w1_d = nc.dram_tensor("w1", [2, 128, HID], bf, kind="ExternalInput")
    w2_d = nc.dram_tensor("w2", [2, 128, OUTC], bf, kind="ExternalInput")
    iota_d = nc.dram_tensor("iota", [128, 128], bf, kind="ExternalInput")
    ident_d = nc.dram_tensor("ident", [128, 128], bf, kind="ExternalInput")
    dc1_d = nc.dram_tensor("dcol1", [128, WPC], f32, kind="ExternalInput")
    dc2_d = nc.dram_tensor("dcol2", [128, WPC], f32, kind="ExternalInput")
    idx_d = nc.dram_tensor("idx", [128, TT * 8], mybir.dt.int16, kind="ExternalInput")
    slots_d = nc.dram_tensor("slots", [128, TT], bf, kind="ExternalInput")
    bcnt_d = nc.dram_tensor("bcnt", [128, WPC * 2], mybir.dt.int32,
                            kind="ExternalInput")
    out_d = nc.dram_tensor("out", [RPAD, OUTC], f32, kind="ExternalOutput")

    BKT = {}
    for _w in WORDER:
        for _h in range(2):
            BKT[(_w, _h)] = len(BKT)

    with tile.TileContext(nc) as tc:
        nc.gpsimd.load_library(mlp)
        with (
            tc.tile_pool(name="const", bufs=1) as cpool,
            tc.tile_pool(name="own", bufs=1) as opool,
            tc.tile_pool(name="evac", bufs=4) as epool,
            tc.tile_pool(name="att", bufs=4) as apool,
            tc.tile_pool(name="msg", bufs=14) as mpool,
            tc.tile_pool(name="sel", bufs=8) as spool,
            tc.tile_pool(name="part", bufs=WPC) as ppool,
            tc.tile_pool(name="p256", bufs=4, space="PSUM") as p256,
            tc.tile_pool(name="p128", bufs=3, space="PSUM") as p128,
            tc.tile_pool(name="ptr", bufs=1, space="PSUM") as ptr,
            tc.tile_pool(name="dram", bufs=1, space="DRAM") as dram,
        ):
            # ---- constants to SBUF; idx loaded in two slices so the first
            # gathers don't wait on the full 1.8MB index transfer
            w1_s = cpool.tile([128, 2, HID], bf)
            w2_s = cpool.tile([128, 2, OUTC], bf)
            iota_s = cpool.tile([128, 128], bf)
            ident_s = cpool.tile([128, 128], bf)
            dc1_s = cpool.tile([128, WPC], f32)
            dc2_s = cpool.tile([128, WPC], f32)
            idx_s = cpool.tile([128, TT * 8], mybir.dt.int16)
            slots_s = cpool.tile([128, TT], bf)
            bcnt_s = cpool.tile([128, WPC * 2], mybir.dt.int32)
            ownx_s = opool.tile([128, WPC, INC], bf)    # own xd rows per window
            own2_s = opool.tile([128, WPC, OUTC], bf)   # own table2 rows
            ISPL = min(2048, TT * 8)
            nc.sync.dma_start(bcnt_s[:], bcnt_d[:])
            nc.sync.dma_start(slots_s[:], slots_d[:])
            nc.sync.dma_start(idx_s[:, :ISPL], idx_d[:, :ISPL])
            nc.sync.dma_start(iota_s[:], iota_d[:])
            for k in range(2):
                nc.sync.dma_start(w1_s[:, k, :], w1_d[k])
                nc.sync.dma_start(w2_s[:, k, :], w2_d[k])
            nc.sync.dma_start(ident_s[:], ident_d[:])
            nc.sync.dma_start(dc1_s[:], dc1_d[:])
            nc.sync.dma_start(dc2_s[:], dc2_d[:])
            if ISPL < TT * 8:
                nc.scalar.dma_start(idx_s[:, ISPL:], idx_d[:, ISPL:])
            nc.scalar.dma_start(
                ownx_s[:], ownx_d[:].rearrange("p (w c) -> p w c", w=WPC))

            ag2a_in = dram.tile([LSPL, OUTC], bf)
            ag2b_in = dram.tile([RPAD - LSPL, OUTC], bf)
            tb2a = dram.tile([NA, OUTC], bf)
            tb2b = dram.tile([NB, OUTC], bf)

            qctr = [0]
            creg = nc.gpsimd.alloc_register("gcnt")
            creg2 = nc.gpsimd.alloc_register("gcnt2")

            # zero the message pool once: rows the trimmed gathers skip then
            # hold finite stale data, which the S sentinel zeroes exactly
            MAXT = int(Twh.max())
            for _ in range(14):
                z = mpool.tile([128, MAXT, INC], bf, tag="msg")
                nc.vector.memset(z[:], 0)

            # ---- one gather + selection-matrix build for (window, sub);
            # the count register trims the gather to this core's real edges.
            # dt is fp8 for the layer-1 xd tables (S matches so the matmul
            # runs in double-fp8 mode), bf16 for layer 2.
            def gather_win(w, h, tbl, width, dt, reg=None):
                T = int(Twh[w, h])
                if T == 0:
                    return None, None
                b = int(base[w, h])
                bkt = BKT[(w, h)]
                m_s = mpool.tile([128, T, width], dt, tag="msg")
                if reg is None:
                    reg = creg
                    nc.gpsimd.reg_load(reg, bcnt_s[0:1, bkt : bkt + 1])
                nc.gpsimd.dma_gather(
                    m_s[:], tbl[:, :], idx_s[:, b * 8 : (b + T) * 8],
                    T * 128, reg, width,
                    single_packet=False, queue_num=qctr[0] % 4)
                qctr[0] += 1
                S_s = spool.tile([128, T, 128], dt, tag="sel")
                nc.vector.tensor_tensor(
                    out=S_s[:],
                    in0=slots_s[:, b : b + T, None].to_broadcast([128, T, 128]),
                    in1=iota_s[:, None, :].to_broadcast([128, T, 128]),
                    op=mybir.AluOpType.is_equal)
                return m_s, S_s

            # ---- layer-1 window: gather both subs, aggregate raw features,
            # then W1 -> relu -> W2 to produce this window's table2 rows
            def l1_window(w):
                # both subs' counts are adjacent in bcnt: one 2-register load
                bkt = BKT[(w, 0)]
                nc.gpsimd.reg_load([creg, creg2], bcnt_s[0:1, bkt : bkt + 2])
                units = [gather_win(w, h, xda_d if h == 0 else xdb_d, INC, f8,
                                    reg=(creg if h == 0 else creg2))
                         for h in range(2)]
                ps = p256.tile([128, INC], f32, tag="p256")
                started = False
                for h in range(2):
                    m_s, S_s = units[h]
                    if m_s is None:
                        continue
                    T = int(Twh[w, h])
                    for t in range(T):
                        nc.tensor.matmul(ps[:], lhsT=S_s[:, t, :], rhs=m_s[:, t, :],
                                         start=(not started and t == 0), stop=False)
                        started = True
                nc.tensor.matmul(ps[:], lhsT=ident_s[:], rhs=ownx_s[:, w, :],
                                 start=not started, stop=True)
                a_s = epool.tile([128, INC], bf, tag="a")
                nc.scalar.activation(a_s[:], ps[:],
                                     mybir.ActivationFunctionType.Copy)
                at_s = apool.tile([128, 2, 128], bf, tag="at")
                for k in range(2):
                    pt = ptr.tile([128, 128], bf, tag="pt")
                    nc.tensor.transpose(pt[:], a_s[:, k * 128 : (k + 1) * 128],
                                        ident_s[:])
                    nc.vector.tensor_copy(at_s[:, k, :], pt[:])
                psH = p256.tile([128, HID], f32, tag="p256")
                for k in range(2):
                    nc.tensor.matmul(psH[:], lhsT=at_s[:, k, :], rhs=w1_s[:, k, :],
                                     start=(k == 0), stop=(k == 1))
                g_s = epool.tile([128, HID], bf, tag="g")
                nc.scalar.activation(g_s[:], psH[:],
                                     mybir.ActivationFunctionType.Relu,
                                     scale=dc2_s[:, w : w + 1])
                gt_s = apool.tile([128, 2, 128], bf, tag="at")
                for k in range(2):
                    pt = ptr.tile([128, 128], bf, tag="pt")
                    nc.tensor.transpose(pt[:], g_s[:, k * 128 : (k + 1) * 128],
                                        ident_s[:])
                    nc.vector.tensor_copy(gt_s[:, k, :], pt[:])
                ps2 = p128.tile([128, OUTC], f32, tag="p128")
                for k in range(2):
                    nc.tensor.matmul(ps2[:], lhsT=gt_s[:, k, :], rhs=w2_s[:, k, :],
                                     start=(k == 0), stop=(k == 1))
                nc.vector.tensor_copy(own2_s[:, w, :], ps2[:])
                if w < WA:
                    nc.sync.dma_start(ag2a_in[w * 128 : (w + 1) * 128, :],
                                      own2_s[:, w, :])
                else:
                    lw = w - WA
                    nc.sync.dma_start(ag2b_in[lw * 128 : (lw + 1) * 128, :],
                                      own2_s[:, w, :])

            partials = {}

            # ---- layer-2 stage 1: self + sub-B messages -> partial
            def p6b_window(w):
                m_s, S_s = gather_win(w, 1, tb2b, OUTC, bf)
                ps = p128.tile([128, OUTC], f32, tag="p128")
                started = False
                if m_s is not None:
                    for t in range(int(Twh[w, 1])):
                        nc.tensor.matmul(ps[:], lhsT=S_s[:, t, :], rhs=m_s[:, t, :],
                                         start=(t == 0), stop=False)
                    started = True
                nc.tensor.matmul(ps[:], lhsT=ident_s[:], rhs=own2_s[:, w, :],
                                 start=not started, stop=True)
                pp = ppool.tile([128, OUTC], bf, tag="partial")
                nc.scalar.activation(pp[:], ps[:],
                                     mybir.ActivationFunctionType.Copy)
                partials[w] = pp

            # ---- layer-2 stage 2: partial + sub-A messages -> output
            def p6a_window(w):
                m_s, S_s = gather_win(w, 0, tb2a, OUTC, bf)
                ps = p128.tile([128, OUTC], f32, tag="p128")
                nc.tensor.matmul(ps[:], lhsT=ident_s[:], rhs=partials[w][:],
                                 start=True, stop=m_s is None)
                if m_s is not None:
                    T = int(Twh[w, 0])
                    for t in range(T):
                        nc.tensor.matmul(ps[:], lhsT=S_s[:, t, :], rhs=m_s[:, t, :],
                                         start=False, stop=(t == T - 1))
                o_s = epool.tile([128, OUTC], f32, tag="o")
                nc.scalar.activation(o_s[:], ps[:],
                                     mybir.ActivationFunctionType.Copy,
                                     scale=dc1_s[:, w : w + 1])
                nc.sync.dma_start(out_d[w * 128 : (w + 1) * 128, :], o_s[:])

            def ag_piece(which, p):
                lo, hi = (PA[p], PA[p + 1]) if which == "a" else (PB[p], PB[p + 1])
                inp = ag2a_in if which == "a" else ag2b_in
                outp = tb2a if which == "a" else tb2b
                with nc.named_scope(f"ag2{which}{p}"):
                    nc.gpsimd.collective_compute(
                        "AllGather", mybir.AluOpType.bypass,
                        replica_groups=[list(range(NCORES))],
                        ins=[inp[lo:hi, :].opt()],
                        outs=[outp[8 * lo : 8 * hi, :].opt()])

            # AG piece triggers are delayed ~4 windows past the window that
            # completes their input so their dependency wait is already
            # satisfied and never stalls the in-order gather stream.
            with nc.named_scope("p3_l1"):
                for w in WORDER:
                    l1_window(w)
                    if w == 28:
                        ag_piece("b", 0)      # windows 17-24, done at w24
                    elif w == 36:
                        ag_piece("b", 1)      # windows 25-32
                    elif w == 44:
                        ag_piece("b", 2)      # windows 33-40
                    elif w == 2:
                        ag_piece("b", 3)      # windows 41-48, done at B end
                    elif w == 7:
                        ag_piece("a", 0)      # windows 0-3
                    elif w == 11:
                        ag_piece("a", 1)      # windows 4-7
                    elif w == 15:
                        ag_piece("a", 2)      # windows 8-11
            with nc.named_scope("p6_b"):
                for n, w in enumerate(WORDER):
                    p6b_window(w)
                    if n == 3:
                        ag_piece("a", 3)      # windows 12-16, done at A end
            with nc.named_scope("p6_a"):
                for w in WORDER:
                    p6a_window(w)

    nc.compile()
    return nc


def kernel(x, edge_index, W1, b1, W2, b2):
    x = np.asarray(x, np.float32)
    W1 = np.asarray(W1, np.float32)
    W2 = np.asarray(W2, np.float32)
    assert not np.any(np.asarray(b1)) and not np.any(np.asarray(b2)), \
        "kernel assumes zero biases (as in the reference setup)"

    idx16, slots, Twh, base, TT, dcol1, dcol2, dinv, bseq = \
        _preprocess(np.asarray(edge_index))
    nc = _build(TT, Twh, base)

    iota = np.broadcast_to(np.arange(128, dtype=np.float32), (128, 128)).astype(ml_dtypes.bfloat16)
    ident = np.eye(128, dtype=np.float32).astype(ml_dtypes.bfloat16)
    w1_in = np.ascontiguousarray(W1.reshape(2, 128, HID)).astype(ml_dtypes.bfloat16)
    w2_in = np.ascontiguousarray(W2.reshape(2, 128, OUTC)).astype(ml_dtypes.bfloat16)
    xda, xdb, ownx = _xd_tables(x, dinv)
    slots_bf = slots.astype(ml_dtypes.bfloat16)

    in_maps = []
    for c in range(NCORES):
        in_maps.append({
            "xda": xda, "xdb": xdb, "ownx": ownx[c],
            "w1": w1_in, "w2": w2_in, "iota": iota, "ident": ident,
            "dcol1": dcol1[c], "dcol2": dcol2[c] / XSCL,
            "idx": idx16[c], "slots": slots_bf[c], "bcnt": bseq[c],
        })

    trace = bool(int(os.environ.get("GCN_KERNEL_TRACE", "0")))
    try:
        res = run_bass_kernel_spmd(nc, in_maps, core_ids=list(range(NCORES)), trace=trace)
    except Exception:
        # rare transient NRT exec failure: retry once on a fresh dispatch
        time_mod = __import__("time"); time_mod.sleep(2.0)
        res = run_bass_kernel_spmd(nc, in_maps, core_ids=list(range(NCORES)), trace=False)
    kernel.last_results = res
    if trace:
        print(f"HW exec time: {res.exec_time_ns} ns")
        kernel.last_exec_time_ns = res.exec_time_ns

    out = np.concatenate([res.results[c]["out"][:RPC] for c in range(NCORES)], axis=0)
    return out.astype(np.float32)
